# revision 11
# baseline (speedup 1.0000x reference)
"""VDP (variance-propagating) attention kernel for Trainium2, 8 NeuronCores.

Sharding: core c -> (batch b = c//2, head-group g = c%2) [8 heads each].
Each core computes LN + its QKV slice + attention for its 8 heads + the
partial out-projection for its 512 inner columns. Host sums the two
head-group partials per batch. No collectives needed.

Layout trick: everything on-device lives transposed as [feature, token]
(activations) / [contraction, out] (weights), prepared host-side, so the
contraction dim is always on partitions and no on-device transposes are
needed anywhere. LayerNorm stats (reduce over features = partitions) are
done with ones-vector matmuls on the PE; softmax denominators come for
free from a ones-augmented column in the V operand of the mu-attention AV
matmul, and are broadcast back across partitions with a K=1 PE matmul.
"""

import os
import sys

import numpy as np

for _p in ("/opt/trn_rl_repo", "/root/.axon_site/_ro/trn_rl_repo"):
    if os.path.isdir(_p) and _p not in sys.path:
        sys.path.insert(0, _p)

HEADS = 16
DH = 64
SCALE = DH ** -0.5
EPS = 1e-5
B, N, D = 4, 1024, 1024
HPC = 8          # heads per core
RQK = 1024       # q+k rows per core (2 * 8 heads * 64)
RV = 512         # v rows per core
P = 128

_NC_CACHE = {}


def _build_nc():
    import concourse.bass as bass  # noqa: F401
    import concourse.tile as tile
    from concourse import bacc, mybir

    f32 = mybir.dt.float32
    AF = mybir.ActivationFunctionType
    ALU = mybir.AluOpType

    nc = bacc.Bacc(None, target_bir_lowering=False)

    io = {}
    for name, shape in [
        ("muT", [D, N]), ("sgT", [D, N]), ("gb", [P, 16]),
        ("wqk_mu", [D, RQK]), ("wqk_sr", [D, RQK]),
        ("wv_mu", [D, RV]), ("wv_sr", [D, RV]),
        ("wo_mu", [RV, D]), ("wo_sr", [RV, D]),
    ]:
        io[name] = nc.dram_tensor(name, shape, f32, kind="ExternalInput")
    for name, shape in [("yT_mu", [D, N]), ("yT_sg", [D, N])]:
        io[name] = nc.dram_tensor(name, shape, f32, kind="ExternalOutput")
    # internal DRAM staging
    io["qkT_mu"] = nc.dram_tensor("qkT_mu", [RQK, N], f32)
    io["qkT_sg"] = nc.dram_tensor("qkT_sg", [RQK, N], f32)
    io["vg_mu"] = nc.dram_tensor("vg_mu", [N, HPC * 65], f32)   # 65-stride: 64 v cols + ones col per head
    io["vg_sg"] = nc.dram_tensor("vg_sg", [N, RV], f32)
    io["oT_mu"] = nc.dram_tensor("oT_mu", [RV, N], f32)
    io["oT_sg"] = nc.dram_tensor("oT_sg", [RV, N], f32)

    with tile.TileContext(nc) as tc:
        _emit(nc, tc, io, f32, AF, ALU)
    nc.compile()
    return nc


def _emit(nc, tc, io, f32, AF, ALU):
    from contextlib import ExitStack

    # ============ Phase A: LayerNorm + QKV projection ============
    with ExitStack() as actx:
        acts = actx.enter_context(tc.tile_pool(name="acts", bufs=1))
        smallA = actx.enter_context(tc.tile_pool(name="smallA", bufs=1))

        gb_sb = smallA.tile([P, 16], f32)
        nc.sync.dma_start(out=gb_sb, in_=io["gb"][:])
        g2_sb = smallA.tile([P, 8], f32)
        nc.vector.tensor_mul(g2_sb, gb_sb[:, 0:8], gb_sb[:, 0:8])
        ones_col = smallA.tile([P, 1], f32)
        nc.vector.memset(ones_col, 1.0)
        ones_row = smallA.tile([1, P], f32)
        nc.vector.memset(ones_row, 1.0)
        eps1 = smallA.tile([1, 1], f32)
        nc.vector.memset(eps1, EPS)

        inv_b = acts.tile([P, N], f32)
        minv_b = acts.tile([P, N], f32)
        inv2_b = acts.tile([P, N], f32)
        mu_nT = acts.tile([P, 8, N], f32)
        sg_nT = acts.tile([P, 8, N], f32)
        a2T = acts.tile([P, 8, N], f32)

        # --- A1: stats + normalize ---
        with ExitStack() as ctx:
            muP = ctx.enter_context(tc.tile_pool(name="muP", bufs=1))
            ioA = ctx.enter_context(tc.tile_pool(name="ioA", bufs=2))
            psS = ctx.enter_context(tc.tile_pool(name="psS", bufs=1, space="PSUM"))
            psA = ctx.enter_context(tc.tile_pool(name="psA", bufs=2, space="PSUM"))

            muT_sb = muP.tile([P, 8, N], f32)
            for j in range(8):
                nc.sync.dma_start(out=muT_sb[:, j, :], in_=io["muT"][j * P:(j + 1) * P, :])

            sum_ps = [psS.tile([1, 512], f32, tag=f"sum{c}", name=f"sum{c}") for c in range(2)]
            sq_ps = [psS.tile([1, 512], f32, tag=f"sq{c}", name=f"sq{c}") for c in range(2)]
            for j in range(8):
                mu2 = ioA.tile([P, N], f32, tag="mu2")
                nc.vector.tensor_mul(mu2, muT_sb[:, j, :], muT_sb[:, j, :])
                for c in range(2):
                    cs = slice(c * 512, (c + 1) * 512)
                    nc.tensor.matmul(sum_ps[c], ones_col, muT_sb[:, j, cs],
                                     start=(j == 0), stop=(j == 7), skip_group_check=True)
                    nc.tensor.matmul(sq_ps[c], ones_col, mu2[:, cs],
                                     start=(j == 0), stop=(j == 7), skip_group_check=True)

            inv_sb = smallA.tile([1, N], f32)
            minv_sb = smallA.tile([1, N], f32)
            for c in range(2):
                cs = slice(c * 512, (c + 1) * 512)
                mean_t = ioA.tile([1, 512], f32, tag="mean")
                nc.vector.tensor_scalar_mul(mean_t, sum_ps[c], 1.0 / D)
                m2_t = ioA.tile([1, 512], f32, tag="m2")
                nc.vector.tensor_mul(m2_t, mean_t, mean_t)
                var_t = ioA.tile([1, 512], f32, tag="var")
                nc.vector.scalar_tensor_tensor(var_t, sq_ps[c], 1.0 / D, m2_t,
                                               ALU.mult, ALU.subtract)
                std_t = ioA.tile([1, 512], f32, tag="std")
                nc.scalar.activation(std_t, var_t, AF.Sqrt, bias=eps1)
                nc.vector.reciprocal(inv_sb[:, cs], std_t)
                nc.vector.scalar_tensor_tensor(minv_sb[:, cs], mean_t, -1.0, inv_sb[:, cs],
                                               ALU.mult, ALU.mult)

            for c in range(2):
                cs = slice(c * 512, (c + 1) * 512)
                bp1 = psA.tile([P, 512], f32, tag="bcast")
                nc.tensor.matmul(bp1, ones_row, inv_sb[:, cs], start=True, stop=True)
                nc.vector.tensor_copy(inv_b[:, cs], bp1)
                bp2 = psA.tile([P, 512], f32, tag="bcast")
                nc.tensor.matmul(bp2, ones_row, minv_sb[:, cs], start=True, stop=True)
                nc.vector.tensor_copy(minv_b[:, cs], bp2)
            nc.vector.tensor_mul(inv2_b, inv_b, inv_b)

            for j in range(8):
                x2 = ioA.tile([P, N], f32, tag="x2")
                nc.vector.tensor_mul(x2, muT_sb[:, j, :], inv_b)
                nc.vector.tensor_add(x2, x2, minv_b)
                nc.vector.tensor_scalar(mu_nT[:, j, :], x2, gb_sb[:, j:j + 1],
                                        gb_sb[:, 8 + j:9 + j], ALU.mult, ALU.add)
                sgt = ioA.tile([P, N], f32, tag="sgt")
                nc.sync.dma_start(out=sgt, in_=io["sgT"][j * P:(j + 1) * P, :])
                nc.vector.scalar_tensor_tensor(sg_nT[:, j, :], sgt, g2_sb[:, j:j + 1],
                                               inv2_b, ALU.mult, ALU.mult)
                z = ioA.tile([P, N], f32, tag="z")
                nc.scalar.activation(z, mu_nT[:, j, :], AF.Square)
                nc.vector.tensor_add(a2T[:, j, :], z, sg_nT[:, j, :])

        # --- A2a: QKV q,k rows (transposed out) ---
        with ExitStack() as ctx:
            wq = ctx.enter_context(tc.tile_pool(name="wq", bufs=2))
            evA = ctx.enter_context(tc.tile_pool(name="evA", bufs=3))
            psQ = ctx.enter_context(tc.tile_pool(name="psQ", bufs=2, space="PSUM"))
            for rb in range(8):
                rsl = slice(rb * P, (rb + 1) * P)
                wmu = wq.tile([P, 8, P], f32, tag="wmu")
                nc.sync.dma_start(out=wmu, in_=io["wqk_mu"][:, rsl].rearrange("(j p) r -> p j r", p=P))
                wsr = wq.tile([P, 8, P], f32, tag="wsr")
                nc.sync.dma_start(out=wsr, in_=io["wqk_sr"][:, rsl].rearrange("(j p) r -> p j r", p=P))
                wsig = wq.tile([P, 8, P], f32, tag="wsig")
                nc.scalar.activation(wsig, wsr, AF.Exp)
                nc.scalar.activation(wsig, wsig, AF.Ln, bias=1.0)
                wmu2 = wq.tile([P, 8, P], f32, tag="wmu2")
                nc.vector.tensor_mul(wmu2, wmu, wmu)
                for c in range(2):
                    cs = slice(c * 512, (c + 1) * 512)
                    ps_mu = psQ.tile([P, 512], f32, tag="qkmu")
                    for j in range(8):
                        nc.tensor.matmul(ps_mu, wmu[:, j, :], mu_nT[:, j, cs],
                                         start=(j == 0), stop=(j == 7))
                    ev1 = evA.tile([P, 512], f32, tag="ev1")
                    nc.vector.tensor_copy(ev1, ps_mu)
                    nc.sync.dma_start(out=io["qkT_mu"][rsl, cs], in_=ev1)
                    ps_sg = psQ.tile([P, 512], f32, tag="qksg")
                    for j in range(8):
                        nc.tensor.matmul(ps_sg, wsig[:, j, :], a2T[:, j, cs],
                                         start=(j == 0), stop=False)
                    for j in range(8):
                        nc.tensor.matmul(ps_sg, wmu2[:, j, :], sg_nT[:, j, cs],
                                         start=False, stop=(j == 7))
                    ev2 = evA.tile([P, 512], f32, tag="ev2")
                    nc.scalar.copy(ev2, ps_sg)
                    nc.sync.dma_start(out=io["qkT_sg"][rsl, cs], in_=ev2)

        # --- A2b: V (natural layout out) ---
        with ExitStack() as ctx:
            wv = ctx.enter_context(tc.tile_pool(name="wv", bufs=1))
            evV = ctx.enter_context(tc.tile_pool(name="evV", bufs=3))
            psV = ctx.enter_context(tc.tile_pool(name="psV", bufs=2, space="PSUM"))
            wv_mu = wv.tile([P, 8, 512], f32)
            nc.sync.dma_start(out=wv_mu, in_=io["wv_mu"][:].rearrange("(j p) r -> p j r", p=P))
            wv_sr = wv.tile([P, 8, 512], f32)
            nc.sync.dma_start(out=wv_sr, in_=io["wv_sr"][:].rearrange("(j p) r -> p j r", p=P))
            wv_sig = wv.tile([P, 8, 512], f32)
            nc.scalar.activation(wv_sig, wv_sr, AF.Exp)
            nc.scalar.activation(wv_sig, wv_sig, AF.Ln, bias=1.0)
            wv_mu2 = wv.tile([P, 8, 512], f32)
            nc.vector.tensor_mul(wv_mu2, wv_mu, wv_mu)
            for tb in range(8):
                tsl = slice(tb * P, (tb + 1) * P)
                ps_mu = psV.tile([P, 512], f32, tag="vmu")
                for j in range(8):
                    nc.tensor.matmul(ps_mu, mu_nT[:, j, tsl], wv_mu[:, j, :],
                                     start=(j == 0), stop=(j == 7))
                evv = evV.tile([P, HPC * 65], f32, tag="evv")
                nc.vector.memset(evv, 1.0)
                nc.vector.tensor_copy(
                    evv.rearrange("p (h c) -> p h c", c=65)[:, :, 0:64],
                    ps_mu.rearrange("p (h c) -> p h c", c=64))
                nc.sync.dma_start(out=io["vg_mu"][tsl, :], in_=evv)
                ps_sg = psV.tile([P, 512], f32, tag="vsg")
                for j in range(8):
                    nc.tensor.matmul(ps_sg, a2T[:, j, tsl], wv_sig[:, j, :],
                                     start=(j == 0), stop=False)
                for j in range(8):
                    nc.tensor.matmul(ps_sg, sg_nT[:, j, tsl], wv_mu2[:, j, :],
                                     start=False, stop=(j == 7))
                ev3 = evV.tile([P, 512], f32, tag="ev3")
                nc.scalar.copy(ev3, ps_sg)
                nc.sync.dma_start(out=io["vg_sg"][tsl, :], in_=ev3)

    # ============ Phase B: per-head attention ============
    with ExitStack() as ctx:
        pin = ctx.enter_context(tc.tile_pool(name="pin", bufs=2))
        ep = ctx.enter_context(tc.tile_pool(name="ep", bufs=18))
        sb3 = ctx.enter_context(tc.tile_pool(name="sb3", bufs=4))
        outsb = ctx.enter_context(tc.tile_pool(name="outsb", bufs=4))
        smallB = ctx.enter_context(tc.tile_pool(name="smallB", bufs=4))
        onesB = ctx.enter_context(tc.tile_pool(name="onesB", bufs=1))
        psD = ctx.enter_context(tc.tile_pool(name="psD", bufs=2, space="PSUM"))
        psS2 = ctx.enter_context(tc.tile_pool(name="psS2", bufs=2, space="PSUM"))
        psAVm = ctx.enter_context(tc.tile_pool(name="psAVm", bufs=1, space="PSUM"))
        psAVs = ctx.enter_context(tc.tile_pool(name="psAVs", bufs=1, space="PSUM"))
        psDB = ctx.enter_context(tc.tile_pool(name="psDB", bufs=1, space="PSUM"))

        ones_blk = onesB.tile([P, P], f32)
        nc.vector.memset(ones_blk, 1.0)
        sc128 = onesB.tile([P, 1], f32)
        nc.vector.memset(sc128, SCALE)

        for pr in range(4):
            v_mu = pin.tile([P, 8, 130], f32, tag="v_mu")
            nc.sync.dma_start(out=v_mu,
                              in_=io["vg_mu"][:, pr * 130:(pr + 1) * 130].rearrange("(j p) c -> p j c", p=P))
            v_sg = pin.tile([P, 8, P], f32, tag="v_sg")
            nc.sync.dma_start(out=v_sg,
                              in_=io["vg_sg"][:, pr * P:(pr + 1) * P].rearrange("(j p) c -> p j c", p=P))
            for hh in range(2):
                hq = 2 * pr + hh  # core-local head index
                q_mu = pin.tile([64, N], f32, tag="q_mu")
                nc.sync.dma_start(out=q_mu, in_=io["qkT_mu"][hq * 64:(hq + 1) * 64, :])
                q_sg = pin.tile([64, N], f32, tag="q_sg")
                nc.sync.dma_start(out=q_sg, in_=io["qkT_sg"][hq * 64:(hq + 1) * 64, :])
                k_mu = pin.tile([64, N], f32, tag="k_mu")
                nc.sync.dma_start(out=k_mu, in_=io["qkT_mu"][512 + hq * 64:512 + (hq + 1) * 64, :])
                k_sg = pin.tile([64, N], f32, tag="k_sg")
                nc.sync.dma_start(out=k_sg, in_=io["qkT_sg"][512 + hq * 64:512 + (hq + 1) * 64, :])
                for c in range(2):
                    cs = slice(c * 512, (c + 1) * 512)
                    # pass 1: mu scores -> exp -> mu AV (ones-augmented for denom)
                    av_mu = psAVm.tile([65, 512], f32, tag="avmu")
                    e_ts = []
                    for kb in range(8):
                        dots = psD.tile([P, 512], f32, tag="dots")
                        nc.tensor.matmul(dots, k_mu[:, kb * P:(kb + 1) * P], q_mu[:, cs],
                                         start=True, stop=True)
                        e_t = ep.tile([P, 512], f32, tag="e")
                        nc.scalar.activation(e_t, dots, AF.Exp, scale=sc128)
                        e_ts.append(e_t)
                        nc.tensor.matmul(av_mu, v_mu[:, kb, hh * 65:(hh + 1) * 65], e_t,
                                         start=(kb == 0), stop=(kb == 7))
                    r_sb = smallB.tile([P, 512], f32, tag="r")
                    nc.vector.reciprocal(r_sb[64:65, :], av_mu[64:65, :])
                    dbp = psDB.tile([P, 512], f32, tag="db")
                    nc.tensor.matmul(dbp, ones_blk[64:65, :], r_sb[64:65, :], start=True, stop=True)
                    db = sb3.tile([P, 512], f32, tag="db_sb")
                    nc.vector.tensor_copy(db, dbp)
                    muo = outsb.tile([64, 512], f32, tag="muo")
                    nc.vector.tensor_mul(muo, av_mu[0:64, :], db[0:64, :])
                    nc.sync.dma_start(out=io["oT_mu"][hq * 64:(hq + 1) * 64, cs], in_=muo)
                    # pass 2: sigma scores -> J^2*sigma -> sigma AV
                    av_sg = psAVs.tile([64, 512], f32, tag="avsg")
                    for kb in range(8):
                        sdots = psS2.tile([P, 512], f32, tag="sdots")
                        nc.tensor.matmul(sdots, k_sg[:, kb * P:(kb + 1) * P], q_sg[:, cs],
                                         start=True, stop=True)
                        p_t = sb3.tile([P, 512], f32, tag="p")
                        nc.gpsimd.tensor_mul(p_t, e_ts[kb], db)
                        t_t = sb3.tile([P, 512], f32, tag="t")
                        nc.vector.scalar_tensor_tensor(t_t, p_t, 1.0, p_t,
                                                       ALU.subtract, ALU.mult)
                        u_t = sb3.tile([P, 512], f32, tag="u")
                        nc.scalar.activation(u_t, t_t, AF.Square)
                        w_t = sb3.tile([P, 512], f32, tag="w")
                        nc.vector.scalar_tensor_tensor(w_t, u_t, SCALE, sdots,
                                                       ALU.mult, ALU.mult)
                        nc.tensor.matmul(av_sg, v_sg[:, kb, hh * 64:(hh + 1) * 64], w_t,
                                         start=(kb == 0), stop=(kb == 7))
                    sgo = outsb.tile([64, 512], f32, tag="sgo")
                    nc.vector.tensor_copy(sgo, av_sg)
                    nc.sync.dma_start(out=io["oT_sg"][hq * 64:(hq + 1) * 64, cs], in_=sgo)

    # ============ Phase C: out-projection (partial over this core's 512 cols) ============
    with ExitStack() as ctx:
        wo = ctx.enter_context(tc.tile_pool(name="wo", bufs=1))
        oin = ctx.enter_context(tc.tile_pool(name="oin", bufs=1))
        evC = ctx.enter_context(tc.tile_pool(name="evC", bufs=4))
        psC = ctx.enter_context(tc.tile_pool(name="psC", bufs=2, space="PSUM"))

        wo_mu = wo.tile([P, 4, D], f32)
        nc.sync.dma_start(out=wo_mu, in_=io["wo_mu"][:].rearrange("(j p) o -> p j o", p=P))
        wo_sr = wo.tile([P, 4, D], f32)
        nc.sync.dma_start(out=wo_sr, in_=io["wo_sr"][:].rearrange("(j p) o -> p j o", p=P))
        wo_sig = wo.tile([P, 4, D], f32)
        nc.scalar.activation(wo_sig, wo_sr, AF.Exp)
        nc.scalar.activation(wo_sig, wo_sig, AF.Ln, bias=1.0)
        wo_mu2 = wo.tile([P, 4, D], f32)
        nc.vector.tensor_mul(wo_mu2, wo_mu, wo_mu)

        o_mu = oin.tile([P, 4, N], f32)
        nc.sync.dma_start(out=o_mu, in_=io["oT_mu"][:].rearrange("(j p) t -> p j t", p=P))
        o_sg = oin.tile([P, 4, N], f32)
        nc.sync.dma_start(out=o_sg, in_=io["oT_sg"][:].rearrange("(j p) t -> p j t", p=P))
        a2o = oin.tile([P, 4, N], f32)
        nc.scalar.activation(a2o, o_mu, AF.Square)
        nc.vector.tensor_add(a2o, a2o, o_sg)

        for ob in range(8):
            osl = slice(ob * P, (ob + 1) * P)
            for c in range(2):
                cs = slice(c * 512, (c + 1) * 512)
                ps_mu = psC.tile([P, 512], f32, tag="ymu")
                for j in range(4):
                    nc.tensor.matmul(ps_mu, wo_mu[:, j, osl], o_mu[:, j, cs],
                                     start=(j == 0), stop=(j == 3))
                ev1 = evC.tile([P, 512], f32, tag="ev1")
                nc.vector.tensor_copy(ev1, ps_mu)
                nc.sync.dma_start(out=io["yT_mu"][osl, cs], in_=ev1)
                ps_sg = psC.tile([P, 512], f32, tag="ysg")
                for j in range(4):
                    nc.tensor.matmul(ps_sg, wo_sig[:, j, osl], a2o[:, j, cs],
                                     start=(j == 0), stop=False)
                for j in range(4):
                    nc.tensor.matmul(ps_sg, wo_mu2[:, j, osl], o_sg[:, j, cs],
                                     start=False, stop=(j == 3))
                ev2 = evC.tile([P, 512], f32, tag="ev2")
                nc.scalar.copy(ev2, ps_sg)
                nc.sync.dma_start(out=io["yT_sg"][osl, cs], in_=ev2)


def _get_nc():
    if "nc" not in _NC_CACHE:
        _NC_CACHE["nc"] = _build_nc()
    return _NC_CACHE["nc"]


def _prep_core_inputs(c, mu, sigma, ln_gamma, ln_beta, Wqkv_mu, Wqkv_sigma_raw,
                      Wout_mu, Wout_sigma_raw):
    f = np.float32
    asc = np.ascontiguousarray
    b, g = divmod(c, 2)
    qs = slice(512 * g, 512 * (g + 1))
    ks = slice(1024 + 512 * g, 1024 + 512 * (g + 1))
    vs = slice(2048 + 512 * g, 2048 + 512 * (g + 1))
    gb = np.zeros((P, 16), f)
    gb[:, :8] = np.asarray(ln_gamma, f).reshape(8, P).T
    gb[:, 8:] = np.asarray(ln_beta, f).reshape(8, P).T
    wqk_mu = np.concatenate([Wqkv_mu[qs], Wqkv_mu[ks]], 0)
    wqk_sr = np.concatenate([Wqkv_sigma_raw[qs], Wqkv_sigma_raw[ks]], 0)
    return {
        "muT": asc(np.asarray(mu[b], f).T),
        "sgT": asc(np.asarray(sigma[b], f).T),
        "gb": gb,
        "wqk_mu": asc(np.asarray(wqk_mu, f).T),
        "wqk_sr": asc(np.asarray(wqk_sr, f).T),
        "wv_mu": asc(np.asarray(Wqkv_mu[vs], f).T),
        "wv_sr": asc(np.asarray(Wqkv_sigma_raw[vs], f).T),
        "wo_mu": asc(np.asarray(Wout_mu[:, 512 * g:512 * (g + 1)], f).T),
        "wo_sr": asc(np.asarray(Wout_sigma_raw[:, 512 * g:512 * (g + 1)], f).T),
    }


def _emulate_core(m):
    """Pure-numpy mirror of the on-device program (for validation only)."""
    sp = lambda x: np.log1p(np.exp(x))
    muT, sgT = m["muT"], m["sgT"]
    gamma = m["gb"][:, :8].T.reshape(-1)[:, None]   # [D,1] indexed by d
    beta = m["gb"][:, 8:].T.reshape(-1)[:, None]
    mean = muT.mean(0, keepdims=True)
    var = muT.var(0, keepdims=True)
    inv = 1.0 / np.sqrt(var + EPS)
    mu_nT = (muT * inv - mean * inv) * gamma + beta
    sg_nT = sgT * gamma * gamma * inv * inv
    a2T = mu_nT * mu_nT + sg_nT
    qkT_mu = m["wqk_mu"].T @ mu_nT
    qkT_sg = sp(m["wqk_sr"]).T @ a2T + (m["wqk_mu"] ** 2).T @ sg_nT
    v_mu = mu_nT.T @ m["wv_mu"]
    v_sg = a2T.T @ sp(m["wv_sr"]) + sg_nT.T @ m["wv_mu"] ** 2
    oT_mu = np.zeros((RV, N), np.float32)
    oT_sg = np.zeros((RV, N), np.float32)
    for h in range(HPC):
        hs = slice(h * 64, (h + 1) * 64)
        sT = m_kT = qkT_mu[512 + h * 64:512 + (h + 1) * 64].T @ qkT_mu[hs]  # [kt, qt]
        e = np.exp(SCALE * sT)
        den = e.sum(0, keepdims=True)
        db = 1.0 / den
        p = e * db
        oT_mu[hs] = (v_mu[:, hs].T @ e) * db
        sdT = qkT_sg[512 + h * 64:512 + (h + 1) * 64].T @ qkT_sg[hs]
        t = (p - 1.0) * p
        w = (t * t) * SCALE * sdT
        oT_sg[hs] = v_sg[:, hs].T @ w
    a2o = oT_mu * oT_mu + oT_sg
    yT_mu = m["wo_mu"].T @ oT_mu
    yT_sg = sp(m["wo_sr"]).T @ a2o + (m["wo_mu"] ** 2).T @ oT_sg
    return yT_mu.astype(np.float32), yT_sg.astype(np.float32)


def kernel(mu, sigma, ln_gamma, ln_beta, Wqkv_mu, Wqkv_sigma_raw, Wout_mu,
           Wout_sigma_raw, _trace=False):
    from concourse.bass_utils import run_bass_kernel_spmd

    nc = _get_nc()
    args = (mu, sigma, ln_gamma, ln_beta, Wqkv_mu, Wqkv_sigma_raw, Wout_mu,
            Wout_sigma_raw)
    in_maps = [_prep_core_inputs(c, *args) for c in range(8)]
    res = run_bass_kernel_spmd(nc, in_maps, list(range(8)), trace=_trace)
    out_mu = np.zeros((B, N, D), np.float32)
    out_sg = np.zeros((B, N, D), np.float32)
    for c in range(8):
        b = c // 2
        out_mu[b] += res.results[c]["yT_mu"].T
        out_sg[b] += res.results[c]["yT_sg"].T
    if _trace:
        kernel._last_result = res
    return out_mu, out_sg


# revision 22
# speedup vs baseline: 198.5056x; 198.5056x over previous
"""VDP (variance-propagating) attention kernel for Trainium2, 8 NeuronCores.

Sharding: core c -> (batch b = c//2, head-group g = c%2) [8 heads each].
Each core computes LN + its QKV slice + attention for its 8 heads + the
partial out-projection for its 512 inner columns. Host sums the two
head-group partials per batch. No collectives needed.

Layout trick: everything on-device lives transposed as [feature, token]
(activations) / [contraction, out] (weights), prepared host-side, so the
contraction dim is always on partitions and no on-device transposes are
needed anywhere. LayerNorm stats (reduce over features = partitions) are
done with ones-vector matmuls on the PE; softmax denominators come for
free from a ones-augmented column in the V operand of the mu-attention AV
matmul, and are broadcast back across partitions with a K=1 PE matmul.
"""

import os
import sys

import numpy as np

for _p in ("/opt/trn_rl_repo", "/root/.axon_site/_ro/trn_rl_repo"):
    if os.path.isdir(_p) and _p not in sys.path:
        sys.path.insert(0, _p)

HEADS = 16
DH = 64
SCALE = DH ** -0.5
EPS = 1e-5
B, N, D = 4, 1024, 1024
HPC = 8          # heads per core
RQK = 1024       # q+k rows per core (2 * 8 heads * 64)
RV = 512         # v rows per core
P = 128

_NC_CACHE = {}


def _build_nc(tiny_out=False):
    import concourse.bass as bass  # noqa: F401
    import concourse.tile as tile
    from concourse import bacc, mybir

    f32 = mybir.dt.float32
    AF = mybir.ActivationFunctionType
    ALU = mybir.AluOpType

    nc = bacc.Bacc(None, target_bir_lowering=False)

    io = {}
    for name, shape in [
        ("muT", [D, N]), ("sgT", [D, N]), ("gb", [P, 16]),
        ("wqk_mu", [D, RQK]), ("wqk_sr", [D, RQK]),
        ("wv_mu", [D, RV]), ("wv_sr", [D, RV]),
        ("wo_mu", [RV, D]), ("wo_sr", [RV, D]),
    ]:
        io[name] = nc.dram_tensor(name, shape, f32, kind="ExternalInput")
    if tiny_out:
        for name, shape in [("yT_mu", [D, N]), ("yT_sg", [D, N])]:
            io[name] = nc.dram_tensor(name, shape, f32)
        io["done"] = nc.dram_tensor("done", [1, 16], f32, kind="ExternalOutput")
    else:
        for name, shape in [("yT_mu", [D, N]), ("yT_sg", [D, N])]:
            io[name] = nc.dram_tensor(name, shape, f32, kind="ExternalOutput")
    # internal DRAM staging
    bf = mybir.dt.bfloat16

    with tile.TileContext(nc) as tc:
        _emit(nc, tc, io, f32, bf, AF, ALU)
        if tiny_out:
            with tc.tile_pool(name="doneP", bufs=1) as dp:
                dt = dp.tile([1, 16], f32)
                nc.vector.memset(dt, 1.0)
                nc.sync.dma_start(out=io["done"][:], in_=dt)
    nc.compile()
    return nc


def _build_floor_nc():
    import concourse.tile as tile
    from concourse import bacc, mybir

    f32 = mybir.dt.float32
    nc = bacc.Bacc(None, target_bir_lowering=False)
    done = nc.dram_tensor("done", [1, 16], f32, kind="ExternalOutput")
    with tile.TileContext(nc) as tc:
        with tc.tile_pool(name="dp", bufs=1) as dp:
            dt = dp.tile([1, 16], f32)
            nc.vector.memset(dt, 1.0)
            nc.sync.dma_start(out=done[:], in_=dt)
    nc.compile()
    return nc


def _emit(nc, tc, io, f32, bf, AF, ALU):
    from contextlib import ExitStack

    with ExitStack() as tctx:
        stage = tctx.enter_context(tc.tile_pool(name="stage", bufs=1))
        # persistent SBUF staging (bf16): no DRAM round trips between phases
        qk_mu_sb = stage.tile([P, 8, N], bf)    # rows: 0-3 q-blocks, 4-7 k-blocks
        qk_sg_sb = stage.tile([P, 8, N], bf)
        v_mu_sb = stage.tile([P, 8, HPC * 65], bf)   # per tok-block: 8 heads x (64 v + ones)
        v_sg_sb = stage.tile([P, 8, RV], bf)
        oT_mu_sb = stage.tile([P, 4, N], bf)
        oT_sg_sb = stage.tile([P, 4, N], bf)

        # ============ Phase A: LayerNorm + QKV ============
        with ExitStack() as actx:
            acts = actx.enter_context(tc.tile_pool(name="acts", bufs=1))
            smallA = actx.enter_context(tc.tile_pool(name="smallA", bufs=1))

            gb_sb = smallA.tile([P, 16], f32)
            nc.sync.dma_start(out=gb_sb, in_=io["gb"][:])
            g2_sb = smallA.tile([P, 8], f32)
            nc.vector.tensor_mul(g2_sb, gb_sb[:, 0:8], gb_sb[:, 0:8])
            ones_col = smallA.tile([P, 1], f32)
            nc.vector.memset(ones_col, 1.0)
            ones_row = smallA.tile([1, P], f32)
            nc.vector.memset(ones_row, 1.0)
            eps1 = smallA.tile([1, 1], f32)
            nc.vector.memset(eps1, EPS)
            scA = smallA.tile([P, 1], f32)
            nc.vector.memset(scA, SCALE)

            inv_b = acts.tile([P, N], f32)
            minv_b = acts.tile([P, N], f32)
            inv2_b = acts.tile([P, N], f32)
            mu_nT = acts.tile([P, 8, N], bf)
            sg_nT = acts.tile([P, 8, N], bf)
            a2T = acts.tile([P, 8, N], bf)

            # --- A1: stats + normalize (muT streamed twice, not resident) ---
            with ExitStack() as ctx:
                ioA = ctx.enter_context(tc.tile_pool(name="ioA", bufs=2))
                psS = ctx.enter_context(tc.tile_pool(name="psS", bufs=1, space="PSUM"))
                psA = ctx.enter_context(tc.tile_pool(name="psA", bufs=2, space="PSUM"))

                sum_ps = [psS.tile([1, 512], f32, tag=f"sum{c}", name=f"sum{c}") for c in range(2)]
                sq_ps = [psS.tile([1, 512], f32, tag=f"sq{c}", name=f"sq{c}") for c in range(2)]
                for j in range(8):
                    mut = ioA.tile([P, N], f32, tag="mut")
                    nc.sync.dma_start(out=mut, in_=io["muT"][j * P:(j + 1) * P, :])
                    mu2 = ioA.tile([P, N], f32, tag="mu2")
                    nc.scalar.activation(mu2, mut, AF.Square)
                    for c in range(2):
                        cs = slice(c * 512, (c + 1) * 512)
                        nc.tensor.matmul(sum_ps[c], ones_col, mut[:, cs],
                                         start=(j == 0), stop=(j == 7), skip_group_check=True)
                        nc.tensor.matmul(sq_ps[c], ones_col, mu2[:, cs],
                                         start=(j == 0), stop=(j == 7), skip_group_check=True)

                inv_sb = smallA.tile([1, N], f32)
                minv_sb = smallA.tile([1, N], f32)
                for c in range(2):
                    cs = slice(c * 512, (c + 1) * 512)
                    mean_t = ioA.tile([1, 512], f32, tag="mean")
                    nc.vector.tensor_scalar_mul(mean_t, sum_ps[c], 1.0 / D)
                    m2_t = ioA.tile([1, 512], f32, tag="m2")
                    nc.vector.tensor_mul(m2_t, mean_t, mean_t)
                    var_t = ioA.tile([1, 512], f32, tag="var")
                    nc.vector.scalar_tensor_tensor(var_t, sq_ps[c], 1.0 / D, m2_t,
                                                   ALU.mult, ALU.subtract)
                    std_t = ioA.tile([1, 512], f32, tag="std")
                    nc.scalar.activation(std_t, var_t, AF.Sqrt, bias=eps1)
                    nc.vector.reciprocal(inv_sb[:, cs], std_t)
                    nc.vector.scalar_tensor_tensor(minv_sb[:, cs], mean_t, -1.0, inv_sb[:, cs],
                                                   ALU.mult, ALU.mult)

                for c in range(2):
                    cs = slice(c * 512, (c + 1) * 512)
                    bp1 = psA.tile([P, 512], f32, tag="bcast")
                    nc.tensor.matmul(bp1, ones_row, inv_sb[:, cs], start=True, stop=True)
                    nc.vector.tensor_copy(inv_b[:, cs], bp1)
                    bp2 = psA.tile([P, 512], f32, tag="bcast")
                    nc.tensor.matmul(bp2, ones_row, minv_sb[:, cs], start=True, stop=True)
                    nc.vector.tensor_copy(minv_b[:, cs], bp2)
                nc.vector.tensor_mul(inv2_b, inv_b, inv_b)

                for j in range(8):
                    mut = ioA.tile([P, N], f32, tag="mut")
                    nc.sync.dma_start(out=mut, in_=io["muT"][j * P:(j + 1) * P, :])
                    x2 = ioA.tile([P, N], f32, tag="x2")
                    nc.vector.tensor_mul(x2, mut, inv_b)
                    nc.vector.tensor_add(x2, x2, minv_b)
                    nc.vector.tensor_scalar(mu_nT[:, j, :], x2, gb_sb[:, j:j + 1],
                                            gb_sb[:, 8 + j:9 + j], ALU.mult, ALU.add)
                    sgt = ioA.tile([P, N], f32, tag="sgt")
                    nc.sync.dma_start(out=sgt, in_=io["sgT"][j * P:(j + 1) * P, :])
                    nc.vector.scalar_tensor_tensor(sg_nT[:, j, :], sgt, g2_sb[:, j:j + 1],
                                                   inv2_b, ALU.mult, ALU.mult)
                    z = ioA.tile([P, N], f32, tag="z")
                    nc.scalar.activation(z, mu_nT[:, j, :], AF.Square)
                    nc.gpsimd.tensor_add(a2T[:, j, :], z, sg_nT[:, j, :])

            # --- A2a: QKV q,k rows (transposed out), evict straight to SBUF stage ---
            with ExitStack() as ctx:
                wq = ctx.enter_context(tc.tile_pool(name="wq", bufs=2))
                psQ = ctx.enter_context(tc.tile_pool(name="psQ", bufs=2, space="PSUM"))
                for rb in range(8):
                    rsl = slice(rb * P, (rb + 1) * P)
                    wmu = wq.tile([P, 8, P], bf, tag="wmu")
                    nc.gpsimd.dma_start(out=wmu, in_=io["wqk_mu"][:, rsl].rearrange("(j p) r -> p j r", p=P))
                    wsr = wq.tile([P, 8, P], f32, tag="wsr")
                    nc.sync.dma_start(out=wsr, in_=io["wqk_sr"][:, rsl].rearrange("(j p) r -> p j r", p=P))
                    wsg0 = wq.tile([P, 8, P], f32, tag="wsg0")
                    nc.scalar.activation(wsg0, wsr, AF.Exp)
                    wsig = wq.tile([P, 8, P], bf, tag="wsig")
                    nc.scalar.activation(wsig, wsg0, AF.Ln, bias=1.0)
                    wmu2 = wq.tile([P, 8, P], bf, tag="wmu2")
                    nc.vector.tensor_mul(wmu2, wmu, wmu)
                    for c in range(2):
                        cs = slice(c * 512, (c + 1) * 512)
                        ps_mu = psQ.tile([P, 512], f32, tag="qkmu")
                        for j in range(8):
                            nc.tensor.matmul(ps_mu, wmu[:, j, :], mu_nT[:, j, cs],
                                             start=(j == 0), stop=(j == 7))
                        nc.vector.tensor_copy(qk_mu_sb[:, rb, cs], ps_mu)
                        ps_sg = psQ.tile([P, 512], f32, tag="qksg")
                        for j in range(8):
                            nc.tensor.matmul(ps_sg, wsig[:, j, :], a2T[:, j, cs],
                                             start=(j == 0), stop=False)
                        for j in range(8):
                            nc.tensor.matmul(ps_sg, wmu2[:, j, :], sg_nT[:, j, cs],
                                             start=False, stop=(j == 7))
                        if rb < 4:
                            nc.scalar.activation(qk_sg_sb[:, rb, cs], ps_sg, AF.Copy, scale=scA)
                        else:
                            nc.scalar.copy(qk_sg_sb[:, rb, cs], ps_sg)

            # --- A2b: V (natural layout), evict straight to SBUF stage ---
            with ExitStack() as ctx:
                wv = ctx.enter_context(tc.tile_pool(name="wv", bufs=1))
                psV = ctx.enter_context(tc.tile_pool(name="psV", bufs=2, space="PSUM"))
                wv_mu = wv.tile([P, 8, 512], bf)
                nc.gpsimd.dma_start(out=wv_mu, in_=io["wv_mu"][:].rearrange("(j p) r -> p j r", p=P))
                wv_sr = wv.tile([P, 8, 512], f32)
                nc.sync.dma_start(out=wv_sr, in_=io["wv_sr"][:].rearrange("(j p) r -> p j r", p=P))
                wv_sg0 = wv.tile([P, 8, 512], f32)
                nc.scalar.activation(wv_sg0, wv_sr, AF.Exp)
                wv_sig = wv.tile([P, 8, 512], bf)
                nc.scalar.activation(wv_sig, wv_sg0, AF.Ln, bias=1.0)
                wv_mu2 = wv.tile([P, 8, 512], bf)
                nc.vector.tensor_mul(wv_mu2, wv_mu, wv_mu)
                for tb in range(8):
                    tsl = slice(tb * P, (tb + 1) * P)
                    ps_mu = psV.tile([P, 512], f32, tag="vmu")
                    for j in range(8):
                        nc.tensor.matmul(ps_mu, mu_nT[:, j, tsl], wv_mu[:, j, :],
                                         start=(j == 0), stop=(j == 7))
                    nc.vector.memset(v_mu_sb[:, tb, :], 1.0)
                    nc.vector.tensor_copy(
                        v_mu_sb[:, tb, :].rearrange("p (h c) -> p h c", c=65)[:, :, 0:64],
                        ps_mu.rearrange("p (h c) -> p h c", c=64))
                    ps_sg = psV.tile([P, 512], f32, tag="vsg")
                    for j in range(8):
                        nc.tensor.matmul(ps_sg, a2T[:, j, tsl], wv_sig[:, j, :],
                                         start=(j == 0), stop=False)
                    for j in range(8):
                        nc.tensor.matmul(ps_sg, sg_nT[:, j, tsl], wv_mu2[:, j, :],
                                         start=False, stop=(j == 7))
                    nc.scalar.copy(v_sg_sb[:, tb, :], ps_sg)

        # ============ Phase B: attention (all operands already in SBUF) ============
        with ExitStack() as ctx:
            ep = ctx.enter_context(tc.tile_pool(name="ep", bufs=20))
            sb3 = ctx.enter_context(tc.tile_pool(name="sb3", bufs=4))
            outsb = ctx.enter_context(tc.tile_pool(name="outsb", bufs=4))
            smallB = ctx.enter_context(tc.tile_pool(name="smallB", bufs=4))
            onesB = ctx.enter_context(tc.tile_pool(name="onesB", bufs=1))
            psD = ctx.enter_context(tc.tile_pool(name="psD", bufs=2, space="PSUM"))
            psS2 = ctx.enter_context(tc.tile_pool(name="psS2", bufs=2, space="PSUM"))
            psAVm = ctx.enter_context(tc.tile_pool(name="psAVm", bufs=2, space="PSUM"))
            psAVs = ctx.enter_context(tc.tile_pool(name="psAVs", bufs=1, space="PSUM"))
            psDB = ctx.enter_context(tc.tile_pool(name="psDB", bufs=1, space="PSUM"))

            ones_blk = onesB.tile([P, P], f32)
            nc.vector.memset(ones_blk, 1.0)
            sc128 = onesB.tile([P, 1], f32)
            nc.vector.memset(sc128, SCALE)

            def pass1(hq, c):
                pr, hh = divmod(hq, 2)
                pb = (hq % 2) * 64
                qrb, krb = hq // 2, 4 + hq // 2
                vco = pr * 130 + hh * 65
                cs = slice(c * 512, (c + 1) * 512)
                av_mu = psAVm.tile([65, 512], f32, tag="avmu", name=f"avmu{hq}_{c}")
                e_ts = []
                for kb in range(8):
                    dots = psD.tile([P, 512], f32, tag="dots", name=f"dots{hq}_{c}_{kb}")
                    nc.tensor.matmul(dots,
                                     qk_mu_sb[pb:pb + 64, krb, kb * P:(kb + 1) * P],
                                     qk_mu_sb[pb:pb + 64, qrb, cs],
                                     start=True, stop=True)
                    e_t = ep.tile([P, 512], bf, tag="e", name=f"e{hq}_{c}_{kb}")
                    nc.scalar.activation(e_t, dots, AF.Exp, scale=sc128)
                    e_ts.append(e_t)
                    nc.tensor.matmul(av_mu, v_mu_sb[:, kb, vco:vco + 65], e_t,
                                     start=(kb == 0), stop=(kb == 7))
                r_sb = smallB.tile([P, 512], f32, tag="r", name=f"r{hq}_{c}")
                nc.vector.reciprocal(r_sb[64:65, :], av_mu[64:65, :])
                dbp = psDB.tile([P, 512], f32, tag="db", name=f"dbp{hq}_{c}")
                nc.tensor.matmul(dbp, ones_blk[64:65, :], r_sb[64:65, :], start=True, stop=True)
                db = sb3.tile([P, 512], f32, tag="db_sb", name=f"db{hq}_{c}")
                nc.vector.tensor_copy(db, dbp)
                muo = outsb.tile([64, 512], bf, tag="muo", name=f"muo{hq}_{c}")
                nc.vector.tensor_mul(muo, av_mu[0:64, :], db[0:64, :])
                nc.sync.dma_start(out=oT_mu_sb[pb:pb + 64, qrb, cs], in_=muo)
                return (hq, c, e_ts, db)

            def pass2(state):
                hq, c, e_ts, db = state
                pr, hh = divmod(hq, 2)
                pb = (hq % 2) * 64
                qrb, krb = hq // 2, 4 + hq // 2
                cs = slice(c * 512, (c + 1) * 512)
                av_sg = psAVs.tile([64, 512], f32, tag="avsg", name=f"avsg{hq}_{c}")
                for kb in range(8):
                    sdots = psS2.tile([P, 512], f32, tag="sdots", name=f"sd{hq}_{c}_{kb}")
                    nc.tensor.matmul(sdots,
                                     qk_sg_sb[pb:pb + 64, krb, kb * P:(kb + 1) * P],
                                     qk_sg_sb[pb:pb + 64, qrb, cs],
                                     start=True, stop=True)
                    p_t = sb3.tile([P, 512], f32, tag="p", name=f"p{hq}_{c}_{kb}")
                    nc.gpsimd.tensor_mul(p_t, e_ts[kb], db)
                    t_t = sb3.tile([P, 512], f32, tag="t", name=f"t{hq}_{c}_{kb}")
                    if kb % 2 == 0:
                        nc.vector.scalar_tensor_tensor(t_t, p_t, 1.0, p_t,
                                                       ALU.subtract, ALU.mult)
                    else:
                        m_t = sb3.tile([P, 512], f32, tag="m", name=f"m{hq}_{c}_{kb}")
                        nc.gpsimd.tensor_mul(m_t, p_t, p_t)
                        nc.gpsimd.tensor_sub(t_t, p_t, m_t)
                    u_t = sb3.tile([P, 512], f32, tag="u", name=f"u{hq}_{c}_{kb}")
                    nc.scalar.activation(u_t, t_t, AF.Square)
                    w_t = sb3.tile([P, 512], bf, tag="w", name=f"w{hq}_{c}_{kb}")
                    nc.vector.tensor_mul(w_t, u_t, sdots)
                    nc.tensor.matmul(av_sg, v_sg_sb[:, kb, hq * 64:(hq + 1) * 64], w_t,
                                     start=(kb == 0), stop=(kb == 7))
                sgo = outsb.tile([64, 512], bf, tag="sgo", name=f"sgo{hq}_{c}")
                nc.vector.tensor_copy(sgo, av_sg)
                nc.sync.dma_start(out=oT_sg_sb[pb:pb + 64, qrb, cs], in_=sgo)

            prev = None
            for hq in range(HPC):
                for c in range(2):
                    cur = pass1(hq, c)
                    if prev is not None:
                        pass2(prev)
                    prev = cur
            pass2(prev)

        # ============ Phase C: out-projection ============
        with ExitStack() as ctx:
            wo = ctx.enter_context(tc.tile_pool(name="wo", bufs=1))
            oin = ctx.enter_context(tc.tile_pool(name="oin", bufs=1))
            evC = ctx.enter_context(tc.tile_pool(name="evC", bufs=4))
            psC = ctx.enter_context(tc.tile_pool(name="psC", bufs=2, space="PSUM"))

            wo_mu = wo.tile([P, 4, D], bf)
            nc.gpsimd.dma_start(out=wo_mu, in_=io["wo_mu"][:].rearrange("(j p) o -> p j o", p=P))
            wo_sr = wo.tile([P, 4, D], f32)
            nc.sync.dma_start(out=wo_sr, in_=io["wo_sr"][:].rearrange("(j p) o -> p j o", p=P))
            wo_sg0 = wo.tile([P, 4, D], f32)
            nc.scalar.activation(wo_sg0, wo_sr, AF.Exp)
            wo_sig = wo.tile([P, 4, D], bf)
            nc.scalar.activation(wo_sig, wo_sg0, AF.Ln, bias=1.0)
            wo_mu2 = wo.tile([P, 4, D], bf)
            nc.vector.tensor_mul(wo_mu2, wo_mu, wo_mu)

            a2o = oin.tile([P, 4, N], bf)
            zsq = oin.tile([P, 4, N], f32)
            nc.scalar.activation(zsq, oT_mu_sb, AF.Square)
            nc.vector.tensor_add(a2o, zsq, oT_sg_sb)

            for ob in range(8):
                osl = slice(ob * P, (ob + 1) * P)
                for c in range(2):
                    cs = slice(c * 512, (c + 1) * 512)
                    ps_mu = psC.tile([P, 512], f32, tag="ymu")
                    for j in range(4):
                        nc.tensor.matmul(ps_mu, wo_mu[:, j, osl], oT_mu_sb[:, j, cs],
                                         start=(j == 0), stop=(j == 3))
                    ev1 = evC.tile([P, 512], f32, tag="ev1")
                    nc.vector.tensor_copy(ev1, ps_mu)
                    nc.sync.dma_start(out=io["yT_mu"][osl, cs], in_=ev1)
                    ps_sg = psC.tile([P, 512], f32, tag="ysg")
                    for j in range(4):
                        nc.tensor.matmul(ps_sg, wo_sig[:, j, osl], a2o[:, j, cs],
                                         start=(j == 0), stop=False)
                    for j in range(4):
                        nc.tensor.matmul(ps_sg, wo_mu2[:, j, osl], oT_sg_sb[:, j, cs],
                                         start=False, stop=(j == 3))
                    ev2 = evC.tile([P, 512], f32, tag="ev2")
                    nc.scalar.copy(ev2, ps_sg)
                    nc.sync.dma_start(out=io["yT_sg"][osl, cs], in_=ev2)


def _get_nc():
    if "nc" not in _NC_CACHE:
        _NC_CACHE["nc"] = _build_nc()
    return _NC_CACHE["nc"]


def _prep_core_inputs(c, mu, sigma, ln_gamma, ln_beta, Wqkv_mu, Wqkv_sigma_raw,
                      Wout_mu, Wout_sigma_raw):
    f = np.float32
    asc = np.ascontiguousarray
    b, g = divmod(c, 2)
    qs = slice(512 * g, 512 * (g + 1))
    ks = slice(1024 + 512 * g, 1024 + 512 * (g + 1))
    vs = slice(2048 + 512 * g, 2048 + 512 * (g + 1))
    gb = np.zeros((P, 16), f)
    gb[:, :8] = np.asarray(ln_gamma, f).reshape(8, P).T
    gb[:, 8:] = np.asarray(ln_beta, f).reshape(8, P).T
    wqk_mu = np.concatenate([Wqkv_mu[qs], Wqkv_mu[ks]], 0)
    wqk_sr = np.concatenate([Wqkv_sigma_raw[qs], Wqkv_sigma_raw[ks]], 0)
    return {
        "muT": asc(np.asarray(mu[b], f).T),
        "sgT": asc(np.asarray(sigma[b], f).T),
        "gb": gb,
        "wqk_mu": asc(np.asarray(wqk_mu, f).T),
        "wqk_sr": asc(np.asarray(wqk_sr, f).T),
        "wv_mu": asc(np.asarray(Wqkv_mu[vs], f).T),
        "wv_sr": asc(np.asarray(Wqkv_sigma_raw[vs], f).T),
        "wo_mu": asc(np.asarray(Wout_mu[:, 512 * g:512 * (g + 1)], f).T),
        "wo_sr": asc(np.asarray(Wout_sigma_raw[:, 512 * g:512 * (g + 1)], f).T),
    }


def _emulate_core(m):
    """Pure-numpy mirror of the on-device program (for validation only)."""
    sp = lambda x: np.log1p(np.exp(x))
    muT, sgT = m["muT"], m["sgT"]
    gamma = m["gb"][:, :8].T.reshape(-1)[:, None]   # [D,1] indexed by d
    beta = m["gb"][:, 8:].T.reshape(-1)[:, None]
    mean = muT.mean(0, keepdims=True)
    var = muT.var(0, keepdims=True)
    inv = 1.0 / np.sqrt(var + EPS)
    mu_nT = (muT * inv - mean * inv) * gamma + beta
    sg_nT = sgT * gamma * gamma * inv * inv
    a2T = mu_nT * mu_nT + sg_nT
    qkT_mu = m["wqk_mu"].T @ mu_nT
    qkT_sg = sp(m["wqk_sr"]).T @ a2T + (m["wqk_mu"] ** 2).T @ sg_nT
    v_mu = mu_nT.T @ m["wv_mu"]
    v_sg = a2T.T @ sp(m["wv_sr"]) + sg_nT.T @ m["wv_mu"] ** 2
    oT_mu = np.zeros((RV, N), np.float32)
    oT_sg = np.zeros((RV, N), np.float32)
    for h in range(HPC):
        hs = slice(h * 64, (h + 1) * 64)
        sT = m_kT = qkT_mu[512 + h * 64:512 + (h + 1) * 64].T @ qkT_mu[hs]  # [kt, qt]
        e = np.exp(SCALE * sT)
        den = e.sum(0, keepdims=True)
        db = 1.0 / den
        p = e * db
        oT_mu[hs] = (v_mu[:, hs].T @ e) * db
        sdT = qkT_sg[512 + h * 64:512 + (h + 1) * 64].T @ qkT_sg[hs]
        t = (p - 1.0) * p
        w = (t * t) * SCALE * sdT
        oT_sg[hs] = v_sg[:, hs].T @ w
    a2o = oT_mu * oT_mu + oT_sg
    yT_mu = m["wo_mu"].T @ oT_mu
    yT_sg = sp(m["wo_sr"]).T @ a2o + (m["wo_mu"] ** 2).T @ oT_sg
    return yT_mu.astype(np.float32), yT_sg.astype(np.float32)


def kernel(mu, sigma, ln_gamma, ln_beta, Wqkv_mu, Wqkv_sigma_raw, Wout_mu,
           Wout_sigma_raw, _trace=False):
    from concourse.bass_utils import run_bass_kernel_spmd

    nc = _get_nc()
    args = (mu, sigma, ln_gamma, ln_beta, Wqkv_mu, Wqkv_sigma_raw, Wout_mu,
            Wout_sigma_raw)
    in_maps = [_prep_core_inputs(c, *args) for c in range(8)]
    res = run_bass_kernel_spmd(nc, in_maps, list(range(8)), trace=_trace)
    out_mu = np.zeros((B, N, D), np.float32)
    out_sg = np.zeros((B, N, D), np.float32)
    for c in range(8):
        b = c // 2
        out_mu[b] += res.results[c]["yT_mu"].T
        out_sg[b] += res.results[c]["yT_sg"].T
    if _trace:
        kernel._last_result = res
    return out_mu, out_sg


# revision 24
# speedup vs baseline: 199.0814x; 1.0029x over previous
"""VDP (variance-propagating) attention kernel for Trainium2, 8 NeuronCores.

Sharding: core c -> (batch b = c//2, head-group g = c%2) [8 heads each].
Each core computes LN + its QKV slice + attention for its 8 heads + the
partial out-projection for its 512 inner columns. Host sums the two
head-group partials per batch. No collectives needed.

Layout trick: everything on-device lives transposed as [feature, token]
(activations) / [contraction, out] (weights), prepared host-side, so the
contraction dim is always on partitions and no on-device transposes are
needed anywhere. LayerNorm stats (reduce over features = partitions) are
done with ones-vector matmuls on the PE; softmax denominators come for
free from a ones-augmented column in the V operand of the mu-attention AV
matmul, and are broadcast back across partitions with a K=1 PE matmul.
"""

import os
import sys

import numpy as np

for _p in ("/opt/trn_rl_repo", "/root/.axon_site/_ro/trn_rl_repo"):
    if os.path.isdir(_p) and _p not in sys.path:
        sys.path.insert(0, _p)

HEADS = 16
DH = 64
SCALE = DH ** -0.5
EPS = 1e-5
B, N, D = 4, 1024, 1024
HPC = 8          # heads per core
RQK = 1024       # q+k rows per core (2 * 8 heads * 64)
RV = 512         # v rows per core
P = 128

_NC_CACHE = {}


def _build_nc(tiny_out=False):
    import concourse.bass as bass  # noqa: F401
    import concourse.tile as tile
    from concourse import bacc, mybir

    f32 = mybir.dt.float32
    AF = mybir.ActivationFunctionType
    ALU = mybir.AluOpType

    nc = bacc.Bacc(None, target_bir_lowering=False)

    io = {}
    for name, shape in [
        ("muT", [D, N]), ("sgT", [D, N]), ("gb", [P, 16]),
        ("wqk_mu", [D, RQK]), ("wqk_sr", [D, RQK]),
        ("wv_mu", [D, RV]), ("wv_sr", [D, RV]),
        ("wo_mu", [RV, D]), ("wo_sr", [RV, D]),
    ]:
        io[name] = nc.dram_tensor(name, shape, f32, kind="ExternalInput")
    if tiny_out:
        for name, shape in [("yT_mu", [D, N]), ("yT_sg", [D, N])]:
            io[name] = nc.dram_tensor(name, shape, f32)
        io["done"] = nc.dram_tensor("done", [1, 16], f32, kind="ExternalOutput")
    else:
        for name, shape in [("yT_mu", [D, N]), ("yT_sg", [D, N])]:
            io[name] = nc.dram_tensor(name, shape, f32, kind="ExternalOutput")
    # internal DRAM staging
    bf = mybir.dt.bfloat16

    with tile.TileContext(nc) as tc:
        _emit(nc, tc, io, f32, bf, AF, ALU)
        if tiny_out:
            with tc.tile_pool(name="doneP", bufs=1) as dp:
                dt = dp.tile([1, 16], f32)
                nc.vector.memset(dt, 1.0)
                nc.sync.dma_start(out=io["done"][:], in_=dt)
    nc.compile()
    return nc


def _build_floor_nc():
    import concourse.tile as tile
    from concourse import bacc, mybir

    f32 = mybir.dt.float32
    nc = bacc.Bacc(None, target_bir_lowering=False)
    done = nc.dram_tensor("done", [1, 16], f32, kind="ExternalOutput")
    with tile.TileContext(nc) as tc:
        with tc.tile_pool(name="dp", bufs=1) as dp:
            dt = dp.tile([1, 16], f32)
            nc.vector.memset(dt, 1.0)
            nc.sync.dma_start(out=done[:], in_=dt)
    nc.compile()
    return nc


def _emit(nc, tc, io, f32, bf, AF, ALU):
    from contextlib import ExitStack

    with ExitStack() as tctx:
        stage = tctx.enter_context(tc.tile_pool(name="stage", bufs=1))
        # persistent SBUF staging (bf16): no DRAM round trips between phases
        qk_mu_sb = stage.tile([P, 8, N], bf)    # rows: 0-3 q-blocks, 4-7 k-blocks
        qk_sg_sb = stage.tile([P, 8, N], bf)
        v_mu_sb = stage.tile([P, 8, HPC * 65], bf)   # per tok-block: 8 heads x (64 v + ones)
        v_sg_sb = stage.tile([P, 8, RV], bf)
        oT_mu_sb = stage.tile([P, 4, N], bf)
        oT_sg_sb = stage.tile([P, 4, N], bf)

        # ============ Phase A: LayerNorm + QKV ============
        with ExitStack() as actx:
            acts = actx.enter_context(tc.tile_pool(name="acts", bufs=1))
            smallA = actx.enter_context(tc.tile_pool(name="smallA", bufs=1))

            gb_sb = smallA.tile([P, 16], f32)
            nc.sync.dma_start(out=gb_sb, in_=io["gb"][:])
            g2_sb = smallA.tile([P, 8], f32)
            nc.vector.tensor_mul(g2_sb, gb_sb[:, 0:8], gb_sb[:, 0:8])
            ones_col = smallA.tile([P, 1], f32)
            nc.vector.memset(ones_col, 1.0)
            ones_row = smallA.tile([1, P], f32)
            nc.vector.memset(ones_row, 1.0)
            eps1 = smallA.tile([1, 1], f32)
            nc.vector.memset(eps1, EPS)
            scA = smallA.tile([P, 1], f32)
            nc.vector.memset(scA, SCALE)

            inv_b = acts.tile([P, N], f32)
            minv_b = acts.tile([P, N], f32)
            inv2_b = acts.tile([P, N], f32)
            mu_nT = acts.tile([P, 8, N], bf)
            sg_nT = acts.tile([P, 8, N], bf)
            a2T = acts.tile([P, 8, N], bf)

            # --- A1: stats + normalize (muT streamed twice, not resident) ---
            with ExitStack() as ctx:
                ioA = ctx.enter_context(tc.tile_pool(name="ioA", bufs=2))
                psS = ctx.enter_context(tc.tile_pool(name="psS", bufs=1, space="PSUM"))
                psA = ctx.enter_context(tc.tile_pool(name="psA", bufs=2, space="PSUM"))

                sum_ps = [psS.tile([1, 512], f32, tag=f"sum{c}", name=f"sum{c}") for c in range(2)]
                sq_ps = [psS.tile([1, 512], f32, tag=f"sq{c}", name=f"sq{c}") for c in range(2)]
                for j in range(8):
                    mut = ioA.tile([P, N], f32, tag="mut")
                    nc.sync.dma_start(out=mut, in_=io["muT"][j * P:(j + 1) * P, :])
                    mu2 = ioA.tile([P, N], f32, tag="mu2")
                    nc.scalar.activation(mu2, mut, AF.Square)
                    for c in range(2):
                        cs = slice(c * 512, (c + 1) * 512)
                        nc.tensor.matmul(sum_ps[c], ones_col, mut[:, cs],
                                         start=(j == 0), stop=(j == 7), skip_group_check=True)
                        nc.tensor.matmul(sq_ps[c], ones_col, mu2[:, cs],
                                         start=(j == 0), stop=(j == 7), skip_group_check=True)

                inv_sb = smallA.tile([1, N], f32)
                minv_sb = smallA.tile([1, N], f32)
                for c in range(2):
                    cs = slice(c * 512, (c + 1) * 512)
                    mean_t = ioA.tile([1, 512], f32, tag="mean")
                    nc.vector.tensor_scalar_mul(mean_t, sum_ps[c], 1.0 / D)
                    m2_t = ioA.tile([1, 512], f32, tag="m2")
                    nc.vector.tensor_mul(m2_t, mean_t, mean_t)
                    var_t = ioA.tile([1, 512], f32, tag="var")
                    nc.vector.scalar_tensor_tensor(var_t, sq_ps[c], 1.0 / D, m2_t,
                                                   ALU.mult, ALU.subtract)
                    std_t = ioA.tile([1, 512], f32, tag="std")
                    nc.scalar.activation(std_t, var_t, AF.Sqrt, bias=eps1)
                    nc.vector.reciprocal(inv_sb[:, cs], std_t)
                    nc.vector.scalar_tensor_tensor(minv_sb[:, cs], mean_t, -1.0, inv_sb[:, cs],
                                                   ALU.mult, ALU.mult)

                for c in range(2):
                    cs = slice(c * 512, (c + 1) * 512)
                    bp1 = psA.tile([P, 512], f32, tag="bcast")
                    nc.tensor.matmul(bp1, ones_row, inv_sb[:, cs], start=True, stop=True)
                    nc.vector.tensor_copy(inv_b[:, cs], bp1)
                    bp2 = psA.tile([P, 512], f32, tag="bcast")
                    nc.tensor.matmul(bp2, ones_row, minv_sb[:, cs], start=True, stop=True)
                    nc.vector.tensor_copy(minv_b[:, cs], bp2)
                nc.vector.tensor_mul(inv2_b, inv_b, inv_b)

                for j in range(8):
                    mut = ioA.tile([P, N], f32, tag="mut")
                    nc.sync.dma_start(out=mut, in_=io["muT"][j * P:(j + 1) * P, :])
                    x2 = ioA.tile([P, N], f32, tag="x2")
                    nc.vector.tensor_mul(x2, mut, inv_b)
                    nc.vector.tensor_add(x2, x2, minv_b)
                    nc.vector.tensor_scalar(mu_nT[:, j, :], x2, gb_sb[:, j:j + 1],
                                            gb_sb[:, 8 + j:9 + j], ALU.mult, ALU.add)
                    sgt = ioA.tile([P, N], f32, tag="sgt")
                    nc.sync.dma_start(out=sgt, in_=io["sgT"][j * P:(j + 1) * P, :])
                    nc.vector.scalar_tensor_tensor(sg_nT[:, j, :], sgt, g2_sb[:, j:j + 1],
                                                   inv2_b, ALU.mult, ALU.mult)
                    z = ioA.tile([P, N], f32, tag="z")
                    nc.scalar.activation(z, mu_nT[:, j, :], AF.Square)
                    nc.gpsimd.tensor_add(a2T[:, j, :], z, sg_nT[:, j, :])

            # --- A2a: QKV q,k rows (transposed out), evict straight to SBUF stage ---
            with ExitStack() as ctx:
                wq = ctx.enter_context(tc.tile_pool(name="wq", bufs=2))
                psQ = ctx.enter_context(tc.tile_pool(name="psQ", bufs=2, space="PSUM"))
                for rb in range(8):
                    rsl = slice(rb * P, (rb + 1) * P)
                    wmu = wq.tile([P, 8, P], bf, tag="wmu")
                    nc.gpsimd.dma_start(out=wmu, in_=io["wqk_mu"][:, rsl].rearrange("(j p) r -> p j r", p=P))
                    wsr = wq.tile([P, 8, P], f32, tag="wsr")
                    nc.sync.dma_start(out=wsr, in_=io["wqk_sr"][:, rsl].rearrange("(j p) r -> p j r", p=P))
                    wsg0 = wq.tile([P, 8, P], f32, tag="wsg0")
                    nc.scalar.activation(wsg0, wsr, AF.Exp)
                    wsig = wq.tile([P, 8, P], bf, tag="wsig")
                    nc.scalar.activation(wsig, wsg0, AF.Ln, bias=1.0)
                    wmu2 = wq.tile([P, 8, P], bf, tag="wmu2")
                    nc.vector.tensor_mul(wmu2, wmu, wmu)
                    for c in range(2):
                        cs = slice(c * 512, (c + 1) * 512)
                        ps_mu = psQ.tile([P, 512], f32, tag="qkmu")
                        for j in range(8):
                            nc.tensor.matmul(ps_mu, wmu[:, j, :], mu_nT[:, j, cs],
                                             start=(j == 0), stop=(j == 7))
                        nc.vector.tensor_copy(qk_mu_sb[:, rb, cs], ps_mu)
                        ps_sg = psQ.tile([P, 512], f32, tag="qksg")
                        for j in range(8):
                            nc.tensor.matmul(ps_sg, wsig[:, j, :], a2T[:, j, cs],
                                             start=(j == 0), stop=False)
                        for j in range(8):
                            nc.tensor.matmul(ps_sg, wmu2[:, j, :], sg_nT[:, j, cs],
                                             start=False, stop=(j == 7))
                        if rb < 4:
                            nc.scalar.activation(qk_sg_sb[:, rb, cs], ps_sg, AF.Copy, scale=scA)
                        else:
                            nc.scalar.copy(qk_sg_sb[:, rb, cs], ps_sg)

            # --- A2b: V (natural layout), evict straight to SBUF stage ---
            with ExitStack() as ctx:
                wv = ctx.enter_context(tc.tile_pool(name="wv", bufs=1))
                psV = ctx.enter_context(tc.tile_pool(name="psV", bufs=2, space="PSUM"))
                wv_mu = wv.tile([P, 8, 512], bf)
                nc.gpsimd.dma_start(out=wv_mu, in_=io["wv_mu"][:].rearrange("(j p) r -> p j r", p=P))
                wv_sr = wv.tile([P, 8, 512], f32)
                nc.sync.dma_start(out=wv_sr, in_=io["wv_sr"][:].rearrange("(j p) r -> p j r", p=P))
                wv_sg0 = wv.tile([P, 8, 512], f32)
                nc.scalar.activation(wv_sg0, wv_sr, AF.Exp)
                wv_sig = wv.tile([P, 8, 512], bf)
                nc.scalar.activation(wv_sig, wv_sg0, AF.Ln, bias=1.0)
                wv_mu2 = wv.tile([P, 8, 512], bf)
                nc.vector.tensor_mul(wv_mu2, wv_mu, wv_mu)
                for tb in range(8):
                    tsl = slice(tb * P, (tb + 1) * P)
                    ps_mu = psV.tile([P, 512], f32, tag="vmu")
                    for j in range(8):
                        nc.tensor.matmul(ps_mu, mu_nT[:, j, tsl], wv_mu[:, j, :],
                                         start=(j == 0), stop=(j == 7))
                    nc.vector.memset(v_mu_sb[:, tb, :], 1.0)
                    nc.vector.tensor_copy(
                        v_mu_sb[:, tb, :].rearrange("p (h c) -> p h c", c=65)[:, :, 0:64],
                        ps_mu.rearrange("p (h c) -> p h c", c=64))
                    ps_sg = psV.tile([P, 512], f32, tag="vsg")
                    for j in range(8):
                        nc.tensor.matmul(ps_sg, a2T[:, j, tsl], wv_sig[:, j, :],
                                         start=(j == 0), stop=False)
                    for j in range(8):
                        nc.tensor.matmul(ps_sg, sg_nT[:, j, tsl], wv_mu2[:, j, :],
                                         start=False, stop=(j == 7))
                    nc.scalar.copy(v_sg_sb[:, tb, :], ps_sg)

        # ============ Phase B: attention (all operands already in SBUF) ============
        with ExitStack() as ctx:
            ep = ctx.enter_context(tc.tile_pool(name="ep", bufs=20))
            sb3 = ctx.enter_context(tc.tile_pool(name="sb3", bufs=4))
            outsb = ctx.enter_context(tc.tile_pool(name="outsb", bufs=4))
            smallB = ctx.enter_context(tc.tile_pool(name="smallB", bufs=4))
            onesB = ctx.enter_context(tc.tile_pool(name="onesB", bufs=1))
            psD = ctx.enter_context(tc.tile_pool(name="psD", bufs=2, space="PSUM"))
            psS2 = ctx.enter_context(tc.tile_pool(name="psS2", bufs=2, space="PSUM"))
            psAVm = ctx.enter_context(tc.tile_pool(name="psAVm", bufs=2, space="PSUM"))
            psAVs = ctx.enter_context(tc.tile_pool(name="psAVs", bufs=1, space="PSUM"))
            psDB = ctx.enter_context(tc.tile_pool(name="psDB", bufs=1, space="PSUM"))

            ones_blk = onesB.tile([P, P], f32)
            nc.vector.memset(ones_blk, 1.0)
            sc128 = onesB.tile([P, 1], f32)
            nc.vector.memset(sc128, SCALE)

            def pass1(hq, c):
                pr, hh = divmod(hq, 2)
                pb = (hq % 2) * 64
                qrb, krb = hq // 2, 4 + hq // 2
                vco = pr * 130 + hh * 65
                cs = slice(c * 512, (c + 1) * 512)
                av_mu = psAVm.tile([65, 512], f32, tag="avmu", name=f"avmu{hq}_{c}")
                e_ts = []
                for kb in range(8):
                    dots = psD.tile([P, 512], f32, tag="dots", name=f"dots{hq}_{c}_{kb}")
                    nc.tensor.matmul(dots,
                                     qk_mu_sb[pb:pb + 64, krb, kb * P:(kb + 1) * P],
                                     qk_mu_sb[pb:pb + 64, qrb, cs],
                                     start=True, stop=True)
                    e_t = ep.tile([P, 512], bf, tag="e", name=f"e{hq}_{c}_{kb}")
                    nc.scalar.activation(e_t, dots, AF.Exp, scale=sc128)
                    e_ts.append(e_t)
                    nc.tensor.matmul(av_mu, v_mu_sb[:, kb, vco:vco + 65], e_t,
                                     start=(kb == 0), stop=(kb == 7))
                r_sb = smallB.tile([P, 512], f32, tag="r", name=f"r{hq}_{c}")
                nc.vector.reciprocal(r_sb[64:65, :], av_mu[64:65, :])
                dbp = psDB.tile([P, 512], f32, tag="db", name=f"dbp{hq}_{c}")
                nc.tensor.matmul(dbp, ones_blk[64:65, :], r_sb[64:65, :], start=True, stop=True)
                db = sb3.tile([P, 512], f32, tag="db_sb", name=f"db{hq}_{c}")
                nc.vector.tensor_copy(db, dbp)
                muo = outsb.tile([64, 512], bf, tag="muo", name=f"muo{hq}_{c}")
                nc.vector.tensor_mul(muo, av_mu[0:64, :], db[0:64, :])
                nc.sync.dma_start(out=oT_mu_sb[pb:pb + 64, qrb, cs], in_=muo)
                return (hq, c, e_ts, db)

            def pass2(stateA, stateB):
                # both heads of a pair: sigma-AV matmuls col-packed via
                # tile_position (0,0)/(0,64) -> run concurrently on the PE,
                # and the packed [128,512] result evicts straight into the
                # contiguous oT_sg_sb slice (no partition-shift DMA).
                hqA, c, e_tsA, dbA = stateA
                hqB, _, e_tsB, dbB = stateB
                pr = hqA // 2
                qrb, krb = pr, 4 + pr
                cs = slice(c * 512, (c + 1) * 512)
                av2 = psAVs.tile([P, 512], f32, tag="avsg", name=f"avsg{hqA}_{c}")
                for kb in range(8):
                    for hq, pb, e_ts, db in ((hqA, 0, e_tsA, dbA), (hqB, 64, e_tsB, dbB)):
                        sdots = psS2.tile([P, 512], f32, tag="sdots", name=f"sd{hq}_{c}_{kb}")
                        nc.tensor.matmul(sdots,
                                         qk_sg_sb[pb:pb + 64, krb, kb * P:(kb + 1) * P],
                                         qk_sg_sb[pb:pb + 64, qrb, cs],
                                         start=True, stop=True)
                        p_t = sb3.tile([P, 512], f32, tag="p", name=f"p{hq}_{c}_{kb}")
                        nc.gpsimd.tensor_mul(p_t, e_ts[kb], db)
                        t_t = sb3.tile([P, 512], f32, tag="t", name=f"t{hq}_{c}_{kb}")
                        if kb % 2 == 0:
                            nc.vector.scalar_tensor_tensor(t_t, p_t, 1.0, p_t,
                                                           ALU.subtract, ALU.mult)
                        else:
                            m_t = sb3.tile([P, 512], f32, tag="m", name=f"m{hq}_{c}_{kb}")
                            nc.gpsimd.tensor_mul(m_t, p_t, p_t)
                            nc.gpsimd.tensor_sub(t_t, p_t, m_t)
                        u_t = sb3.tile([P, 512], f32, tag="u", name=f"u{hq}_{c}_{kb}")
                        nc.scalar.activation(u_t, t_t, AF.Square)
                        w_t = sb3.tile([P, 512], bf, tag="w", name=f"w{hq}_{c}_{kb}")
                        nc.vector.tensor_mul(w_t, u_t, sdots)
                        nc.tensor.matmul(av2[pb:pb + 64, :],
                                         v_sg_sb[:, kb, hq * 64:(hq + 1) * 64], w_t,
                                         start=(kb == 0), stop=(kb == 7),
                                         tile_position=(0, pb),
                                         skip_group_check=True)
                nc.vector.tensor_copy(oT_sg_sb[:, qrb, cs], av2)

            prev = None
            for pr in range(4):
                for c in range(2):
                    curA = pass1(2 * pr, c)
                    curB = pass1(2 * pr + 1, c)
                    if prev is not None:
                        pass2(*prev)
                    prev = (curA, curB)
            pass2(*prev)

        # ============ Phase C: out-projection ============
        with ExitStack() as ctx:
            wo = ctx.enter_context(tc.tile_pool(name="wo", bufs=1))
            oin = ctx.enter_context(tc.tile_pool(name="oin", bufs=1))
            evC = ctx.enter_context(tc.tile_pool(name="evC", bufs=4))
            psC = ctx.enter_context(tc.tile_pool(name="psC", bufs=2, space="PSUM"))

            wo_mu = wo.tile([P, 4, D], bf)
            nc.gpsimd.dma_start(out=wo_mu, in_=io["wo_mu"][:].rearrange("(j p) o -> p j o", p=P))
            wo_sr = wo.tile([P, 4, D], f32)
            nc.sync.dma_start(out=wo_sr, in_=io["wo_sr"][:].rearrange("(j p) o -> p j o", p=P))
            wo_sg0 = wo.tile([P, 4, D], f32)
            nc.scalar.activation(wo_sg0, wo_sr, AF.Exp)
            wo_sig = wo.tile([P, 4, D], bf)
            nc.scalar.activation(wo_sig, wo_sg0, AF.Ln, bias=1.0)
            wo_mu2 = wo.tile([P, 4, D], bf)
            nc.vector.tensor_mul(wo_mu2, wo_mu, wo_mu)

            a2o = oin.tile([P, 4, N], bf)
            zsq = oin.tile([P, 4, N], f32)
            nc.scalar.activation(zsq, oT_mu_sb, AF.Square)
            nc.vector.tensor_add(a2o, zsq, oT_sg_sb)

            for ob in range(8):
                osl = slice(ob * P, (ob + 1) * P)
                for c in range(2):
                    cs = slice(c * 512, (c + 1) * 512)
                    ps_mu = psC.tile([P, 512], f32, tag="ymu")
                    for j in range(4):
                        nc.tensor.matmul(ps_mu, wo_mu[:, j, osl], oT_mu_sb[:, j, cs],
                                         start=(j == 0), stop=(j == 3))
                    ev1 = evC.tile([P, 512], f32, tag="ev1")
                    nc.vector.tensor_copy(ev1, ps_mu)
                    nc.sync.dma_start(out=io["yT_mu"][osl, cs], in_=ev1)
                    ps_sg = psC.tile([P, 512], f32, tag="ysg")
                    for j in range(4):
                        nc.tensor.matmul(ps_sg, wo_sig[:, j, osl], a2o[:, j, cs],
                                         start=(j == 0), stop=False)
                    for j in range(4):
                        nc.tensor.matmul(ps_sg, wo_mu2[:, j, osl], oT_sg_sb[:, j, cs],
                                         start=False, stop=(j == 3))
                    ev2 = evC.tile([P, 512], f32, tag="ev2")
                    nc.scalar.copy(ev2, ps_sg)
                    nc.sync.dma_start(out=io["yT_sg"][osl, cs], in_=ev2)


def _get_nc():
    if "nc" not in _NC_CACHE:
        _NC_CACHE["nc"] = _build_nc()
    return _NC_CACHE["nc"]


def _prep_core_inputs(c, mu, sigma, ln_gamma, ln_beta, Wqkv_mu, Wqkv_sigma_raw,
                      Wout_mu, Wout_sigma_raw):
    f = np.float32
    asc = np.ascontiguousarray
    b, g = divmod(c, 2)
    qs = slice(512 * g, 512 * (g + 1))
    ks = slice(1024 + 512 * g, 1024 + 512 * (g + 1))
    vs = slice(2048 + 512 * g, 2048 + 512 * (g + 1))
    gb = np.zeros((P, 16), f)
    gb[:, :8] = np.asarray(ln_gamma, f).reshape(8, P).T
    gb[:, 8:] = np.asarray(ln_beta, f).reshape(8, P).T
    wqk_mu = np.concatenate([Wqkv_mu[qs], Wqkv_mu[ks]], 0)
    wqk_sr = np.concatenate([Wqkv_sigma_raw[qs], Wqkv_sigma_raw[ks]], 0)
    return {
        "muT": asc(np.asarray(mu[b], f).T),
        "sgT": asc(np.asarray(sigma[b], f).T),
        "gb": gb,
        "wqk_mu": asc(np.asarray(wqk_mu, f).T),
        "wqk_sr": asc(np.asarray(wqk_sr, f).T),
        "wv_mu": asc(np.asarray(Wqkv_mu[vs], f).T),
        "wv_sr": asc(np.asarray(Wqkv_sigma_raw[vs], f).T),
        "wo_mu": asc(np.asarray(Wout_mu[:, 512 * g:512 * (g + 1)], f).T),
        "wo_sr": asc(np.asarray(Wout_sigma_raw[:, 512 * g:512 * (g + 1)], f).T),
    }


def _emulate_core(m):
    """Pure-numpy mirror of the on-device program (for validation only)."""
    sp = lambda x: np.log1p(np.exp(x))
    muT, sgT = m["muT"], m["sgT"]
    gamma = m["gb"][:, :8].T.reshape(-1)[:, None]   # [D,1] indexed by d
    beta = m["gb"][:, 8:].T.reshape(-1)[:, None]
    mean = muT.mean(0, keepdims=True)
    var = muT.var(0, keepdims=True)
    inv = 1.0 / np.sqrt(var + EPS)
    mu_nT = (muT * inv - mean * inv) * gamma + beta
    sg_nT = sgT * gamma * gamma * inv * inv
    a2T = mu_nT * mu_nT + sg_nT
    qkT_mu = m["wqk_mu"].T @ mu_nT
    qkT_sg = sp(m["wqk_sr"]).T @ a2T + (m["wqk_mu"] ** 2).T @ sg_nT
    v_mu = mu_nT.T @ m["wv_mu"]
    v_sg = a2T.T @ sp(m["wv_sr"]) + sg_nT.T @ m["wv_mu"] ** 2
    oT_mu = np.zeros((RV, N), np.float32)
    oT_sg = np.zeros((RV, N), np.float32)
    for h in range(HPC):
        hs = slice(h * 64, (h + 1) * 64)
        sT = m_kT = qkT_mu[512 + h * 64:512 + (h + 1) * 64].T @ qkT_mu[hs]  # [kt, qt]
        e = np.exp(SCALE * sT)
        den = e.sum(0, keepdims=True)
        db = 1.0 / den
        p = e * db
        oT_mu[hs] = (v_mu[:, hs].T @ e) * db
        sdT = qkT_sg[512 + h * 64:512 + (h + 1) * 64].T @ qkT_sg[hs]
        t = (p - 1.0) * p
        w = (t * t) * SCALE * sdT
        oT_sg[hs] = v_sg[:, hs].T @ w
    a2o = oT_mu * oT_mu + oT_sg
    yT_mu = m["wo_mu"].T @ oT_mu
    yT_sg = sp(m["wo_sr"]).T @ a2o + (m["wo_mu"] ** 2).T @ oT_sg
    return yT_mu.astype(np.float32), yT_sg.astype(np.float32)


def kernel(mu, sigma, ln_gamma, ln_beta, Wqkv_mu, Wqkv_sigma_raw, Wout_mu,
           Wout_sigma_raw, _trace=False):
    from concourse.bass_utils import run_bass_kernel_spmd

    nc = _get_nc()
    args = (mu, sigma, ln_gamma, ln_beta, Wqkv_mu, Wqkv_sigma_raw, Wout_mu,
            Wout_sigma_raw)
    in_maps = [_prep_core_inputs(c, *args) for c in range(8)]
    res = run_bass_kernel_spmd(nc, in_maps, list(range(8)), trace=_trace)
    out_mu = np.zeros((B, N, D), np.float32)
    out_sg = np.zeros((B, N, D), np.float32)
    for c in range(8):
        b = c // 2
        out_mu[b] += res.results[c]["yT_mu"].T
        out_sg[b] += res.results[c]["yT_sg"].T
    if _trace:
        kernel._last_result = res
    return out_mu, out_sg


# revision 25
# speedup vs baseline: 200.1194x; 1.0052x over previous
"""VDP (variance-propagating) attention kernel for Trainium2, 8 NeuronCores.

Sharding: core c -> (batch b = c//2, head-group g = c%2) [8 heads each].
Each core computes LN + its QKV slice + attention for its 8 heads + the
partial out-projection for its 512 inner columns. Host sums the two
head-group partials per batch. No collectives needed.

Layout trick: everything on-device lives transposed as [feature, token]
(activations) / [contraction, out] (weights), prepared host-side, so the
contraction dim is always on partitions and no on-device transposes are
needed anywhere. LayerNorm stats (reduce over features = partitions) are
done with ones-vector matmuls on the PE; softmax denominators come for
free from a ones-augmented column in the V operand of the mu-attention AV
matmul, and are broadcast back across partitions with a K=1 PE matmul.
"""

import os
import sys

import numpy as np

for _p in ("/opt/trn_rl_repo", "/root/.axon_site/_ro/trn_rl_repo"):
    if os.path.isdir(_p) and _p not in sys.path:
        sys.path.insert(0, _p)

HEADS = 16
DH = 64
SCALE = DH ** -0.5
EPS = 1e-5
B, N, D = 4, 1024, 1024
HPC = 8          # heads per core
RQK = 1024       # q+k rows per core (2 * 8 heads * 64)
RV = 512         # v rows per core
P = 128

_NC_CACHE = {}


def _build_nc(tiny_out=False):
    import concourse.bass as bass  # noqa: F401
    import concourse.tile as tile
    from concourse import bacc, mybir

    f32 = mybir.dt.float32
    AF = mybir.ActivationFunctionType
    ALU = mybir.AluOpType

    nc = bacc.Bacc(None, target_bir_lowering=False)

    io = {}
    for name, shape in [
        ("muT", [D, N]), ("sgT", [D, N]), ("gb", [P, 16]),
        ("wqk_mu", [D, RQK]), ("wqk_sr", [D, RQK]),
        ("wv_mu", [D, RV]), ("wv_sr", [D, RV]),
        ("wo_mu", [RV, D]), ("wo_sr", [RV, D]),
    ]:
        io[name] = nc.dram_tensor(name, shape, f32, kind="ExternalInput")
    if tiny_out:
        for name, shape in [("yT_mu", [D, N]), ("yT_sg", [D, N])]:
            io[name] = nc.dram_tensor(name, shape, f32)
        io["done"] = nc.dram_tensor("done", [1, 16], f32, kind="ExternalOutput")
    else:
        for name, shape in [("yT_mu", [D, N]), ("yT_sg", [D, N])]:
            io[name] = nc.dram_tensor(name, shape, f32, kind="ExternalOutput")
    # internal DRAM staging
    bf = mybir.dt.bfloat16

    with tile.TileContext(nc) as tc:
        _emit(nc, tc, io, f32, bf, AF, ALU)
        if tiny_out:
            with tc.tile_pool(name="doneP", bufs=1) as dp:
                dt = dp.tile([1, 16], f32)
                nc.vector.memset(dt, 1.0)
                nc.sync.dma_start(out=io["done"][:], in_=dt)
    nc.compile()
    return nc


def _build_floor_nc():
    import concourse.tile as tile
    from concourse import bacc, mybir

    f32 = mybir.dt.float32
    nc = bacc.Bacc(None, target_bir_lowering=False)
    done = nc.dram_tensor("done", [1, 16], f32, kind="ExternalOutput")
    with tile.TileContext(nc) as tc:
        with tc.tile_pool(name="dp", bufs=1) as dp:
            dt = dp.tile([1, 16], f32)
            nc.vector.memset(dt, 1.0)
            nc.sync.dma_start(out=done[:], in_=dt)
    nc.compile()
    return nc


def _emit(nc, tc, io, f32, bf, AF, ALU):
    from contextlib import ExitStack

    with ExitStack() as tctx:
        stage = tctx.enter_context(tc.tile_pool(name="stage", bufs=1))
        # persistent SBUF staging (bf16): no DRAM round trips between phases
        qk_mu_sb = stage.tile([P, 8, N], bf)    # rows: 0-3 q-blocks, 4-7 k-blocks
        qk_sg_sb = stage.tile([P, 8, N], bf)
        v_mu_sb = stage.tile([P, 8, HPC * 65], bf)   # per tok-block: 8 heads x (64 v + ones)
        v_sg_sb = stage.tile([P, 8, RV], bf)
        oT_mu_sb = stage.tile([P, 4, N], bf)
        oT_sg_sb = stage.tile([P, 4, N], bf)

        # ============ Phase A: LayerNorm + QKV ============
        with ExitStack() as actx:
            acts = actx.enter_context(tc.tile_pool(name="acts", bufs=1))
            smallA = actx.enter_context(tc.tile_pool(name="smallA", bufs=1))

            gb_sb = smallA.tile([P, 16], f32)
            nc.sync.dma_start(out=gb_sb, in_=io["gb"][:])
            g2_sb = smallA.tile([P, 8], f32)
            nc.vector.tensor_mul(g2_sb, gb_sb[:, 0:8], gb_sb[:, 0:8])
            ones_col = smallA.tile([P, 1], f32)
            nc.vector.memset(ones_col, 1.0)
            ones_row = smallA.tile([1, P], f32)
            nc.vector.memset(ones_row, 1.0)
            eps1 = smallA.tile([1, 1], f32)
            nc.vector.memset(eps1, EPS)
            scA = smallA.tile([P, 1], f32)
            nc.vector.memset(scA, SCALE)

            inv_b = acts.tile([P, N], f32)
            minv_b = acts.tile([P, N], f32)
            inv2_b = acts.tile([P, N], f32)
            mu_nT = acts.tile([P, 8, N], bf)
            sg_nT = acts.tile([P, 8, N], bf)
            a2T = acts.tile([P, 8, N], bf)

            # --- A1: stats + normalize (muT streamed twice, not resident) ---
            with ExitStack() as ctx:
                ioA = ctx.enter_context(tc.tile_pool(name="ioA", bufs=2))
                psS = ctx.enter_context(tc.tile_pool(name="psS", bufs=1, space="PSUM"))
                psA = ctx.enter_context(tc.tile_pool(name="psA", bufs=2, space="PSUM"))

                sum_ps = [psS.tile([1, 512], f32, tag=f"sum{c}", name=f"sum{c}") for c in range(2)]
                sq_ps = [psS.tile([1, 512], f32, tag=f"sq{c}", name=f"sq{c}") for c in range(2)]
                for j in range(8):
                    mut = ioA.tile([P, N], f32, tag="mut")
                    nc.sync.dma_start(out=mut, in_=io["muT"][j * P:(j + 1) * P, :])
                    mu2 = ioA.tile([P, N], f32, tag="mu2")
                    nc.scalar.activation(mu2, mut, AF.Square)
                    for c in range(2):
                        cs = slice(c * 512, (c + 1) * 512)
                        nc.tensor.matmul(sum_ps[c], ones_col, mut[:, cs],
                                         start=(j == 0), stop=(j == 7), skip_group_check=True)
                        nc.tensor.matmul(sq_ps[c], ones_col, mu2[:, cs],
                                         start=(j == 0), stop=(j == 7), skip_group_check=True)

                inv_sb = smallA.tile([1, N], f32)
                minv_sb = smallA.tile([1, N], f32)
                for c in range(2):
                    cs = slice(c * 512, (c + 1) * 512)
                    mean_t = ioA.tile([1, 512], f32, tag="mean")
                    nc.vector.tensor_scalar_mul(mean_t, sum_ps[c], 1.0 / D)
                    m2_t = ioA.tile([1, 512], f32, tag="m2")
                    nc.vector.tensor_mul(m2_t, mean_t, mean_t)
                    var_t = ioA.tile([1, 512], f32, tag="var")
                    nc.vector.scalar_tensor_tensor(var_t, sq_ps[c], 1.0 / D, m2_t,
                                                   ALU.mult, ALU.subtract)
                    std_t = ioA.tile([1, 512], f32, tag="std")
                    nc.scalar.activation(std_t, var_t, AF.Sqrt, bias=eps1)
                    nc.vector.reciprocal(inv_sb[:, cs], std_t)
                    nc.vector.scalar_tensor_tensor(minv_sb[:, cs], mean_t, -1.0, inv_sb[:, cs],
                                                   ALU.mult, ALU.mult)

                for c in range(2):
                    cs = slice(c * 512, (c + 1) * 512)
                    bp1 = psA.tile([P, 512], f32, tag="bcast")
                    nc.tensor.matmul(bp1, ones_row, inv_sb[:, cs], start=True, stop=True)
                    nc.vector.tensor_copy(inv_b[:, cs], bp1)
                    bp2 = psA.tile([P, 512], f32, tag="bcast")
                    nc.tensor.matmul(bp2, ones_row, minv_sb[:, cs], start=True, stop=True)
                    nc.vector.tensor_copy(minv_b[:, cs], bp2)
                nc.vector.tensor_mul(inv2_b, inv_b, inv_b)

                for j in range(8):
                    mut = ioA.tile([P, N], f32, tag="mut")
                    nc.sync.dma_start(out=mut, in_=io["muT"][j * P:(j + 1) * P, :])
                    x2 = ioA.tile([P, N], f32, tag="x2")
                    nc.vector.tensor_mul(x2, mut, inv_b)
                    nc.vector.tensor_add(x2, x2, minv_b)
                    nc.vector.tensor_scalar(mu_nT[:, j, :], x2, gb_sb[:, j:j + 1],
                                            gb_sb[:, 8 + j:9 + j], ALU.mult, ALU.add)
                    sgt = ioA.tile([P, N], f32, tag="sgt")
                    nc.sync.dma_start(out=sgt, in_=io["sgT"][j * P:(j + 1) * P, :])
                    nc.vector.scalar_tensor_tensor(sg_nT[:, j, :], sgt, g2_sb[:, j:j + 1],
                                                   inv2_b, ALU.mult, ALU.mult)
                    z = ioA.tile([P, N], f32, tag="z")
                    nc.scalar.activation(z, mu_nT[:, j, :], AF.Square)
                    nc.gpsimd.tensor_add(a2T[:, j, :], z, sg_nT[:, j, :])

            # --- A2a: QKV q,k rows (transposed out), evict straight to SBUF stage ---
            with ExitStack() as ctx:
                wq = ctx.enter_context(tc.tile_pool(name="wq", bufs=2))
                psQ = ctx.enter_context(tc.tile_pool(name="psQ", bufs=2, space="PSUM"))
                for rb in range(8):
                    rsl = slice(rb * P, (rb + 1) * P)
                    wmu = wq.tile([P, 8, P], bf, tag="wmu")
                    nc.gpsimd.dma_start(out=wmu, in_=io["wqk_mu"][:, rsl].rearrange("(j p) r -> p j r", p=P))
                    wsr = wq.tile([P, 8, P], f32, tag="wsr")
                    nc.sync.dma_start(out=wsr, in_=io["wqk_sr"][:, rsl].rearrange("(j p) r -> p j r", p=P))
                    wsg0 = wq.tile([P, 8, P], f32, tag="wsg0")
                    nc.scalar.activation(wsg0, wsr, AF.Exp)
                    wsig = wq.tile([P, 8, P], bf, tag="wsig")
                    nc.scalar.activation(wsig, wsg0, AF.Ln, bias=1.0)
                    wmu2 = wq.tile([P, 8, P], bf, tag="wmu2")
                    nc.vector.tensor_mul(wmu2, wmu, wmu)
                    for c in range(2):
                        cs = slice(c * 512, (c + 1) * 512)
                        ps_mu = psQ.tile([P, 512], f32, tag="qkmu")
                        for j in range(8):
                            nc.tensor.matmul(ps_mu, wmu[:, j, :], mu_nT[:, j, cs],
                                             start=(j == 0), stop=(j == 7))
                        nc.vector.tensor_copy(qk_mu_sb[:, rb, cs], ps_mu)
                        ps_sg = psQ.tile([P, 512], f32, tag="qksg")
                        for j in range(8):
                            nc.tensor.matmul(ps_sg, wsig[:, j, :], a2T[:, j, cs],
                                             start=(j == 0), stop=False)
                        for j in range(8):
                            nc.tensor.matmul(ps_sg, wmu2[:, j, :], sg_nT[:, j, cs],
                                             start=False, stop=(j == 7))
                        if rb < 4:
                            nc.scalar.activation(qk_sg_sb[:, rb, cs], ps_sg, AF.Copy, scale=scA)
                        else:
                            nc.scalar.copy(qk_sg_sb[:, rb, cs], ps_sg)

            # --- A2b: V (natural layout), evict straight to SBUF stage ---
            with ExitStack() as ctx:
                wv = ctx.enter_context(tc.tile_pool(name="wv", bufs=1))
                psV = ctx.enter_context(tc.tile_pool(name="psV", bufs=2, space="PSUM"))
                wv_mu = wv.tile([P, 8, 512], bf)
                nc.gpsimd.dma_start(out=wv_mu, in_=io["wv_mu"][:].rearrange("(j p) r -> p j r", p=P))
                wv_sr = wv.tile([P, 8, 512], f32)
                nc.sync.dma_start(out=wv_sr, in_=io["wv_sr"][:].rearrange("(j p) r -> p j r", p=P))
                wv_sg0 = wv.tile([P, 8, 512], f32)
                nc.scalar.activation(wv_sg0, wv_sr, AF.Exp)
                wv_sig = wv.tile([P, 8, 512], bf)
                nc.scalar.activation(wv_sig, wv_sg0, AF.Ln, bias=1.0)
                wv_mu2 = wv.tile([P, 8, 512], bf)
                nc.vector.tensor_mul(wv_mu2, wv_mu, wv_mu)
                for tb in range(8):
                    tsl = slice(tb * P, (tb + 1) * P)
                    ps_mu = psV.tile([P, 512], f32, tag="vmu")
                    for j in range(8):
                        nc.tensor.matmul(ps_mu, mu_nT[:, j, tsl], wv_mu[:, j, :],
                                         start=(j == 0), stop=(j == 7))
                    nc.vector.memset(v_mu_sb[:, tb, :], 1.0)
                    nc.vector.tensor_copy(
                        v_mu_sb[:, tb, :].rearrange("p (h c) -> p h c", c=65)[:, :, 0:64],
                        ps_mu.rearrange("p (h c) -> p h c", c=64))
                    ps_sg = psV.tile([P, 512], f32, tag="vsg")
                    for j in range(8):
                        nc.tensor.matmul(ps_sg, a2T[:, j, tsl], wv_sig[:, j, :],
                                         start=(j == 0), stop=False)
                    for j in range(8):
                        nc.tensor.matmul(ps_sg, sg_nT[:, j, tsl], wv_mu2[:, j, :],
                                         start=False, stop=(j == 7))
                    nc.scalar.copy(v_sg_sb[:, tb, :], ps_sg)

        # ============ Phase B: attention (all operands already in SBUF) ============
        with ExitStack() as ctx:
            ep = ctx.enter_context(tc.tile_pool(name="ep", bufs=36))
            sb3 = ctx.enter_context(tc.tile_pool(name="sb3", bufs=6))
            outsb = ctx.enter_context(tc.tile_pool(name="outsb", bufs=4))
            smallB = ctx.enter_context(tc.tile_pool(name="smallB", bufs=4))
            onesB = ctx.enter_context(tc.tile_pool(name="onesB", bufs=1))
            psD = ctx.enter_context(tc.tile_pool(name="psD", bufs=2, space="PSUM"))
            psS2 = ctx.enter_context(tc.tile_pool(name="psS2", bufs=2, space="PSUM"))
            psAVm = ctx.enter_context(tc.tile_pool(name="psAVm", bufs=2, space="PSUM"))
            psAVs = ctx.enter_context(tc.tile_pool(name="psAVs", bufs=1, space="PSUM"))
            psDB = ctx.enter_context(tc.tile_pool(name="psDB", bufs=1, space="PSUM"))

            ones_blk = onesB.tile([P, P], f32)
            nc.vector.memset(ones_blk, 1.0)
            sc128 = onesB.tile([P, 1], f32)
            nc.vector.memset(sc128, SCALE)

            def pass1(hq, c):
                pr, hh = divmod(hq, 2)
                pb = (hq % 2) * 64
                qrb, krb = hq // 2, 4 + hq // 2
                vco = pr * 130 + hh * 65
                cs = slice(c * 512, (c + 1) * 512)
                av_mu = psAVm.tile([65, 512], f32, tag="avmu", name=f"avmu{hq}_{c}")
                e_ts = []
                for kb in range(8):
                    dots = psD.tile([P, 512], f32, tag="dots", name=f"dots{hq}_{c}_{kb}")
                    nc.tensor.matmul(dots,
                                     qk_mu_sb[pb:pb + 64, krb, kb * P:(kb + 1) * P],
                                     qk_mu_sb[pb:pb + 64, qrb, cs],
                                     start=True, stop=True)
                    e_t = ep.tile([P, 512], bf, tag="e", name=f"e{hq}_{c}_{kb}")
                    nc.scalar.activation(e_t, dots, AF.Exp, scale=sc128)
                    e_ts.append(e_t)
                    nc.tensor.matmul(av_mu, v_mu_sb[:, kb, vco:vco + 65], e_t,
                                     start=(kb == 0), stop=(kb == 7))
                r_sb = smallB.tile([P, 512], f32, tag="r", name=f"r{hq}_{c}")
                nc.vector.reciprocal(r_sb[64:65, :], av_mu[64:65, :])
                dbp = psDB.tile([P, 512], f32, tag="db", name=f"dbp{hq}_{c}")
                nc.tensor.matmul(dbp, ones_blk[64:65, :], r_sb[64:65, :], start=True, stop=True)
                db = sb3.tile([P, 512], f32, tag="db_sb", name=f"db{hq}_{c}")
                nc.vector.tensor_copy(db, dbp)
                muo = outsb.tile([64, 512], bf, tag="muo", name=f"muo{hq}_{c}")
                nc.vector.tensor_mul(muo, av_mu[0:64, :], db[0:64, :])
                nc.sync.dma_start(out=oT_mu_sb[pb:pb + 64, qrb, cs], in_=muo)
                return (hq, c, e_ts, db)

            def pass2(stateA, stateB):
                # both heads of a pair: sigma-AV matmuls col-packed via
                # tile_position (0,0)/(0,64) -> run concurrently on the PE,
                # and the packed [128,512] result evicts straight into the
                # contiguous oT_sg_sb slice (no partition-shift DMA).
                hqA, c, e_tsA, dbA = stateA
                hqB, _, e_tsB, dbB = stateB
                pr = hqA // 2
                qrb, krb = pr, 4 + pr
                cs = slice(c * 512, (c + 1) * 512)
                av2 = psAVs.tile([P, 512], f32, tag="avsg", name=f"avsg{hqA}_{c}")
                for kb in range(8):
                    for hq, pb, e_ts, db in ((hqA, 0, e_tsA, dbA), (hqB, 64, e_tsB, dbB)):
                        sdots = psS2.tile([P, 512], f32, tag="sdots", name=f"sd{hq}_{c}_{kb}")
                        nc.tensor.matmul(sdots,
                                         qk_sg_sb[pb:pb + 64, krb, kb * P:(kb + 1) * P],
                                         qk_sg_sb[pb:pb + 64, qrb, cs],
                                         start=True, stop=True)
                        p_t = sb3.tile([P, 512], f32, tag="p", name=f"p{hq}_{c}_{kb}")
                        nc.gpsimd.tensor_mul(p_t, e_ts[kb], db)
                        t_t = sb3.tile([P, 512], f32, tag="t", name=f"t{hq}_{c}_{kb}")
                        if kb % 2 == 0:
                            nc.vector.scalar_tensor_tensor(t_t, p_t, 1.0, p_t,
                                                           ALU.subtract, ALU.mult)
                        else:
                            m_t = sb3.tile([P, 512], f32, tag="m", name=f"m{hq}_{c}_{kb}")
                            nc.gpsimd.tensor_mul(m_t, p_t, p_t)
                            nc.gpsimd.tensor_sub(t_t, p_t, m_t)
                        u_t = sb3.tile([P, 512], f32, tag="u", name=f"u{hq}_{c}_{kb}")
                        nc.scalar.activation(u_t, t_t, AF.Square)
                        w_t = sb3.tile([P, 512], bf, tag="w", name=f"w{hq}_{c}_{kb}")
                        nc.vector.tensor_mul(w_t, u_t, sdots)
                        nc.tensor.matmul(av2[pb:pb + 64, :],
                                         v_sg_sb[:, kb, hq * 64:(hq + 1) * 64], w_t,
                                         start=(kb == 0), stop=(kb == 7),
                                         tile_position=(0, pb),
                                         skip_group_check=True)
                nc.vector.tensor_copy(oT_sg_sb[:, qrb, cs], av2)

            prev = None
            for pr in range(4):
                for c in range(2):
                    curA = pass1(2 * pr, c)
                    curB = pass1(2 * pr + 1, c)
                    if prev is not None:
                        pass2(*prev)
                    prev = (curA, curB)
            pass2(*prev)

        # ============ Phase C: out-projection ============
        with ExitStack() as ctx:
            wo = ctx.enter_context(tc.tile_pool(name="wo", bufs=1))
            oin = ctx.enter_context(tc.tile_pool(name="oin", bufs=1))
            evC = ctx.enter_context(tc.tile_pool(name="evC", bufs=4))
            psC = ctx.enter_context(tc.tile_pool(name="psC", bufs=2, space="PSUM"))

            wo_mu = wo.tile([P, 4, D], bf)
            nc.gpsimd.dma_start(out=wo_mu, in_=io["wo_mu"][:].rearrange("(j p) o -> p j o", p=P))
            wo_sr = wo.tile([P, 4, D], f32)
            nc.sync.dma_start(out=wo_sr, in_=io["wo_sr"][:].rearrange("(j p) o -> p j o", p=P))
            wo_sg0 = wo.tile([P, 4, D], f32)
            nc.scalar.activation(wo_sg0, wo_sr, AF.Exp)
            wo_sig = wo.tile([P, 4, D], bf)
            nc.scalar.activation(wo_sig, wo_sg0, AF.Ln, bias=1.0)
            wo_mu2 = wo.tile([P, 4, D], bf)
            nc.vector.tensor_mul(wo_mu2, wo_mu, wo_mu)

            a2o = oin.tile([P, 4, N], bf)
            zsq = oin.tile([P, 4, N], f32)
            nc.scalar.activation(zsq, oT_mu_sb, AF.Square)
            nc.vector.tensor_add(a2o, zsq, oT_sg_sb)

            for ob in range(8):
                osl = slice(ob * P, (ob + 1) * P)
                for c in range(2):
                    cs = slice(c * 512, (c + 1) * 512)
                    ps_mu = psC.tile([P, 512], f32, tag="ymu")
                    for j in range(4):
                        nc.tensor.matmul(ps_mu, wo_mu[:, j, osl], oT_mu_sb[:, j, cs],
                                         start=(j == 0), stop=(j == 3))
                    ev1 = evC.tile([P, 512], f32, tag="ev1")
                    nc.vector.tensor_copy(ev1, ps_mu)
                    nc.sync.dma_start(out=io["yT_mu"][osl, cs], in_=ev1)
                    ps_sg = psC.tile([P, 512], f32, tag="ysg")
                    for j in range(4):
                        nc.tensor.matmul(ps_sg, wo_sig[:, j, osl], a2o[:, j, cs],
                                         start=(j == 0), stop=False)
                    for j in range(4):
                        nc.tensor.matmul(ps_sg, wo_mu2[:, j, osl], oT_sg_sb[:, j, cs],
                                         start=False, stop=(j == 3))
                    ev2 = evC.tile([P, 512], f32, tag="ev2")
                    nc.scalar.copy(ev2, ps_sg)
                    nc.sync.dma_start(out=io["yT_sg"][osl, cs], in_=ev2)


def _get_nc():
    if "nc" not in _NC_CACHE:
        _NC_CACHE["nc"] = _build_nc()
    return _NC_CACHE["nc"]


def _prep_core_inputs(c, mu, sigma, ln_gamma, ln_beta, Wqkv_mu, Wqkv_sigma_raw,
                      Wout_mu, Wout_sigma_raw):
    f = np.float32
    asc = np.ascontiguousarray
    b, g = divmod(c, 2)
    qs = slice(512 * g, 512 * (g + 1))
    ks = slice(1024 + 512 * g, 1024 + 512 * (g + 1))
    vs = slice(2048 + 512 * g, 2048 + 512 * (g + 1))
    gb = np.zeros((P, 16), f)
    gb[:, :8] = np.asarray(ln_gamma, f).reshape(8, P).T
    gb[:, 8:] = np.asarray(ln_beta, f).reshape(8, P).T
    wqk_mu = np.concatenate([Wqkv_mu[qs], Wqkv_mu[ks]], 0)
    wqk_sr = np.concatenate([Wqkv_sigma_raw[qs], Wqkv_sigma_raw[ks]], 0)
    return {
        "muT": asc(np.asarray(mu[b], f).T),
        "sgT": asc(np.asarray(sigma[b], f).T),
        "gb": gb,
        "wqk_mu": asc(np.asarray(wqk_mu, f).T),
        "wqk_sr": asc(np.asarray(wqk_sr, f).T),
        "wv_mu": asc(np.asarray(Wqkv_mu[vs], f).T),
        "wv_sr": asc(np.asarray(Wqkv_sigma_raw[vs], f).T),
        "wo_mu": asc(np.asarray(Wout_mu[:, 512 * g:512 * (g + 1)], f).T),
        "wo_sr": asc(np.asarray(Wout_sigma_raw[:, 512 * g:512 * (g + 1)], f).T),
    }


def _emulate_core(m):
    """Pure-numpy mirror of the on-device program (for validation only)."""
    sp = lambda x: np.log1p(np.exp(x))
    muT, sgT = m["muT"], m["sgT"]
    gamma = m["gb"][:, :8].T.reshape(-1)[:, None]   # [D,1] indexed by d
    beta = m["gb"][:, 8:].T.reshape(-1)[:, None]
    mean = muT.mean(0, keepdims=True)
    var = muT.var(0, keepdims=True)
    inv = 1.0 / np.sqrt(var + EPS)
    mu_nT = (muT * inv - mean * inv) * gamma + beta
    sg_nT = sgT * gamma * gamma * inv * inv
    a2T = mu_nT * mu_nT + sg_nT
    qkT_mu = m["wqk_mu"].T @ mu_nT
    qkT_sg = sp(m["wqk_sr"]).T @ a2T + (m["wqk_mu"] ** 2).T @ sg_nT
    v_mu = mu_nT.T @ m["wv_mu"]
    v_sg = a2T.T @ sp(m["wv_sr"]) + sg_nT.T @ m["wv_mu"] ** 2
    oT_mu = np.zeros((RV, N), np.float32)
    oT_sg = np.zeros((RV, N), np.float32)
    for h in range(HPC):
        hs = slice(h * 64, (h + 1) * 64)
        sT = m_kT = qkT_mu[512 + h * 64:512 + (h + 1) * 64].T @ qkT_mu[hs]  # [kt, qt]
        e = np.exp(SCALE * sT)
        den = e.sum(0, keepdims=True)
        db = 1.0 / den
        p = e * db
        oT_mu[hs] = (v_mu[:, hs].T @ e) * db
        sdT = qkT_sg[512 + h * 64:512 + (h + 1) * 64].T @ qkT_sg[hs]
        t = (p - 1.0) * p
        w = (t * t) * SCALE * sdT
        oT_sg[hs] = v_sg[:, hs].T @ w
    a2o = oT_mu * oT_mu + oT_sg
    yT_mu = m["wo_mu"].T @ oT_mu
    yT_sg = sp(m["wo_sr"]).T @ a2o + (m["wo_mu"] ** 2).T @ oT_sg
    return yT_mu.astype(np.float32), yT_sg.astype(np.float32)


def kernel(mu, sigma, ln_gamma, ln_beta, Wqkv_mu, Wqkv_sigma_raw, Wout_mu,
           Wout_sigma_raw, _trace=False):
    from concourse.bass_utils import run_bass_kernel_spmd

    nc = _get_nc()
    args = (mu, sigma, ln_gamma, ln_beta, Wqkv_mu, Wqkv_sigma_raw, Wout_mu,
            Wout_sigma_raw)
    in_maps = [_prep_core_inputs(c, *args) for c in range(8)]
    res = run_bass_kernel_spmd(nc, in_maps, list(range(8)), trace=_trace)
    out_mu = np.zeros((B, N, D), np.float32)
    out_sg = np.zeros((B, N, D), np.float32)
    for c in range(8):
        b = c // 2
        out_mu[b] += res.results[c]["yT_mu"].T
        out_sg[b] += res.results[c]["yT_sg"].T
    if _trace:
        kernel._last_result = res
    return out_mu, out_sg


# revision 28
# speedup vs baseline: 200.2473x; 1.0006x over previous
"""VDP (variance-propagating) attention kernel for Trainium2, 8 NeuronCores.

Sharding: core c -> (batch b = c//2, head-group g = c%2) [8 heads each].
Each core computes LN + its QKV slice + attention for its 8 heads + the
partial out-projection for its 512 inner columns. Host sums the two
head-group partials per batch. No collectives needed.

Layout trick: everything on-device lives transposed as [feature, token]
(activations) / [contraction, out] (weights), prepared host-side, so the
contraction dim is always on partitions and no on-device transposes are
needed anywhere. LayerNorm stats (reduce over features = partitions) are
done with ones-vector matmuls on the PE; softmax denominators come for
free from a ones-augmented column in the V operand of the mu-attention AV
matmul, and are broadcast back across partitions with a K=1 PE matmul.
"""

import os
import sys

import numpy as np

for _p in ("/opt/trn_rl_repo", "/root/.axon_site/_ro/trn_rl_repo"):
    if os.path.isdir(_p) and _p not in sys.path:
        sys.path.insert(0, _p)

HEADS = 16
DH = 64
SCALE = DH ** -0.5
EPS = 1e-5
B, N, D = 4, 1024, 1024
HPC = 8          # heads per core
RQK = 1024       # q+k rows per core (2 * 8 heads * 64)
RV = 512         # v rows per core
P = 128

_NC_CACHE = {}


def _build_nc(tiny_out=False):
    import concourse.bass as bass  # noqa: F401
    import concourse.tile as tile
    from concourse import bacc, mybir

    f32 = mybir.dt.float32
    AF = mybir.ActivationFunctionType
    ALU = mybir.AluOpType

    nc = bacc.Bacc(None, target_bir_lowering=False)

    io = {}
    for name, shape in [
        ("muT", [D, N]), ("sgT", [D, N]), ("gb", [P, 16]),
        ("wqk_mu", [D, RQK]), ("wqk_sr", [D, RQK]),
        ("wv_mu", [D, RV]), ("wv_sr", [D, RV]),
        ("wo_mu", [RV, D]), ("wo_sr", [RV, D]),
    ]:
        io[name] = nc.dram_tensor(name, shape, f32, kind="ExternalInput")
    if tiny_out:
        for name, shape in [("yT_mu", [D, N]), ("yT_sg", [D, N])]:
            io[name] = nc.dram_tensor(name, shape, f32)
        io["done"] = nc.dram_tensor("done", [1, 16], f32, kind="ExternalOutput")
    else:
        for name, shape in [("yT_mu", [D, N]), ("yT_sg", [D, N])]:
            io[name] = nc.dram_tensor(name, shape, f32, kind="ExternalOutput")
    # internal DRAM staging
    bf = mybir.dt.bfloat16

    with tile.TileContext(nc) as tc:
        _emit(nc, tc, io, f32, bf, AF, ALU)
        if tiny_out:
            with tc.tile_pool(name="doneP", bufs=1) as dp:
                dt = dp.tile([1, 16], f32)
                nc.vector.memset(dt, 1.0)
                nc.sync.dma_start(out=io["done"][:], in_=dt)
    nc.compile()
    return nc


def _build_floor_nc():
    import concourse.tile as tile
    from concourse import bacc, mybir

    f32 = mybir.dt.float32
    nc = bacc.Bacc(None, target_bir_lowering=False)
    done = nc.dram_tensor("done", [1, 16], f32, kind="ExternalOutput")
    with tile.TileContext(nc) as tc:
        with tc.tile_pool(name="dp", bufs=1) as dp:
            dt = dp.tile([1, 16], f32)
            nc.vector.memset(dt, 1.0)
            nc.sync.dma_start(out=done[:], in_=dt)
    nc.compile()
    return nc


def _emit(nc, tc, io, f32, bf, AF, ALU):
    from contextlib import ExitStack

    with ExitStack() as tctx:
        stage = tctx.enter_context(tc.tile_pool(name="stage", bufs=1))
        # persistent SBUF staging (bf16): no DRAM round trips between phases
        qk_mu_sb = stage.tile([P, 8, N], bf)    # rows: 0-3 q-blocks, 4-7 k-blocks
        qk_sg_sb = stage.tile([P, 8, N], bf)
        v_mu_sb = stage.tile([P, 8, HPC * 65], bf)   # per tok-block: 8 heads x (64 v + ones)
        v_sg_sb = stage.tile([P, 8, RV], bf)
        oT_mu_sb = stage.tile([P, 4, N], bf)
        oT_sg_sb = stage.tile([P, 4, N], bf)

        # ============ Phase A: LayerNorm + QKV ============
        with ExitStack() as actx:
            acts = actx.enter_context(tc.tile_pool(name="acts", bufs=1))
            smallA = actx.enter_context(tc.tile_pool(name="smallA", bufs=1))

            gb_sb = smallA.tile([P, 16], f32)
            nc.sync.dma_start(out=gb_sb, in_=io["gb"][:])
            g2_sb = smallA.tile([P, 8], f32)
            nc.vector.tensor_mul(g2_sb, gb_sb[:, 0:8], gb_sb[:, 0:8])
            ones_col = smallA.tile([P, 1], f32)
            nc.vector.memset(ones_col, 1.0)
            ones_row = smallA.tile([1, P], f32)
            nc.vector.memset(ones_row, 1.0)
            eps1 = smallA.tile([1, 1], f32)
            nc.vector.memset(eps1, EPS)
            scA = smallA.tile([P, 1], f32)
            nc.vector.memset(scA, SCALE)

            inv_b = acts.tile([P, N], f32)
            minv_b = acts.tile([P, N], f32)
            inv2_b = acts.tile([P, N], f32)
            mu_nT = acts.tile([P, 8, N], bf)
            sg_nT = acts.tile([P, 8, N], bf)
            a2T = acts.tile([P, 8, N], bf)

            # --- A1: stats + normalize (muT streamed twice, not resident) ---
            with ExitStack() as ctx:
                ioA = ctx.enter_context(tc.tile_pool(name="ioA", bufs=2))
                psS = ctx.enter_context(tc.tile_pool(name="psS", bufs=1, space="PSUM"))
                psA = ctx.enter_context(tc.tile_pool(name="psA", bufs=2, space="PSUM"))

                sum_ps = [psS.tile([1, 512], f32, tag=f"sum{c}", name=f"sum{c}") for c in range(2)]
                sq_ps = [psS.tile([1, 512], f32, tag=f"sq{c}", name=f"sq{c}") for c in range(2)]
                for j in range(8):
                    mut = ioA.tile([P, N], f32, tag="mut")
                    nc.sync.dma_start(out=mut, in_=io["muT"][j * P:(j + 1) * P, :])
                    mu2 = ioA.tile([P, N], f32, tag="mu2")
                    nc.gpsimd.tensor_mul(mu2, mut, mut)
                    for c in range(2):
                        cs = slice(c * 512, (c + 1) * 512)
                        nc.tensor.matmul(sum_ps[c], ones_col, mut[:, cs],
                                         start=(j == 0), stop=(j == 7), skip_group_check=True)
                        nc.tensor.matmul(sq_ps[c], ones_col, mu2[:, cs],
                                         start=(j == 0), stop=(j == 7), skip_group_check=True)

                inv_sb = smallA.tile([1, N], f32)
                minv_sb = smallA.tile([1, N], f32)
                for c in range(2):
                    cs = slice(c * 512, (c + 1) * 512)
                    mean_t = ioA.tile([1, 512], f32, tag="mean")
                    nc.vector.tensor_scalar_mul(mean_t, sum_ps[c], 1.0 / D)
                    m2_t = ioA.tile([1, 512], f32, tag="m2")
                    nc.vector.tensor_mul(m2_t, mean_t, mean_t)
                    var_t = ioA.tile([1, 512], f32, tag="var")
                    nc.vector.scalar_tensor_tensor(var_t, sq_ps[c], 1.0 / D, m2_t,
                                                   ALU.mult, ALU.subtract)
                    std_t = ioA.tile([1, 512], f32, tag="std")
                    nc.scalar.activation(std_t, var_t, AF.Sqrt, bias=eps1)
                    nc.vector.reciprocal(inv_sb[:, cs], std_t)
                    nc.vector.scalar_tensor_tensor(minv_sb[:, cs], mean_t, -1.0, inv_sb[:, cs],
                                                   ALU.mult, ALU.mult)

                for c in range(2):
                    cs = slice(c * 512, (c + 1) * 512)
                    bp1 = psA.tile([P, 512], f32, tag="bcast")
                    nc.tensor.matmul(bp1, ones_row, inv_sb[:, cs], start=True, stop=True)
                    nc.vector.tensor_copy(inv_b[:, cs], bp1)
                    bp2 = psA.tile([P, 512], f32, tag="bcast")
                    nc.tensor.matmul(bp2, ones_row, minv_sb[:, cs], start=True, stop=True)
                    nc.vector.tensor_copy(minv_b[:, cs], bp2)
                nc.vector.tensor_mul(inv2_b, inv_b, inv_b)

                for j in range(8):
                    mut = ioA.tile([P, N], f32, tag="mut")
                    nc.sync.dma_start(out=mut, in_=io["muT"][j * P:(j + 1) * P, :])
                    x2 = ioA.tile([P, N], f32, tag="x2")
                    nc.vector.tensor_mul(x2, mut, inv_b)
                    nc.vector.tensor_add(x2, x2, minv_b)
                    nc.vector.tensor_scalar(mu_nT[:, j, :], x2, gb_sb[:, j:j + 1],
                                            gb_sb[:, 8 + j:9 + j], ALU.mult, ALU.add)
                    sgt = ioA.tile([P, N], f32, tag="sgt")
                    nc.sync.dma_start(out=sgt, in_=io["sgT"][j * P:(j + 1) * P, :])
                    nc.vector.scalar_tensor_tensor(sg_nT[:, j, :], sgt, g2_sb[:, j:j + 1],
                                                   inv2_b, ALU.mult, ALU.mult)
                    z = ioA.tile([P, N], f32, tag="z")
                    nc.gpsimd.tensor_mul(z, mu_nT[:, j, :], mu_nT[:, j, :])
                    nc.gpsimd.tensor_add(a2T[:, j, :], z, sg_nT[:, j, :])

            # --- A2a: QKV q,k rows (transposed out), evict straight to SBUF stage ---
            with ExitStack() as ctx:
                wq = ctx.enter_context(tc.tile_pool(name="wq", bufs=2))
                psQ = ctx.enter_context(tc.tile_pool(name="psQ", bufs=2, space="PSUM"))
                for rb in range(8):
                    rsl = slice(rb * P, (rb + 1) * P)
                    wmu = wq.tile([P, 8, P], bf, tag="wmu")
                    nc.gpsimd.dma_start(out=wmu, in_=io["wqk_mu"][:, rsl].rearrange("(j p) r -> p j r", p=P))
                    wsr = wq.tile([P, 8, P], f32, tag="wsr")
                    nc.sync.dma_start(out=wsr, in_=io["wqk_sr"][:, rsl].rearrange("(j p) r -> p j r", p=P))
                    wsg0 = wq.tile([P, 8, P], f32, tag="wsg0")
                    nc.scalar.activation(wsg0, wsr, AF.Exp)
                    wsig = wq.tile([P, 8, P], bf, tag="wsig")
                    nc.scalar.activation(wsig, wsg0, AF.Ln, bias=1.0)
                    wmu2 = wq.tile([P, 8, P], bf, tag="wmu2")
                    nc.vector.tensor_mul(wmu2, wmu, wmu)
                    for c in range(2):
                        cs = slice(c * 512, (c + 1) * 512)
                        ps_mu = psQ.tile([P, 512], f32, tag="qkmu")
                        for j in range(8):
                            nc.tensor.matmul(ps_mu, wmu[:, j, :], mu_nT[:, j, cs],
                                             start=(j == 0), stop=(j == 7))
                        nc.vector.tensor_copy(qk_mu_sb[:, rb, cs], ps_mu)
                        ps_sg = psQ.tile([P, 512], f32, tag="qksg")
                        for j in range(8):
                            nc.tensor.matmul(ps_sg, wsig[:, j, :], a2T[:, j, cs],
                                             start=(j == 0), stop=False)
                        for j in range(8):
                            nc.tensor.matmul(ps_sg, wmu2[:, j, :], sg_nT[:, j, cs],
                                             start=False, stop=(j == 7))
                        if rb < 4:
                            nc.scalar.activation(qk_sg_sb[:, rb, cs], ps_sg, AF.Copy, scale=scA)
                        else:
                            nc.scalar.copy(qk_sg_sb[:, rb, cs], ps_sg)

            # --- A2b: V (natural layout), evict straight to SBUF stage ---
            with ExitStack() as ctx:
                wv = ctx.enter_context(tc.tile_pool(name="wv", bufs=1))
                psV = ctx.enter_context(tc.tile_pool(name="psV", bufs=2, space="PSUM"))
                wv_mu = wv.tile([P, 8, 512], bf)
                nc.gpsimd.dma_start(out=wv_mu, in_=io["wv_mu"][:].rearrange("(j p) r -> p j r", p=P))
                wv_sr = wv.tile([P, 8, 512], f32)
                nc.sync.dma_start(out=wv_sr, in_=io["wv_sr"][:].rearrange("(j p) r -> p j r", p=P))
                wv_sg0 = wv.tile([P, 8, 512], f32)
                nc.scalar.activation(wv_sg0, wv_sr, AF.Exp)
                wv_sig = wv.tile([P, 8, 512], bf)
                nc.scalar.activation(wv_sig, wv_sg0, AF.Ln, bias=1.0)
                wv_mu2 = wv.tile([P, 8, 512], bf)
                nc.vector.tensor_mul(wv_mu2, wv_mu, wv_mu)
                for tb in range(8):
                    tsl = slice(tb * P, (tb + 1) * P)
                    ps_mu = psV.tile([P, 512], f32, tag="vmu")
                    for j in range(8):
                        nc.tensor.matmul(ps_mu, mu_nT[:, j, tsl], wv_mu[:, j, :],
                                         start=(j == 0), stop=(j == 7))
                    nc.vector.memset(v_mu_sb[:, tb, :], 1.0)
                    nc.vector.tensor_copy(
                        v_mu_sb[:, tb, :].rearrange("p (h c) -> p h c", c=65)[:, :, 0:64],
                        ps_mu.rearrange("p (h c) -> p h c", c=64))
                    ps_sg = psV.tile([P, 512], f32, tag="vsg")
                    for j in range(8):
                        nc.tensor.matmul(ps_sg, a2T[:, j, tsl], wv_sig[:, j, :],
                                         start=(j == 0), stop=False)
                    for j in range(8):
                        nc.tensor.matmul(ps_sg, sg_nT[:, j, tsl], wv_mu2[:, j, :],
                                         start=False, stop=(j == 7))
                    nc.scalar.copy(v_sg_sb[:, tb, :], ps_sg)

        # ============ Phase B: attention (all operands already in SBUF) ============
        with ExitStack() as ctx:
            ep = ctx.enter_context(tc.tile_pool(name="ep", bufs=36))
            sb3 = ctx.enter_context(tc.tile_pool(name="sb3", bufs=6))
            outsb = ctx.enter_context(tc.tile_pool(name="outsb", bufs=4))
            smallB = ctx.enter_context(tc.tile_pool(name="smallB", bufs=4))
            onesB = ctx.enter_context(tc.tile_pool(name="onesB", bufs=1))
            psD = ctx.enter_context(tc.tile_pool(name="psD", bufs=2, space="PSUM"))
            psS2 = ctx.enter_context(tc.tile_pool(name="psS2", bufs=2, space="PSUM"))
            psAVm = ctx.enter_context(tc.tile_pool(name="psAVm", bufs=2, space="PSUM"))
            psAVs = ctx.enter_context(tc.tile_pool(name="psAVs", bufs=1, space="PSUM"))
            psDB = ctx.enter_context(tc.tile_pool(name="psDB", bufs=1, space="PSUM"))

            ones_blk = onesB.tile([P, P], f32)
            nc.vector.memset(ones_blk, 1.0)
            sc128 = onesB.tile([P, 1], f32)
            nc.vector.memset(sc128, SCALE)

            def pass1(hq, c):
                pr, hh = divmod(hq, 2)
                pb = (hq % 2) * 64
                qrb, krb = hq // 2, 4 + hq // 2
                vco = pr * 130 + hh * 65
                cs = slice(c * 512, (c + 1) * 512)
                av_mu = psAVm.tile([65, 512], f32, tag="avmu", name=f"avmu{hq}_{c}")
                e_ts = []
                for kb in range(8):
                    dots = psD.tile([P, 512], f32, tag="dots", name=f"dots{hq}_{c}_{kb}")
                    nc.tensor.matmul(dots,
                                     qk_mu_sb[pb:pb + 64, krb, kb * P:(kb + 1) * P],
                                     qk_mu_sb[pb:pb + 64, qrb, cs],
                                     start=True, stop=True)
                    e_t = ep.tile([P, 512], bf, tag="e", name=f"e{hq}_{c}_{kb}")
                    nc.scalar.activation(e_t, dots, AF.Exp, scale=sc128)
                    e_ts.append(e_t)
                    nc.tensor.matmul(av_mu, v_mu_sb[:, kb, vco:vco + 65], e_t,
                                     start=(kb == 0), stop=(kb == 7))
                r_sb = smallB.tile([P, 512], f32, tag="r", name=f"r{hq}_{c}")
                nc.vector.reciprocal(r_sb[64:65, :], av_mu[64:65, :])
                dbp = psDB.tile([P, 512], f32, tag="db", name=f"dbp{hq}_{c}")
                nc.tensor.matmul(dbp, ones_blk[64:65, :], r_sb[64:65, :], start=True, stop=True)
                db = sb3.tile([P, 512], f32, tag="db_sb", name=f"db{hq}_{c}")
                nc.vector.tensor_copy(db, dbp)
                muo = outsb.tile([64, 512], bf, tag="muo", name=f"muo{hq}_{c}")
                nc.vector.tensor_mul(muo, av_mu[0:64, :], db[0:64, :])
                nc.sync.dma_start(out=oT_mu_sb[pb:pb + 64, qrb, cs], in_=muo)
                return (hq, c, e_ts, db)

            def pass2(stateA, stateB):
                # both heads of a pair: sigma-AV matmuls col-packed via
                # tile_position (0,0)/(0,64) -> run concurrently on the PE,
                # and the packed [128,512] result evicts straight into the
                # contiguous oT_sg_sb slice (no partition-shift DMA).
                hqA, c, e_tsA, dbA = stateA
                hqB, _, e_tsB, dbB = stateB
                pr = hqA // 2
                qrb, krb = pr, 4 + pr
                cs = slice(c * 512, (c + 1) * 512)
                av2 = psAVs.tile([P, 512], f32, tag="avsg", name=f"avsg{hqA}_{c}")
                for kb in range(8):
                    for hq, pb, e_ts, db in ((hqA, 0, e_tsA, dbA), (hqB, 64, e_tsB, dbB)):
                        sdots = psS2.tile([P, 512], f32, tag="sdots", name=f"sd{hq}_{c}_{kb}")
                        nc.tensor.matmul(sdots,
                                         qk_sg_sb[pb:pb + 64, krb, kb * P:(kb + 1) * P],
                                         qk_sg_sb[pb:pb + 64, qrb, cs],
                                         start=True, stop=True)
                        p_t = sb3.tile([P, 512], f32, tag="p", name=f"p{hq}_{c}_{kb}")
                        nc.gpsimd.tensor_mul(p_t, e_ts[kb], db)
                        t_t = sb3.tile([P, 512], f32, tag="t", name=f"t{hq}_{c}_{kb}")
                        if kb % 2 == 0:
                            nc.vector.scalar_tensor_tensor(t_t, p_t, 1.0, p_t,
                                                           ALU.subtract, ALU.mult)
                        else:
                            m_t = sb3.tile([P, 512], f32, tag="m", name=f"m{hq}_{c}_{kb}")
                            nc.gpsimd.tensor_mul(m_t, p_t, p_t)
                            nc.gpsimd.tensor_sub(t_t, p_t, m_t)
                        u_t = sb3.tile([P, 512], f32, tag="u", name=f"u{hq}_{c}_{kb}")
                        if kb % 2 == 0:
                            nc.gpsimd.tensor_mul(u_t, t_t, t_t)
                        else:
                            nc.scalar.activation(u_t, t_t, AF.Square)
                        w_t = sb3.tile([P, 512], bf, tag="w", name=f"w{hq}_{c}_{kb}")
                        nc.vector.tensor_mul(w_t, u_t, sdots)
                        nc.tensor.matmul(av2[pb:pb + 64, :],
                                         v_sg_sb[:, kb, hq * 64:(hq + 1) * 64], w_t,
                                         start=(kb == 0), stop=(kb == 7),
                                         tile_position=(0, pb),
                                         skip_group_check=True)
                nc.vector.tensor_copy(oT_sg_sb[:, qrb, cs], av2)

            prev = None
            for pr in range(4):
                for c in range(2):
                    curA = pass1(2 * pr, c)
                    curB = pass1(2 * pr + 1, c)
                    if prev is not None:
                        pass2(*prev)
                    prev = (curA, curB)
            pass2(*prev)

        # ============ Phase C: out-projection ============
        with ExitStack() as ctx:
            wo = ctx.enter_context(tc.tile_pool(name="wo", bufs=1))
            oin = ctx.enter_context(tc.tile_pool(name="oin", bufs=1))
            evC = ctx.enter_context(tc.tile_pool(name="evC", bufs=4))
            psC = ctx.enter_context(tc.tile_pool(name="psC", bufs=2, space="PSUM"))

            wo_mu = wo.tile([P, 4, D], bf)
            nc.gpsimd.dma_start(out=wo_mu, in_=io["wo_mu"][:].rearrange("(j p) o -> p j o", p=P))
            wo_sr = wo.tile([P, 4, D], f32)
            nc.sync.dma_start(out=wo_sr, in_=io["wo_sr"][:].rearrange("(j p) o -> p j o", p=P))
            wo_sg0 = wo.tile([P, 4, D], f32)
            nc.scalar.activation(wo_sg0, wo_sr, AF.Exp)
            wo_sig = wo.tile([P, 4, D], bf)
            nc.scalar.activation(wo_sig, wo_sg0, AF.Ln, bias=1.0)
            wo_mu2 = wo.tile([P, 4, D], bf)
            nc.vector.tensor_mul(wo_mu2, wo_mu, wo_mu)

            a2o = oin.tile([P, 4, N], bf)
            zsq = oin.tile([P, 4, N], f32)
            nc.scalar.activation(zsq, oT_mu_sb, AF.Square)
            nc.vector.tensor_add(a2o, zsq, oT_sg_sb)

            for ob in range(8):
                osl = slice(ob * P, (ob + 1) * P)
                for c in range(2):
                    cs = slice(c * 512, (c + 1) * 512)
                    ps_mu = psC.tile([P, 512], f32, tag="ymu")
                    for j in range(4):
                        nc.tensor.matmul(ps_mu, wo_mu[:, j, osl], oT_mu_sb[:, j, cs],
                                         start=(j == 0), stop=(j == 3))
                    ev1 = evC.tile([P, 512], f32, tag="ev1")
                    nc.vector.tensor_copy(ev1, ps_mu)
                    nc.sync.dma_start(out=io["yT_mu"][osl, cs], in_=ev1)
                    ps_sg = psC.tile([P, 512], f32, tag="ysg")
                    for j in range(4):
                        nc.tensor.matmul(ps_sg, wo_sig[:, j, osl], a2o[:, j, cs],
                                         start=(j == 0), stop=False)
                    for j in range(4):
                        nc.tensor.matmul(ps_sg, wo_mu2[:, j, osl], oT_sg_sb[:, j, cs],
                                         start=False, stop=(j == 3))
                    ev2 = evC.tile([P, 512], f32, tag="ev2")
                    nc.scalar.copy(ev2, ps_sg)
                    nc.sync.dma_start(out=io["yT_sg"][osl, cs], in_=ev2)


def _get_nc():
    if "nc" not in _NC_CACHE:
        _NC_CACHE["nc"] = _build_nc()
    return _NC_CACHE["nc"]


def _prep_core_inputs(c, mu, sigma, ln_gamma, ln_beta, Wqkv_mu, Wqkv_sigma_raw,
                      Wout_mu, Wout_sigma_raw):
    f = np.float32
    asc = np.ascontiguousarray
    b, g = divmod(c, 2)
    qs = slice(512 * g, 512 * (g + 1))
    ks = slice(1024 + 512 * g, 1024 + 512 * (g + 1))
    vs = slice(2048 + 512 * g, 2048 + 512 * (g + 1))
    gb = np.zeros((P, 16), f)
    gb[:, :8] = np.asarray(ln_gamma, f).reshape(8, P).T
    gb[:, 8:] = np.asarray(ln_beta, f).reshape(8, P).T
    wqk_mu = np.concatenate([Wqkv_mu[qs], Wqkv_mu[ks]], 0)
    wqk_sr = np.concatenate([Wqkv_sigma_raw[qs], Wqkv_sigma_raw[ks]], 0)
    return {
        "muT": asc(np.asarray(mu[b], f).T),
        "sgT": asc(np.asarray(sigma[b], f).T),
        "gb": gb,
        "wqk_mu": asc(np.asarray(wqk_mu, f).T),
        "wqk_sr": asc(np.asarray(wqk_sr, f).T),
        "wv_mu": asc(np.asarray(Wqkv_mu[vs], f).T),
        "wv_sr": asc(np.asarray(Wqkv_sigma_raw[vs], f).T),
        "wo_mu": asc(np.asarray(Wout_mu[:, 512 * g:512 * (g + 1)], f).T),
        "wo_sr": asc(np.asarray(Wout_sigma_raw[:, 512 * g:512 * (g + 1)], f).T),
    }


def _emulate_core(m):
    """Pure-numpy mirror of the on-device program (for validation only)."""
    sp = lambda x: np.log1p(np.exp(x))
    muT, sgT = m["muT"], m["sgT"]
    gamma = m["gb"][:, :8].T.reshape(-1)[:, None]   # [D,1] indexed by d
    beta = m["gb"][:, 8:].T.reshape(-1)[:, None]
    mean = muT.mean(0, keepdims=True)
    var = muT.var(0, keepdims=True)
    inv = 1.0 / np.sqrt(var + EPS)
    mu_nT = (muT * inv - mean * inv) * gamma + beta
    sg_nT = sgT * gamma * gamma * inv * inv
    a2T = mu_nT * mu_nT + sg_nT
    qkT_mu = m["wqk_mu"].T @ mu_nT
    qkT_sg = sp(m["wqk_sr"]).T @ a2T + (m["wqk_mu"] ** 2).T @ sg_nT
    v_mu = mu_nT.T @ m["wv_mu"]
    v_sg = a2T.T @ sp(m["wv_sr"]) + sg_nT.T @ m["wv_mu"] ** 2
    oT_mu = np.zeros((RV, N), np.float32)
    oT_sg = np.zeros((RV, N), np.float32)
    for h in range(HPC):
        hs = slice(h * 64, (h + 1) * 64)
        sT = m_kT = qkT_mu[512 + h * 64:512 + (h + 1) * 64].T @ qkT_mu[hs]  # [kt, qt]
        e = np.exp(SCALE * sT)
        den = e.sum(0, keepdims=True)
        db = 1.0 / den
        p = e * db
        oT_mu[hs] = (v_mu[:, hs].T @ e) * db
        sdT = qkT_sg[512 + h * 64:512 + (h + 1) * 64].T @ qkT_sg[hs]
        t = (p - 1.0) * p
        w = (t * t) * SCALE * sdT
        oT_sg[hs] = v_sg[:, hs].T @ w
    a2o = oT_mu * oT_mu + oT_sg
    yT_mu = m["wo_mu"].T @ oT_mu
    yT_sg = sp(m["wo_sr"]).T @ a2o + (m["wo_mu"] ** 2).T @ oT_sg
    return yT_mu.astype(np.float32), yT_sg.astype(np.float32)


def kernel(mu, sigma, ln_gamma, ln_beta, Wqkv_mu, Wqkv_sigma_raw, Wout_mu,
           Wout_sigma_raw, _trace=False):
    from concourse.bass_utils import run_bass_kernel_spmd

    nc = _get_nc()
    args = (mu, sigma, ln_gamma, ln_beta, Wqkv_mu, Wqkv_sigma_raw, Wout_mu,
            Wout_sigma_raw)
    in_maps = [_prep_core_inputs(c, *args) for c in range(8)]
    res = run_bass_kernel_spmd(nc, in_maps, list(range(8)), trace=_trace)
    out_mu = np.zeros((B, N, D), np.float32)
    out_sg = np.zeros((B, N, D), np.float32)
    for c in range(8):
        b = c // 2
        out_mu[b] += res.results[c]["yT_mu"].T
        out_sg[b] += res.results[c]["yT_sg"].T
    if _trace:
        kernel._last_result = res
    return out_mu, out_sg


# revision 29
# speedup vs baseline: 207.0469x; 1.0340x over previous
"""VDP (variance-propagating) attention kernel for Trainium2, 8 NeuronCores.

Sharding: core c -> (batch b = c//2, head-group g = c%2) [8 heads each].
Each core computes LN + its QKV slice + attention for its 8 heads + the
partial out-projection for its 512 inner columns. Host sums the two
head-group partials per batch. No collectives needed.

Layout trick: everything on-device lives transposed as [feature, token]
(activations) / [contraction, out] (weights), prepared host-side, so the
contraction dim is always on partitions and no on-device transposes are
needed anywhere. LayerNorm stats (reduce over features = partitions) are
done with ones-vector matmuls on the PE; softmax denominators come for
free from a ones-augmented column in the V operand of the mu-attention AV
matmul, and are broadcast back across partitions with a K=1 PE matmul.
"""

import os
import sys

import numpy as np

for _p in ("/opt/trn_rl_repo", "/root/.axon_site/_ro/trn_rl_repo"):
    if os.path.isdir(_p) and _p not in sys.path:
        sys.path.insert(0, _p)

HEADS = 16
DH = 64
SCALE = DH ** -0.5
EPS = 1e-5
B, N, D = 4, 1024, 1024
HPC = 8          # heads per core
RQK = 1024       # q+k rows per core (2 * 8 heads * 64)
RV = 512         # v rows per core
P = 128

_NC_CACHE = {}


def _build_nc(tiny_out=False):
    import concourse.bass as bass  # noqa: F401
    import concourse.tile as tile
    from concourse import bacc, mybir

    f32 = mybir.dt.float32
    AF = mybir.ActivationFunctionType
    ALU = mybir.AluOpType

    nc = bacc.Bacc(None, target_bir_lowering=False)

    io = {}
    for name, shape in [
        ("muT", [D, N]), ("sgT", [D, N]), ("gb", [P, 16]),
        ("wqk_mu", [D, RQK]), ("wqk_sr", [D, RQK]),
        ("wv_mu", [D, RV]), ("wv_sr", [D, RV]),
        ("wo_mu", [RV, D]), ("wo_sr", [RV, D]),
    ]:
        io[name] = nc.dram_tensor(name, shape, f32, kind="ExternalInput")
    if tiny_out:
        for name, shape in [("yT_mu", [D, N]), ("yT_sg", [D, N])]:
            io[name] = nc.dram_tensor(name, shape, f32)
        io["done"] = nc.dram_tensor("done", [1, 16], f32, kind="ExternalOutput")
    else:
        for name, shape in [("yT_mu", [D, N]), ("yT_sg", [D, N])]:
            io[name] = nc.dram_tensor(name, shape, f32, kind="ExternalOutput")
    # internal DRAM staging
    bf = mybir.dt.bfloat16

    with tile.TileContext(nc) as tc:
        _emit(nc, tc, io, f32, bf, AF, ALU)
        if tiny_out:
            with tc.tile_pool(name="doneP", bufs=1) as dp:
                dt = dp.tile([1, 16], f32)
                nc.vector.memset(dt, 1.0)
                nc.sync.dma_start(out=io["done"][:], in_=dt)
    nc.compile()
    return nc


def _build_floor_nc():
    import concourse.tile as tile
    from concourse import bacc, mybir

    f32 = mybir.dt.float32
    nc = bacc.Bacc(None, target_bir_lowering=False)
    done = nc.dram_tensor("done", [1, 16], f32, kind="ExternalOutput")
    with tile.TileContext(nc) as tc:
        with tc.tile_pool(name="dp", bufs=1) as dp:
            dt = dp.tile([1, 16], f32)
            nc.vector.memset(dt, 1.0)
            nc.sync.dma_start(out=done[:], in_=dt)
    nc.compile()
    return nc


def _emit(nc, tc, io, f32, bf, AF, ALU):
    from contextlib import ExitStack

    with ExitStack() as tctx:
        stage = tctx.enter_context(tc.tile_pool(name="stage", bufs=1))
        # persistent SBUF staging (bf16): no DRAM round trips between phases
        qk_mu_sb = stage.tile([P, 8, N], bf)    # rows: 0-3 q-blocks, 4-7 k-blocks
        qk_sg_sb = stage.tile([P, 8, N], bf)
        v_mu_sb = stage.tile([P, 8, HPC * 65], bf)   # per tok-block: 8 heads x (64 v + ones)
        v_sg_sb = stage.tile([P, 8, RV], bf)
        oT_mu_sb = stage.tile([P, 4, N], bf)
        oT_sg_sb = stage.tile([P, 4, N], bf)

        # ============ Phase A: LayerNorm + QKV ============
        with ExitStack() as actx:
            acts = actx.enter_context(tc.tile_pool(name="acts", bufs=1))
            smallA = actx.enter_context(tc.tile_pool(name="smallA", bufs=1))

            gb_sb = smallA.tile([P, 16], f32)
            nc.sync.dma_start(out=gb_sb, in_=io["gb"][:])
            g2_sb = smallA.tile([P, 8], f32)
            nc.vector.tensor_mul(g2_sb, gb_sb[:, 0:8], gb_sb[:, 0:8])
            ones_col = smallA.tile([P, 1], f32)
            nc.vector.memset(ones_col, 1.0)
            ones_row = smallA.tile([1, P], f32)
            nc.vector.memset(ones_row, 1.0)
            eps1 = smallA.tile([1, 1], f32)
            nc.vector.memset(eps1, EPS)
            scA = smallA.tile([P, 1], f32)
            nc.vector.memset(scA, SCALE)

            inv_b = acts.tile([P, N], f32)
            minv_b = acts.tile([P, N], f32)
            inv2_b = acts.tile([P, N], f32)
            mu_nT = acts.tile([P, 8, N], bf)
            sg_nT = acts.tile([P, 8, N], bf)
            a2T = acts.tile([P, 8, N], bf)

            # --- A1: stats + normalize (muT streamed twice, not resident) ---
            with ExitStack() as ctx:
                ioA = ctx.enter_context(tc.tile_pool(name="ioA", bufs=2))
                psS = ctx.enter_context(tc.tile_pool(name="psS", bufs=1, space="PSUM"))
                psA = ctx.enter_context(tc.tile_pool(name="psA", bufs=2, space="PSUM"))

                sum_ps = [psS.tile([1, 512], f32, tag=f"sum{c}", name=f"sum{c}") for c in range(2)]
                sq_ps = [psS.tile([1, 512], f32, tag=f"sq{c}", name=f"sq{c}") for c in range(2)]
                for j in range(8):
                    mut = ioA.tile([P, N], f32, tag="mut")
                    nc.sync.dma_start(out=mut, in_=io["muT"][j * P:(j + 1) * P, :])
                    mu2 = ioA.tile([P, N], f32, tag="mu2")
                    nc.gpsimd.tensor_mul(mu2, mut, mut)
                    for c in range(2):
                        cs = slice(c * 512, (c + 1) * 512)
                        nc.tensor.matmul(sum_ps[c], ones_col, mut[:, cs],
                                         start=(j == 0), stop=(j == 7), skip_group_check=True)
                        nc.tensor.matmul(sq_ps[c], ones_col, mu2[:, cs],
                                         start=(j == 0), stop=(j == 7), skip_group_check=True)

                inv_sb = smallA.tile([1, N], f32)
                minv_sb = smallA.tile([1, N], f32)
                for c in range(2):
                    cs = slice(c * 512, (c + 1) * 512)
                    mean_t = ioA.tile([1, 512], f32, tag="mean")
                    nc.vector.tensor_scalar_mul(mean_t, sum_ps[c], 1.0 / D)
                    m2_t = ioA.tile([1, 512], f32, tag="m2")
                    nc.vector.tensor_mul(m2_t, mean_t, mean_t)
                    var_t = ioA.tile([1, 512], f32, tag="var")
                    nc.vector.scalar_tensor_tensor(var_t, sq_ps[c], 1.0 / D, m2_t,
                                                   ALU.mult, ALU.subtract)
                    std_t = ioA.tile([1, 512], f32, tag="std")
                    nc.scalar.activation(std_t, var_t, AF.Sqrt, bias=eps1)
                    nc.vector.reciprocal(inv_sb[:, cs], std_t)
                    nc.vector.scalar_tensor_tensor(minv_sb[:, cs], mean_t, -1.0, inv_sb[:, cs],
                                                   ALU.mult, ALU.mult)

                for c in range(2):
                    cs = slice(c * 512, (c + 1) * 512)
                    bp1 = psA.tile([P, 512], f32, tag="bcast")
                    nc.tensor.matmul(bp1, ones_row, inv_sb[:, cs], start=True, stop=True)
                    nc.vector.tensor_copy(inv_b[:, cs], bp1)
                    bp2 = psA.tile([P, 512], f32, tag="bcast")
                    nc.tensor.matmul(bp2, ones_row, minv_sb[:, cs], start=True, stop=True)
                    nc.vector.tensor_copy(minv_b[:, cs], bp2)
                nc.vector.tensor_mul(inv2_b, inv_b, inv_b)

                for j in range(8):
                    mut = ioA.tile([P, N], f32, tag="mut")
                    nc.sync.dma_start(out=mut, in_=io["muT"][j * P:(j + 1) * P, :])
                    x2 = ioA.tile([P, N], f32, tag="x2")
                    nc.vector.tensor_mul(x2, mut, inv_b)
                    nc.vector.tensor_add(x2, x2, minv_b)
                    nc.vector.tensor_scalar(mu_nT[:, j, :], x2, gb_sb[:, j:j + 1],
                                            gb_sb[:, 8 + j:9 + j], ALU.mult, ALU.add)
                    sgt = ioA.tile([P, N], f32, tag="sgt")
                    nc.sync.dma_start(out=sgt, in_=io["sgT"][j * P:(j + 1) * P, :])
                    nc.vector.scalar_tensor_tensor(sg_nT[:, j, :], sgt, g2_sb[:, j:j + 1],
                                                   inv2_b, ALU.mult, ALU.mult)
                    z = ioA.tile([P, N], f32, tag="z")
                    nc.gpsimd.tensor_mul(z, mu_nT[:, j, :], mu_nT[:, j, :])
                    nc.gpsimd.tensor_add(a2T[:, j, :], z, sg_nT[:, j, :])

            # --- A2a: QKV q,k rows (transposed out), evict straight to SBUF stage ---
            with ExitStack() as ctx:
                wq = ctx.enter_context(tc.tile_pool(name="wq", bufs=2))
                psQ = ctx.enter_context(tc.tile_pool(name="psQ", bufs=2, space="PSUM"))
                for rb in range(8):
                    rsl = slice(rb * P, (rb + 1) * P)
                    wmu = wq.tile([P, 8, P], bf, tag="wmu")
                    nc.gpsimd.dma_start(out=wmu, in_=io["wqk_mu"][:, rsl].rearrange("(j p) r -> p j r", p=P))
                    wsr = wq.tile([P, 8, P], f32, tag="wsr")
                    nc.sync.dma_start(out=wsr, in_=io["wqk_sr"][:, rsl].rearrange("(j p) r -> p j r", p=P))
                    wsg0 = wq.tile([P, 8, P], f32, tag="wsg0")
                    nc.scalar.activation(wsg0, wsr, AF.Exp)
                    wsig = wq.tile([P, 8, P], bf, tag="wsig")
                    nc.scalar.activation(wsig, wsg0, AF.Ln, bias=1.0)
                    wmu2 = wq.tile([P, 8, P], bf, tag="wmu2")
                    nc.vector.tensor_mul(wmu2, wmu, wmu)
                    for c in range(2):
                        cs = slice(c * 512, (c + 1) * 512)
                        ps_mu = psQ.tile([P, 512], f32, tag="qkmu")
                        for j in range(8):
                            nc.tensor.matmul(ps_mu, wmu[:, j, :], mu_nT[:, j, cs],
                                             start=(j == 0), stop=(j == 7))
                        nc.vector.tensor_copy(qk_mu_sb[:, rb, cs], ps_mu)
                        ps_sg = psQ.tile([P, 512], f32, tag="qksg")
                        for j in range(8):
                            nc.tensor.matmul(ps_sg, wsig[:, j, :], a2T[:, j, cs],
                                             start=(j == 0), stop=False)
                        for j in range(8):
                            nc.tensor.matmul(ps_sg, wmu2[:, j, :], sg_nT[:, j, cs],
                                             start=False, stop=(j == 7))
                        if rb < 4:
                            nc.scalar.activation(qk_sg_sb[:, rb, cs], ps_sg, AF.Copy, scale=scA)
                        else:
                            nc.scalar.copy(qk_sg_sb[:, rb, cs], ps_sg)

            # --- A2b: V (natural layout), evict straight to SBUF stage ---
            with ExitStack() as ctx:
                wv = ctx.enter_context(tc.tile_pool(name="wv", bufs=1))
                psV = ctx.enter_context(tc.tile_pool(name="psV", bufs=2, space="PSUM"))
                wv_mu = wv.tile([P, 8, 512], bf)
                nc.gpsimd.dma_start(out=wv_mu, in_=io["wv_mu"][:].rearrange("(j p) r -> p j r", p=P))
                wv_sr = wv.tile([P, 8, 512], f32)
                nc.sync.dma_start(out=wv_sr, in_=io["wv_sr"][:].rearrange("(j p) r -> p j r", p=P))
                wv_sg0 = wv.tile([P, 8, 512], f32)
                nc.scalar.activation(wv_sg0, wv_sr, AF.Exp)
                wv_sig = wv.tile([P, 8, 512], bf)
                nc.scalar.activation(wv_sig, wv_sg0, AF.Ln, bias=1.0)
                wv_mu2 = wv.tile([P, 8, 512], bf)
                nc.vector.tensor_mul(wv_mu2, wv_mu, wv_mu)
                for tb in range(8):
                    tsl = slice(tb * P, (tb + 1) * P)
                    ps_mu = psV.tile([P, 512], f32, tag="vmu")
                    for j in range(8):
                        nc.tensor.matmul(ps_mu, mu_nT[:, j, tsl], wv_mu[:, j, :],
                                         start=(j == 0), stop=(j == 7))
                    nc.vector.memset(v_mu_sb[:, tb, :], 1.0)
                    nc.vector.tensor_copy(
                        v_mu_sb[:, tb, :].rearrange("p (h c) -> p h c", c=65)[:, :, 0:64],
                        ps_mu.rearrange("p (h c) -> p h c", c=64))
                    ps_sg = psV.tile([P, 512], f32, tag="vsg")
                    for j in range(8):
                        nc.tensor.matmul(ps_sg, a2T[:, j, tsl], wv_sig[:, j, :],
                                         start=(j == 0), stop=False)
                    for j in range(8):
                        nc.tensor.matmul(ps_sg, sg_nT[:, j, tsl], wv_mu2[:, j, :],
                                         start=False, stop=(j == 7))
                    nc.scalar.copy(v_sg_sb[:, tb, :], ps_sg)

        # ============ Phase B: attention (all operands already in SBUF) ============
        with ExitStack() as ctx:
            ep = ctx.enter_context(tc.tile_pool(name="ep", bufs=36))
            sb3 = ctx.enter_context(tc.tile_pool(name="sb3", bufs=6))
            outsb = ctx.enter_context(tc.tile_pool(name="outsb", bufs=4))
            smallB = ctx.enter_context(tc.tile_pool(name="smallB", bufs=4))
            onesB = ctx.enter_context(tc.tile_pool(name="onesB", bufs=1))
            psD = ctx.enter_context(tc.tile_pool(name="psD", bufs=2, space="PSUM"))
            psS2 = ctx.enter_context(tc.tile_pool(name="psS2", bufs=2, space="PSUM"))
            psAVm = ctx.enter_context(tc.tile_pool(name="psAVm", bufs=2, space="PSUM"))
            psAVs = ctx.enter_context(tc.tile_pool(name="psAVs", bufs=1, space="PSUM"))
            psDB = ctx.enter_context(tc.tile_pool(name="psDB", bufs=1, space="PSUM"))

            ones_blk = onesB.tile([P, P], f32)
            nc.vector.memset(ones_blk, 1.0)
            sc128 = onesB.tile([P, 1], f32)
            nc.vector.memset(sc128, SCALE)

            def pass1(hq, c):
                pr, hh = divmod(hq, 2)
                pb = (hq % 2) * 64
                qrb, krb = hq // 2, 4 + hq // 2
                vco = pr * 130 + hh * 65
                cs = slice(c * 512, (c + 1) * 512)
                av_mu = psAVm.tile([65, 512], f32, tag="avmu", name=f"avmu{hq}_{c}")
                e_ts = []
                for kb in range(8):
                    dots = psD.tile([P, 512], f32, tag="dots", name=f"dots{hq}_{c}_{kb}")
                    nc.tensor.matmul(dots,
                                     qk_mu_sb[pb:pb + 64, krb, kb * P:(kb + 1) * P],
                                     qk_mu_sb[pb:pb + 64, qrb, cs],
                                     start=True, stop=True)
                    e_t = ep.tile([P, 512], bf, tag="e", name=f"e{hq}_{c}_{kb}")
                    nc.scalar.activation(e_t, dots, AF.Exp, scale=sc128)
                    e_ts.append(e_t)
                    nc.tensor.matmul(av_mu, v_mu_sb[:, kb, vco:vco + 65], e_t,
                                     start=(kb == 0), stop=(kb == 7))
                r_sb = smallB.tile([P, 512], f32, tag="r", name=f"r{hq}_{c}")
                nc.vector.reciprocal(r_sb[64:65, :], av_mu[64:65, :])
                dbp = psDB.tile([P, 512], f32, tag="db", name=f"dbp{hq}_{c}")
                nc.tensor.matmul(dbp, ones_blk[64:65, :], r_sb[64:65, :], start=True, stop=True)
                db = sb3.tile([P, 512], f32, tag="db_sb", name=f"db{hq}_{c}")
                nc.scalar.copy(db, dbp)
                muo = outsb.tile([64, 512], bf, tag="muo", name=f"muo{hq}_{c}")
                nc.vector.tensor_mul(muo, av_mu[0:64, :], db[0:64, :])
                nc.sync.dma_start(out=oT_mu_sb[pb:pb + 64, qrb, cs], in_=muo)
                return (hq, c, e_ts, db)

            def pass2(stateA, stateB):
                # both heads of a pair: sigma-AV matmuls col-packed via
                # tile_position (0,0)/(0,64) -> run concurrently on the PE,
                # and the packed [128,512] result evicts straight into the
                # contiguous oT_sg_sb slice (no partition-shift DMA).
                hqA, c, e_tsA, dbA = stateA
                hqB, _, e_tsB, dbB = stateB
                pr = hqA // 2
                qrb, krb = pr, 4 + pr
                cs = slice(c * 512, (c + 1) * 512)
                av2 = psAVs.tile([P, 512], f32, tag="avsg", name=f"avsg{hqA}_{c}")
                for kb in range(8):
                    for hq, pb, e_ts, db in ((hqA, 0, e_tsA, dbA), (hqB, 64, e_tsB, dbB)):
                        sdots = psS2.tile([P, 512], f32, tag="sdots", name=f"sd{hq}_{c}_{kb}")
                        nc.tensor.matmul(sdots,
                                         qk_sg_sb[pb:pb + 64, krb, kb * P:(kb + 1) * P],
                                         qk_sg_sb[pb:pb + 64, qrb, cs],
                                         start=True, stop=True)
                        p_t = sb3.tile([P, 512], f32, tag="p", name=f"p{hq}_{c}_{kb}")
                        nc.gpsimd.tensor_mul(p_t, e_ts[kb], db)
                        t_t = sb3.tile([P, 512], f32, tag="t", name=f"t{hq}_{c}_{kb}")
                        if kb % 2 == 0:
                            nc.vector.scalar_tensor_tensor(t_t, p_t, 1.0, p_t,
                                                           ALU.subtract, ALU.mult)
                        else:
                            m_t = sb3.tile([P, 512], f32, tag="m", name=f"m{hq}_{c}_{kb}")
                            nc.gpsimd.tensor_mul(m_t, p_t, p_t)
                            nc.gpsimd.tensor_sub(t_t, p_t, m_t)
                        u_t = sb3.tile([P, 512], f32, tag="u", name=f"u{hq}_{c}_{kb}")
                        if kb % 2 == 0:
                            nc.gpsimd.tensor_mul(u_t, t_t, t_t)
                        else:
                            nc.scalar.activation(u_t, t_t, AF.Square)
                        w_t = sb3.tile([P, 512], bf, tag="w", name=f"w{hq}_{c}_{kb}")
                        nc.vector.tensor_mul(w_t, u_t, sdots)
                        nc.tensor.matmul(av2[pb:pb + 64, :],
                                         v_sg_sb[:, kb, hq * 64:(hq + 1) * 64], w_t,
                                         start=(kb == 0), stop=(kb == 7),
                                         tile_position=(0, pb),
                                         skip_group_check=True)
                nc.scalar.copy(oT_sg_sb[:, qrb, cs], av2)

            prev = None
            for pr in range(4):
                for c in range(2):
                    curA = pass1(2 * pr, c)
                    curB = pass1(2 * pr + 1, c)
                    if prev is not None:
                        pass2(*prev)
                    prev = (curA, curB)
            pass2(*prev)

        # ============ Phase C: out-projection ============
        with ExitStack() as ctx:
            wo = ctx.enter_context(tc.tile_pool(name="wo", bufs=1))
            oin = ctx.enter_context(tc.tile_pool(name="oin", bufs=1))
            evC = ctx.enter_context(tc.tile_pool(name="evC", bufs=4))
            psC = ctx.enter_context(tc.tile_pool(name="psC", bufs=2, space="PSUM"))

            wo_mu = wo.tile([P, 4, D], bf)
            nc.gpsimd.dma_start(out=wo_mu, in_=io["wo_mu"][:].rearrange("(j p) o -> p j o", p=P))
            wo_sr = wo.tile([P, 4, D], f32)
            nc.sync.dma_start(out=wo_sr, in_=io["wo_sr"][:].rearrange("(j p) o -> p j o", p=P))
            wo_sg0 = wo.tile([P, 4, D], f32)
            nc.scalar.activation(wo_sg0, wo_sr, AF.Exp)
            wo_sig = wo.tile([P, 4, D], bf)
            nc.scalar.activation(wo_sig, wo_sg0, AF.Ln, bias=1.0)
            wo_mu2 = wo.tile([P, 4, D], bf)
            nc.vector.tensor_mul(wo_mu2, wo_mu, wo_mu)

            a2o = oin.tile([P, 4, N], bf)
            zsq = oin.tile([P, 4, N], f32)
            nc.scalar.activation(zsq, oT_mu_sb, AF.Square)
            nc.vector.tensor_add(a2o, zsq, oT_sg_sb)

            for ob in range(8):
                osl = slice(ob * P, (ob + 1) * P)
                for c in range(2):
                    cs = slice(c * 512, (c + 1) * 512)
                    ps_mu = psC.tile([P, 512], f32, tag="ymu")
                    for j in range(4):
                        nc.tensor.matmul(ps_mu, wo_mu[:, j, osl], oT_mu_sb[:, j, cs],
                                         start=(j == 0), stop=(j == 3))
                    ev1 = evC.tile([P, 512], f32, tag="ev1")
                    nc.vector.tensor_copy(ev1, ps_mu)
                    nc.sync.dma_start(out=io["yT_mu"][osl, cs], in_=ev1)
                    ps_sg = psC.tile([P, 512], f32, tag="ysg")
                    for j in range(4):
                        nc.tensor.matmul(ps_sg, wo_sig[:, j, osl], a2o[:, j, cs],
                                         start=(j == 0), stop=False)
                    for j in range(4):
                        nc.tensor.matmul(ps_sg, wo_mu2[:, j, osl], oT_sg_sb[:, j, cs],
                                         start=False, stop=(j == 3))
                    ev2 = evC.tile([P, 512], f32, tag="ev2")
                    nc.scalar.copy(ev2, ps_sg)
                    nc.sync.dma_start(out=io["yT_sg"][osl, cs], in_=ev2)


def _get_nc():
    if "nc" not in _NC_CACHE:
        _NC_CACHE["nc"] = _build_nc()
    return _NC_CACHE["nc"]


def _prep_core_inputs(c, mu, sigma, ln_gamma, ln_beta, Wqkv_mu, Wqkv_sigma_raw,
                      Wout_mu, Wout_sigma_raw):
    f = np.float32
    asc = np.ascontiguousarray
    b, g = divmod(c, 2)
    qs = slice(512 * g, 512 * (g + 1))
    ks = slice(1024 + 512 * g, 1024 + 512 * (g + 1))
    vs = slice(2048 + 512 * g, 2048 + 512 * (g + 1))
    gb = np.zeros((P, 16), f)
    gb[:, :8] = np.asarray(ln_gamma, f).reshape(8, P).T
    gb[:, 8:] = np.asarray(ln_beta, f).reshape(8, P).T
    wqk_mu = np.concatenate([Wqkv_mu[qs], Wqkv_mu[ks]], 0)
    wqk_sr = np.concatenate([Wqkv_sigma_raw[qs], Wqkv_sigma_raw[ks]], 0)
    return {
        "muT": asc(np.asarray(mu[b], f).T),
        "sgT": asc(np.asarray(sigma[b], f).T),
        "gb": gb,
        "wqk_mu": asc(np.asarray(wqk_mu, f).T),
        "wqk_sr": asc(np.asarray(wqk_sr, f).T),
        "wv_mu": asc(np.asarray(Wqkv_mu[vs], f).T),
        "wv_sr": asc(np.asarray(Wqkv_sigma_raw[vs], f).T),
        "wo_mu": asc(np.asarray(Wout_mu[:, 512 * g:512 * (g + 1)], f).T),
        "wo_sr": asc(np.asarray(Wout_sigma_raw[:, 512 * g:512 * (g + 1)], f).T),
    }


def _emulate_core(m):
    """Pure-numpy mirror of the on-device program (for validation only)."""
    sp = lambda x: np.log1p(np.exp(x))
    muT, sgT = m["muT"], m["sgT"]
    gamma = m["gb"][:, :8].T.reshape(-1)[:, None]   # [D,1] indexed by d
    beta = m["gb"][:, 8:].T.reshape(-1)[:, None]
    mean = muT.mean(0, keepdims=True)
    var = muT.var(0, keepdims=True)
    inv = 1.0 / np.sqrt(var + EPS)
    mu_nT = (muT * inv - mean * inv) * gamma + beta
    sg_nT = sgT * gamma * gamma * inv * inv
    a2T = mu_nT * mu_nT + sg_nT
    qkT_mu = m["wqk_mu"].T @ mu_nT
    qkT_sg = sp(m["wqk_sr"]).T @ a2T + (m["wqk_mu"] ** 2).T @ sg_nT
    v_mu = mu_nT.T @ m["wv_mu"]
    v_sg = a2T.T @ sp(m["wv_sr"]) + sg_nT.T @ m["wv_mu"] ** 2
    oT_mu = np.zeros((RV, N), np.float32)
    oT_sg = np.zeros((RV, N), np.float32)
    for h in range(HPC):
        hs = slice(h * 64, (h + 1) * 64)
        sT = m_kT = qkT_mu[512 + h * 64:512 + (h + 1) * 64].T @ qkT_mu[hs]  # [kt, qt]
        e = np.exp(SCALE * sT)
        den = e.sum(0, keepdims=True)
        db = 1.0 / den
        p = e * db
        oT_mu[hs] = (v_mu[:, hs].T @ e) * db
        sdT = qkT_sg[512 + h * 64:512 + (h + 1) * 64].T @ qkT_sg[hs]
        t = (p - 1.0) * p
        w = (t * t) * SCALE * sdT
        oT_sg[hs] = v_sg[:, hs].T @ w
    a2o = oT_mu * oT_mu + oT_sg
    yT_mu = m["wo_mu"].T @ oT_mu
    yT_sg = sp(m["wo_sr"]).T @ a2o + (m["wo_mu"] ** 2).T @ oT_sg
    return yT_mu.astype(np.float32), yT_sg.astype(np.float32)


def kernel(mu, sigma, ln_gamma, ln_beta, Wqkv_mu, Wqkv_sigma_raw, Wout_mu,
           Wout_sigma_raw, _trace=False):
    from concourse.bass_utils import run_bass_kernel_spmd

    nc = _get_nc()
    args = (mu, sigma, ln_gamma, ln_beta, Wqkv_mu, Wqkv_sigma_raw, Wout_mu,
            Wout_sigma_raw)
    in_maps = [_prep_core_inputs(c, *args) for c in range(8)]
    res = run_bass_kernel_spmd(nc, in_maps, list(range(8)), trace=_trace)
    out_mu = np.zeros((B, N, D), np.float32)
    out_sg = np.zeros((B, N, D), np.float32)
    for c in range(8):
        b = c // 2
        out_mu[b] += res.results[c]["yT_mu"].T
        out_sg[b] += res.results[c]["yT_sg"].T
    if _trace:
        kernel._last_result = res
    return out_mu, out_sg


# revision 30
# speedup vs baseline: 212.0798x; 1.0243x over previous
"""VDP (variance-propagating) attention kernel for Trainium2, 8 NeuronCores.

Sharding: core c -> (batch b = c//2, head-group g = c%2) [8 heads each].
Each core computes LN + its QKV slice + attention for its 8 heads + the
partial out-projection for its 512 inner columns. Host sums the two
head-group partials per batch. No collectives needed.

Layout trick: everything on-device lives transposed as [feature, token]
(activations) / [contraction, out] (weights), prepared host-side, so the
contraction dim is always on partitions and no on-device transposes are
needed anywhere. LayerNorm stats (reduce over features = partitions) are
done with ones-vector matmuls on the PE; softmax denominators come for
free from a ones-augmented column in the V operand of the mu-attention AV
matmul, and are broadcast back across partitions with a K=1 PE matmul.
"""

import os
import sys

import numpy as np

for _p in ("/opt/trn_rl_repo", "/root/.axon_site/_ro/trn_rl_repo"):
    if os.path.isdir(_p) and _p not in sys.path:
        sys.path.insert(0, _p)

HEADS = 16
DH = 64
SCALE = DH ** -0.5
EPS = 1e-5
B, N, D = 4, 1024, 1024
HPC = 8          # heads per core
RQK = 1024       # q+k rows per core (2 * 8 heads * 64)
RV = 512         # v rows per core
P = 128

_NC_CACHE = {}


def _build_nc(tiny_out=False):
    import concourse.bass as bass  # noqa: F401
    import concourse.tile as tile
    from concourse import bacc, mybir

    f32 = mybir.dt.float32
    AF = mybir.ActivationFunctionType
    ALU = mybir.AluOpType

    nc = bacc.Bacc(None, target_bir_lowering=False)

    io = {}
    for name, shape in [
        ("muT", [D, N]), ("sgT", [D, N]), ("gb", [P, 16]),
        ("wqk_mu", [D, RQK]), ("wqk_sr", [D, RQK]),
        ("wv_mu", [D, RV]), ("wv_sr", [D, RV]),
        ("wo_mu", [RV, D]), ("wo_sr", [RV, D]),
    ]:
        io[name] = nc.dram_tensor(name, shape, f32, kind="ExternalInput")
    if tiny_out:
        for name, shape in [("yT_mu", [D, N]), ("yT_sg", [D, N])]:
            io[name] = nc.dram_tensor(name, shape, f32)
        io["done"] = nc.dram_tensor("done", [1, 16], f32, kind="ExternalOutput")
    else:
        for name, shape in [("yT_mu", [D, N]), ("yT_sg", [D, N])]:
            io[name] = nc.dram_tensor(name, shape, f32, kind="ExternalOutput")
    # internal DRAM staging
    bf = mybir.dt.bfloat16

    with tile.TileContext(nc) as tc:
        _emit(nc, tc, io, f32, bf, AF, ALU)
        if tiny_out:
            with tc.tile_pool(name="doneP", bufs=1) as dp:
                dt = dp.tile([1, 16], f32)
                nc.vector.memset(dt, 1.0)
                nc.sync.dma_start(out=io["done"][:], in_=dt)
    nc.compile()
    return nc


def _build_floor_nc():
    import concourse.tile as tile
    from concourse import bacc, mybir

    f32 = mybir.dt.float32
    nc = bacc.Bacc(None, target_bir_lowering=False)
    done = nc.dram_tensor("done", [1, 16], f32, kind="ExternalOutput")
    with tile.TileContext(nc) as tc:
        with tc.tile_pool(name="dp", bufs=1) as dp:
            dt = dp.tile([1, 16], f32)
            nc.vector.memset(dt, 1.0)
            nc.sync.dma_start(out=done[:], in_=dt)
    nc.compile()
    return nc


def _emit(nc, tc, io, f32, bf, AF, ALU):
    from contextlib import ExitStack

    with ExitStack() as tctx:
        stage = tctx.enter_context(tc.tile_pool(name="stage", bufs=1))
        # persistent SBUF staging (bf16): no DRAM round trips between phases
        qk_mu_sb = stage.tile([P, 8, N], bf)    # rows: 0-3 q-blocks, 4-7 k-blocks
        qk_sg_sb = stage.tile([P, 8, N], bf)
        v_mu_sb = stage.tile([P, 8, HPC * 65], bf)   # per tok-block: 8 heads x (64 v + ones)
        v_sg_sb = stage.tile([P, 8, RV], bf)
        oT_mu_sb = stage.tile([P, 4, N], bf)
        oT_sg_sb = stage.tile([P, 4, N], bf)

        # ============ Phase A: LayerNorm + QKV ============
        with ExitStack() as actx:
            acts = actx.enter_context(tc.tile_pool(name="acts", bufs=1))
            smallA = actx.enter_context(tc.tile_pool(name="smallA", bufs=1))

            gb_sb = smallA.tile([P, 16], f32)
            nc.sync.dma_start(out=gb_sb, in_=io["gb"][:])
            g2_sb = smallA.tile([P, 8], f32)
            nc.vector.tensor_mul(g2_sb, gb_sb[:, 0:8], gb_sb[:, 0:8])
            ones_col = smallA.tile([P, 1], f32)
            nc.vector.memset(ones_col, 1.0)
            ones_row = smallA.tile([1, P], f32)
            nc.vector.memset(ones_row, 1.0)
            eps1 = smallA.tile([1, 1], f32)
            nc.vector.memset(eps1, EPS)
            scA = smallA.tile([P, 1], f32)
            nc.vector.memset(scA, SCALE)

            inv_b = acts.tile([P, N], f32)
            minv_b = acts.tile([P, N], f32)
            inv2_b = acts.tile([P, N], f32)
            mu_nT = acts.tile([P, 8, N], bf)
            sg_nT = acts.tile([P, 8, N], bf)
            a2T = acts.tile([P, 8, N], bf)

            # --- A1: stats + normalize (muT streamed twice, not resident) ---
            with ExitStack() as ctx:
                ioA = ctx.enter_context(tc.tile_pool(name="ioA", bufs=2))
                psS = ctx.enter_context(tc.tile_pool(name="psS", bufs=1, space="PSUM"))
                psA = ctx.enter_context(tc.tile_pool(name="psA", bufs=2, space="PSUM"))

                sum_ps = [psS.tile([1, 512], f32, tag=f"sum{c}", name=f"sum{c}") for c in range(2)]
                sq_ps = [psS.tile([1, 512], f32, tag=f"sq{c}", name=f"sq{c}") for c in range(2)]
                for j in range(8):
                    mut = ioA.tile([P, N], f32, tag="mut")
                    nc.sync.dma_start(out=mut, in_=io["muT"][j * P:(j + 1) * P, :])
                    mu2 = ioA.tile([P, N], f32, tag="mu2")
                    nc.gpsimd.tensor_mul(mu2, mut, mut)
                    for c in range(2):
                        cs = slice(c * 512, (c + 1) * 512)
                        nc.tensor.matmul(sum_ps[c], ones_col, mut[:, cs],
                                         start=(j == 0), stop=(j == 7), skip_group_check=True)
                        nc.tensor.matmul(sq_ps[c], ones_col, mu2[:, cs],
                                         start=(j == 0), stop=(j == 7), skip_group_check=True)

                inv_sb = smallA.tile([1, N], f32)
                minv_sb = smallA.tile([1, N], f32)
                for c in range(2):
                    cs = slice(c * 512, (c + 1) * 512)
                    mean_t = ioA.tile([1, 512], f32, tag="mean")
                    nc.vector.tensor_scalar_mul(mean_t, sum_ps[c], 1.0 / D)
                    m2_t = ioA.tile([1, 512], f32, tag="m2")
                    nc.vector.tensor_mul(m2_t, mean_t, mean_t)
                    var_t = ioA.tile([1, 512], f32, tag="var")
                    nc.vector.scalar_tensor_tensor(var_t, sq_ps[c], 1.0 / D, m2_t,
                                                   ALU.mult, ALU.subtract)
                    std_t = ioA.tile([1, 512], f32, tag="std")
                    nc.scalar.activation(std_t, var_t, AF.Sqrt, bias=eps1)
                    nc.vector.reciprocal(inv_sb[:, cs], std_t)
                    nc.vector.scalar_tensor_tensor(minv_sb[:, cs], mean_t, -1.0, inv_sb[:, cs],
                                                   ALU.mult, ALU.mult)

                for c in range(2):
                    cs = slice(c * 512, (c + 1) * 512)
                    bp1 = psA.tile([P, 512], f32, tag="bcast")
                    nc.tensor.matmul(bp1, ones_row, inv_sb[:, cs], start=True, stop=True)
                    nc.scalar.copy(inv_b[:, cs], bp1)
                    bp2 = psA.tile([P, 512], f32, tag="bcast")
                    nc.tensor.matmul(bp2, ones_row, minv_sb[:, cs], start=True, stop=True)
                    nc.scalar.copy(minv_b[:, cs], bp2)
                nc.vector.tensor_mul(inv2_b, inv_b, inv_b)

                for j in range(8):
                    mut = ioA.tile([P, N], f32, tag="mut")
                    nc.sync.dma_start(out=mut, in_=io["muT"][j * P:(j + 1) * P, :])
                    x2 = ioA.tile([P, N], f32, tag="x2")
                    nc.vector.scalar_tensor_tensor(x2, mut, gb_sb[:, j:j + 1], inv_b,
                                                   ALU.mult, ALU.mult)
                    cb = ioA.tile([P, N], f32, tag="cb")
                    nc.vector.tensor_scalar(cb, minv_b, gb_sb[:, j:j + 1],
                                            gb_sb[:, 8 + j:9 + j], ALU.mult, ALU.add)
                    nc.gpsimd.tensor_add(mu_nT[:, j, :], x2, cb)
                    sgt = ioA.tile([P, N], f32, tag="sgt")
                    nc.sync.dma_start(out=sgt, in_=io["sgT"][j * P:(j + 1) * P, :])
                    nc.vector.scalar_tensor_tensor(sg_nT[:, j, :], sgt, g2_sb[:, j:j + 1],
                                                   inv2_b, ALU.mult, ALU.mult)
                    z = ioA.tile([P, N], f32, tag="z")
                    nc.gpsimd.tensor_mul(z, mu_nT[:, j, :], mu_nT[:, j, :])
                    nc.gpsimd.tensor_add(a2T[:, j, :], z, sg_nT[:, j, :])

            # --- A2a: QKV q,k rows (transposed out), evict straight to SBUF stage ---
            with ExitStack() as ctx:
                wq = ctx.enter_context(tc.tile_pool(name="wq", bufs=2))
                psQ = ctx.enter_context(tc.tile_pool(name="psQ", bufs=2, space="PSUM"))
                for rb in range(8):
                    rsl = slice(rb * P, (rb + 1) * P)
                    wmu = wq.tile([P, 8, P], bf, tag="wmu")
                    nc.gpsimd.dma_start(out=wmu, in_=io["wqk_mu"][:, rsl].rearrange("(j p) r -> p j r", p=P))
                    wsr = wq.tile([P, 8, P], f32, tag="wsr")
                    nc.sync.dma_start(out=wsr, in_=io["wqk_sr"][:, rsl].rearrange("(j p) r -> p j r", p=P))
                    wsg0 = wq.tile([P, 8, P], f32, tag="wsg0")
                    nc.scalar.activation(wsg0, wsr, AF.Exp)
                    wsig = wq.tile([P, 8, P], bf, tag="wsig")
                    nc.scalar.activation(wsig, wsg0, AF.Ln, bias=1.0)
                    wmu2 = wq.tile([P, 8, P], bf, tag="wmu2")
                    nc.vector.tensor_mul(wmu2, wmu, wmu)
                    for c in range(2):
                        cs = slice(c * 512, (c + 1) * 512)
                        ps_mu = psQ.tile([P, 512], f32, tag="qkmu")
                        for j in range(8):
                            nc.tensor.matmul(ps_mu, wmu[:, j, :], mu_nT[:, j, cs],
                                             start=(j == 0), stop=(j == 7))
                        nc.vector.tensor_copy(qk_mu_sb[:, rb, cs], ps_mu)
                        ps_sg = psQ.tile([P, 512], f32, tag="qksg")
                        for j in range(8):
                            nc.tensor.matmul(ps_sg, wsig[:, j, :], a2T[:, j, cs],
                                             start=(j == 0), stop=False)
                        for j in range(8):
                            nc.tensor.matmul(ps_sg, wmu2[:, j, :], sg_nT[:, j, cs],
                                             start=False, stop=(j == 7))
                        if rb < 4:
                            nc.scalar.activation(qk_sg_sb[:, rb, cs], ps_sg, AF.Copy, scale=scA)
                        else:
                            nc.scalar.copy(qk_sg_sb[:, rb, cs], ps_sg)

            # --- A2b: V (natural layout), evict straight to SBUF stage ---
            with ExitStack() as ctx:
                wv = ctx.enter_context(tc.tile_pool(name="wv", bufs=1))
                psV = ctx.enter_context(tc.tile_pool(name="psV", bufs=2, space="PSUM"))
                wv_mu = wv.tile([P, 8, 512], bf)
                nc.gpsimd.dma_start(out=wv_mu, in_=io["wv_mu"][:].rearrange("(j p) r -> p j r", p=P))
                wv_sr = wv.tile([P, 8, 512], f32)
                nc.sync.dma_start(out=wv_sr, in_=io["wv_sr"][:].rearrange("(j p) r -> p j r", p=P))
                wv_sg0 = wv.tile([P, 8, 512], f32)
                nc.scalar.activation(wv_sg0, wv_sr, AF.Exp)
                wv_sig = wv.tile([P, 8, 512], bf)
                nc.scalar.activation(wv_sig, wv_sg0, AF.Ln, bias=1.0)
                wv_mu2 = wv.tile([P, 8, 512], bf)
                nc.vector.tensor_mul(wv_mu2, wv_mu, wv_mu)
                for tb in range(8):
                    tsl = slice(tb * P, (tb + 1) * P)
                    ps_mu = psV.tile([P, 512], f32, tag="vmu")
                    for j in range(8):
                        nc.tensor.matmul(ps_mu, mu_nT[:, j, tsl], wv_mu[:, j, :],
                                         start=(j == 0), stop=(j == 7))
                    nc.vector.memset(v_mu_sb[:, tb, :], 1.0)
                    nc.vector.tensor_copy(
                        v_mu_sb[:, tb, :].rearrange("p (h c) -> p h c", c=65)[:, :, 0:64],
                        ps_mu.rearrange("p (h c) -> p h c", c=64))
                    ps_sg = psV.tile([P, 512], f32, tag="vsg")
                    for j in range(8):
                        nc.tensor.matmul(ps_sg, a2T[:, j, tsl], wv_sig[:, j, :],
                                         start=(j == 0), stop=False)
                    for j in range(8):
                        nc.tensor.matmul(ps_sg, sg_nT[:, j, tsl], wv_mu2[:, j, :],
                                         start=False, stop=(j == 7))
                    nc.scalar.copy(v_sg_sb[:, tb, :], ps_sg)

        # ============ Phase B: attention (all operands already in SBUF) ============
        with ExitStack() as ctx:
            ep = ctx.enter_context(tc.tile_pool(name="ep", bufs=36))
            sb3 = ctx.enter_context(tc.tile_pool(name="sb3", bufs=6))
            outsb = ctx.enter_context(tc.tile_pool(name="outsb", bufs=4))
            smallB = ctx.enter_context(tc.tile_pool(name="smallB", bufs=4))
            onesB = ctx.enter_context(tc.tile_pool(name="onesB", bufs=1))
            psD = ctx.enter_context(tc.tile_pool(name="psD", bufs=2, space="PSUM"))
            psS2 = ctx.enter_context(tc.tile_pool(name="psS2", bufs=2, space="PSUM"))
            psAVm = ctx.enter_context(tc.tile_pool(name="psAVm", bufs=2, space="PSUM"))
            psAVs = ctx.enter_context(tc.tile_pool(name="psAVs", bufs=1, space="PSUM"))
            psDB = ctx.enter_context(tc.tile_pool(name="psDB", bufs=1, space="PSUM"))

            ones_blk = onesB.tile([P, P], f32)
            nc.vector.memset(ones_blk, 1.0)
            sc128 = onesB.tile([P, 1], f32)
            nc.vector.memset(sc128, SCALE)

            def pass1(hq, c):
                pr, hh = divmod(hq, 2)
                pb = (hq % 2) * 64
                qrb, krb = hq // 2, 4 + hq // 2
                vco = pr * 130 + hh * 65
                cs = slice(c * 512, (c + 1) * 512)
                av_mu = psAVm.tile([65, 512], f32, tag="avmu", name=f"avmu{hq}_{c}")
                e_ts = []
                for kb in range(8):
                    dots = psD.tile([P, 512], f32, tag="dots", name=f"dots{hq}_{c}_{kb}")
                    nc.tensor.matmul(dots,
                                     qk_mu_sb[pb:pb + 64, krb, kb * P:(kb + 1) * P],
                                     qk_mu_sb[pb:pb + 64, qrb, cs],
                                     start=True, stop=True)
                    e_t = ep.tile([P, 512], bf, tag="e", name=f"e{hq}_{c}_{kb}")
                    nc.scalar.activation(e_t, dots, AF.Exp, scale=sc128)
                    e_ts.append(e_t)
                    nc.tensor.matmul(av_mu, v_mu_sb[:, kb, vco:vco + 65], e_t,
                                     start=(kb == 0), stop=(kb == 7))
                r_sb = smallB.tile([P, 512], f32, tag="r", name=f"r{hq}_{c}")
                nc.vector.reciprocal(r_sb[64:65, :], av_mu[64:65, :])
                dbp = psDB.tile([P, 512], f32, tag="db", name=f"dbp{hq}_{c}")
                nc.tensor.matmul(dbp, ones_blk[64:65, :], r_sb[64:65, :], start=True, stop=True)
                db = sb3.tile([P, 512], f32, tag="db_sb", name=f"db{hq}_{c}")
                nc.scalar.copy(db, dbp)
                muo = outsb.tile([64, 512], bf, tag="muo", name=f"muo{hq}_{c}")
                nc.vector.tensor_mul(muo, av_mu[0:64, :], db[0:64, :])
                nc.sync.dma_start(out=oT_mu_sb[pb:pb + 64, qrb, cs], in_=muo)
                return (hq, c, e_ts, db)

            def pass2(stateA, stateB):
                # both heads of a pair: sigma-AV matmuls col-packed via
                # tile_position (0,0)/(0,64) -> run concurrently on the PE,
                # and the packed [128,512] result evicts straight into the
                # contiguous oT_sg_sb slice (no partition-shift DMA).
                hqA, c, e_tsA, dbA = stateA
                hqB, _, e_tsB, dbB = stateB
                pr = hqA // 2
                qrb, krb = pr, 4 + pr
                cs = slice(c * 512, (c + 1) * 512)
                av2 = psAVs.tile([P, 512], f32, tag="avsg", name=f"avsg{hqA}_{c}")
                for kb in range(8):
                    for hq, pb, e_ts, db in ((hqA, 0, e_tsA, dbA), (hqB, 64, e_tsB, dbB)):
                        sdots = psS2.tile([P, 512], f32, tag="sdots", name=f"sd{hq}_{c}_{kb}")
                        nc.tensor.matmul(sdots,
                                         qk_sg_sb[pb:pb + 64, krb, kb * P:(kb + 1) * P],
                                         qk_sg_sb[pb:pb + 64, qrb, cs],
                                         start=True, stop=True)
                        p_t = sb3.tile([P, 512], f32, tag="p", name=f"p{hq}_{c}_{kb}")
                        nc.gpsimd.tensor_mul(p_t, e_ts[kb], db)
                        t_t = sb3.tile([P, 512], f32, tag="t", name=f"t{hq}_{c}_{kb}")
                        if kb % 2 == 0:
                            nc.vector.scalar_tensor_tensor(t_t, p_t, 1.0, p_t,
                                                           ALU.subtract, ALU.mult)
                        else:
                            m_t = sb3.tile([P, 512], f32, tag="m", name=f"m{hq}_{c}_{kb}")
                            nc.gpsimd.tensor_mul(m_t, p_t, p_t)
                            nc.gpsimd.tensor_sub(t_t, p_t, m_t)
                        u_t = sb3.tile([P, 512], f32, tag="u", name=f"u{hq}_{c}_{kb}")
                        if kb % 2 == 0:
                            nc.gpsimd.tensor_mul(u_t, t_t, t_t)
                        else:
                            nc.scalar.activation(u_t, t_t, AF.Square)
                        w_t = sb3.tile([P, 512], bf, tag="w", name=f"w{hq}_{c}_{kb}")
                        nc.vector.tensor_mul(w_t, u_t, sdots)
                        nc.tensor.matmul(av2[pb:pb + 64, :],
                                         v_sg_sb[:, kb, hq * 64:(hq + 1) * 64], w_t,
                                         start=(kb == 0), stop=(kb == 7),
                                         tile_position=(0, pb),
                                         skip_group_check=True)
                nc.scalar.copy(oT_sg_sb[:, qrb, cs], av2)

            prev = None
            for pr in range(4):
                for c in range(2):
                    curA = pass1(2 * pr, c)
                    curB = pass1(2 * pr + 1, c)
                    if prev is not None:
                        pass2(*prev)
                    prev = (curA, curB)
            pass2(*prev)

        # ============ Phase C: out-projection ============
        with ExitStack() as ctx:
            wo = ctx.enter_context(tc.tile_pool(name="wo", bufs=1))
            oin = ctx.enter_context(tc.tile_pool(name="oin", bufs=1))
            evC = ctx.enter_context(tc.tile_pool(name="evC", bufs=4))
            psC = ctx.enter_context(tc.tile_pool(name="psC", bufs=2, space="PSUM"))

            wo_mu = wo.tile([P, 4, D], bf)
            nc.gpsimd.dma_start(out=wo_mu, in_=io["wo_mu"][:].rearrange("(j p) o -> p j o", p=P))
            wo_sr = wo.tile([P, 4, D], f32)
            nc.sync.dma_start(out=wo_sr, in_=io["wo_sr"][:].rearrange("(j p) o -> p j o", p=P))
            wo_sg0 = wo.tile([P, 4, D], f32)
            nc.scalar.activation(wo_sg0, wo_sr, AF.Exp)
            wo_sig = wo.tile([P, 4, D], bf)
            nc.scalar.activation(wo_sig, wo_sg0, AF.Ln, bias=1.0)
            wo_mu2 = wo.tile([P, 4, D], bf)
            nc.vector.tensor_mul(wo_mu2, wo_mu, wo_mu)

            a2o = oin.tile([P, 4, N], bf)
            zsq = oin.tile([P, 4, N], f32)
            nc.scalar.activation(zsq, oT_mu_sb, AF.Square)
            nc.vector.tensor_add(a2o, zsq, oT_sg_sb)

            for ob in range(8):
                osl = slice(ob * P, (ob + 1) * P)
                for c in range(2):
                    cs = slice(c * 512, (c + 1) * 512)
                    ps_mu = psC.tile([P, 512], f32, tag="ymu")
                    for j in range(4):
                        nc.tensor.matmul(ps_mu, wo_mu[:, j, osl], oT_mu_sb[:, j, cs],
                                         start=(j == 0), stop=(j == 3))
                    ev1 = evC.tile([P, 512], f32, tag="ev1")
                    nc.vector.tensor_copy(ev1, ps_mu)
                    nc.sync.dma_start(out=io["yT_mu"][osl, cs], in_=ev1)
                    ps_sg = psC.tile([P, 512], f32, tag="ysg")
                    for j in range(4):
                        nc.tensor.matmul(ps_sg, wo_sig[:, j, osl], a2o[:, j, cs],
                                         start=(j == 0), stop=False)
                    for j in range(4):
                        nc.tensor.matmul(ps_sg, wo_mu2[:, j, osl], oT_sg_sb[:, j, cs],
                                         start=False, stop=(j == 3))
                    ev2 = evC.tile([P, 512], f32, tag="ev2")
                    nc.scalar.copy(ev2, ps_sg)
                    nc.sync.dma_start(out=io["yT_sg"][osl, cs], in_=ev2)


def _get_nc():
    if "nc" not in _NC_CACHE:
        _NC_CACHE["nc"] = _build_nc()
    return _NC_CACHE["nc"]


def _prep_core_inputs(c, mu, sigma, ln_gamma, ln_beta, Wqkv_mu, Wqkv_sigma_raw,
                      Wout_mu, Wout_sigma_raw):
    f = np.float32
    asc = np.ascontiguousarray
    b, g = divmod(c, 2)
    qs = slice(512 * g, 512 * (g + 1))
    ks = slice(1024 + 512 * g, 1024 + 512 * (g + 1))
    vs = slice(2048 + 512 * g, 2048 + 512 * (g + 1))
    gb = np.zeros((P, 16), f)
    gb[:, :8] = np.asarray(ln_gamma, f).reshape(8, P).T
    gb[:, 8:] = np.asarray(ln_beta, f).reshape(8, P).T
    wqk_mu = np.concatenate([Wqkv_mu[qs], Wqkv_mu[ks]], 0)
    wqk_sr = np.concatenate([Wqkv_sigma_raw[qs], Wqkv_sigma_raw[ks]], 0)
    return {
        "muT": asc(np.asarray(mu[b], f).T),
        "sgT": asc(np.asarray(sigma[b], f).T),
        "gb": gb,
        "wqk_mu": asc(np.asarray(wqk_mu, f).T),
        "wqk_sr": asc(np.asarray(wqk_sr, f).T),
        "wv_mu": asc(np.asarray(Wqkv_mu[vs], f).T),
        "wv_sr": asc(np.asarray(Wqkv_sigma_raw[vs], f).T),
        "wo_mu": asc(np.asarray(Wout_mu[:, 512 * g:512 * (g + 1)], f).T),
        "wo_sr": asc(np.asarray(Wout_sigma_raw[:, 512 * g:512 * (g + 1)], f).T),
    }


def _emulate_core(m):
    """Pure-numpy mirror of the on-device program (for validation only)."""
    sp = lambda x: np.log1p(np.exp(x))
    muT, sgT = m["muT"], m["sgT"]
    gamma = m["gb"][:, :8].T.reshape(-1)[:, None]   # [D,1] indexed by d
    beta = m["gb"][:, 8:].T.reshape(-1)[:, None]
    mean = muT.mean(0, keepdims=True)
    var = muT.var(0, keepdims=True)
    inv = 1.0 / np.sqrt(var + EPS)
    mu_nT = (muT * inv - mean * inv) * gamma + beta
    sg_nT = sgT * gamma * gamma * inv * inv
    a2T = mu_nT * mu_nT + sg_nT
    qkT_mu = m["wqk_mu"].T @ mu_nT
    qkT_sg = sp(m["wqk_sr"]).T @ a2T + (m["wqk_mu"] ** 2).T @ sg_nT
    v_mu = mu_nT.T @ m["wv_mu"]
    v_sg = a2T.T @ sp(m["wv_sr"]) + sg_nT.T @ m["wv_mu"] ** 2
    oT_mu = np.zeros((RV, N), np.float32)
    oT_sg = np.zeros((RV, N), np.float32)
    for h in range(HPC):
        hs = slice(h * 64, (h + 1) * 64)
        sT = m_kT = qkT_mu[512 + h * 64:512 + (h + 1) * 64].T @ qkT_mu[hs]  # [kt, qt]
        e = np.exp(SCALE * sT)
        den = e.sum(0, keepdims=True)
        db = 1.0 / den
        p = e * db
        oT_mu[hs] = (v_mu[:, hs].T @ e) * db
        sdT = qkT_sg[512 + h * 64:512 + (h + 1) * 64].T @ qkT_sg[hs]
        t = (p - 1.0) * p
        w = (t * t) * SCALE * sdT
        oT_sg[hs] = v_sg[:, hs].T @ w
    a2o = oT_mu * oT_mu + oT_sg
    yT_mu = m["wo_mu"].T @ oT_mu
    yT_sg = sp(m["wo_sr"]).T @ a2o + (m["wo_mu"] ** 2).T @ oT_sg
    return yT_mu.astype(np.float32), yT_sg.astype(np.float32)


def kernel(mu, sigma, ln_gamma, ln_beta, Wqkv_mu, Wqkv_sigma_raw, Wout_mu,
           Wout_sigma_raw, _trace=False):
    from concourse.bass_utils import run_bass_kernel_spmd

    nc = _get_nc()
    args = (mu, sigma, ln_gamma, ln_beta, Wqkv_mu, Wqkv_sigma_raw, Wout_mu,
            Wout_sigma_raw)
    in_maps = [_prep_core_inputs(c, *args) for c in range(8)]
    res = run_bass_kernel_spmd(nc, in_maps, list(range(8)), trace=_trace)
    out_mu = np.zeros((B, N, D), np.float32)
    out_sg = np.zeros((B, N, D), np.float32)
    for c in range(8):
        b = c // 2
        out_mu[b] += res.results[c]["yT_mu"].T
        out_sg[b] += res.results[c]["yT_sg"].T
    if _trace:
        kernel._last_result = res
    return out_mu, out_sg


# revision 31
# speedup vs baseline: 213.5928x; 1.0071x over previous
"""VDP (variance-propagating) attention kernel for Trainium2, 8 NeuronCores.

Sharding: core c -> (batch b = c//2, head-group g = c%2) [8 heads each].
Each core computes LN + its QKV slice + attention for its 8 heads + the
partial out-projection for its 512 inner columns. Host sums the two
head-group partials per batch. No collectives needed.

Layout trick: everything on-device lives transposed as [feature, token]
(activations) / [contraction, out] (weights), prepared host-side, so the
contraction dim is always on partitions and no on-device transposes are
needed anywhere. LayerNorm stats (reduce over features = partitions) are
done with ones-vector matmuls on the PE; softmax denominators come for
free from a ones-augmented column in the V operand of the mu-attention AV
matmul, and are broadcast back across partitions with a K=1 PE matmul.
"""

import os
import sys

import numpy as np

for _p in ("/opt/trn_rl_repo", "/root/.axon_site/_ro/trn_rl_repo"):
    if os.path.isdir(_p) and _p not in sys.path:
        sys.path.insert(0, _p)

HEADS = 16
DH = 64
SCALE = DH ** -0.5
EPS = 1e-5
B, N, D = 4, 1024, 1024
HPC = 8          # heads per core
RQK = 1024       # q+k rows per core (2 * 8 heads * 64)
RV = 512         # v rows per core
P = 128

_NC_CACHE = {}


def _build_nc(tiny_out=False):
    import concourse.bass as bass  # noqa: F401
    import concourse.tile as tile
    from concourse import bacc, mybir

    f32 = mybir.dt.float32
    AF = mybir.ActivationFunctionType
    ALU = mybir.AluOpType

    nc = bacc.Bacc(None, target_bir_lowering=False)

    io = {}
    for name, shape in [
        ("muT", [D, N]), ("sgT", [D, N]), ("gb", [P, 16]),
        ("wqk_mu", [D, RQK]), ("wqk_sr", [D, RQK]),
        ("wv_mu", [D, RV]), ("wv_sr", [D, RV]),
        ("wo_mu", [RV, D]), ("wo_sr", [RV, D]),
    ]:
        io[name] = nc.dram_tensor(name, shape, f32, kind="ExternalInput")
    if tiny_out:
        for name, shape in [("yT_mu", [D, N]), ("yT_sg", [D, N])]:
            io[name] = nc.dram_tensor(name, shape, f32)
        io["done"] = nc.dram_tensor("done", [1, 16], f32, kind="ExternalOutput")
    else:
        for name, shape in [("yT_mu", [D, N]), ("yT_sg", [D, N])]:
            io[name] = nc.dram_tensor(name, shape, f32, kind="ExternalOutput")
    # internal DRAM staging
    bf = mybir.dt.bfloat16

    with tile.TileContext(nc) as tc:
        _emit(nc, tc, io, f32, bf, AF, ALU)
        if tiny_out:
            with tc.tile_pool(name="doneP", bufs=1) as dp:
                dt = dp.tile([1, 16], f32)
                nc.vector.memset(dt, 1.0)
                nc.sync.dma_start(out=io["done"][:], in_=dt)
    nc.compile()
    return nc


def _build_floor_nc():
    import concourse.tile as tile
    from concourse import bacc, mybir

    f32 = mybir.dt.float32
    nc = bacc.Bacc(None, target_bir_lowering=False)
    done = nc.dram_tensor("done", [1, 16], f32, kind="ExternalOutput")
    with tile.TileContext(nc) as tc:
        with tc.tile_pool(name="dp", bufs=1) as dp:
            dt = dp.tile([1, 16], f32)
            nc.vector.memset(dt, 1.0)
            nc.sync.dma_start(out=done[:], in_=dt)
    nc.compile()
    return nc


def _emit(nc, tc, io, f32, bf, AF, ALU):
    from contextlib import ExitStack

    with ExitStack() as tctx:
        stage = tctx.enter_context(tc.tile_pool(name="stage", bufs=1))
        # persistent SBUF staging (bf16): no DRAM round trips between phases
        qk_mu_sb = stage.tile([P, 8, N], bf)    # rows: 0-3 q-blocks, 4-7 k-blocks
        qk_sg_sb = stage.tile([P, 8, N], bf)
        v_mu_sb = stage.tile([P, 8, HPC * 65], bf)   # per tok-block: 8 heads x (64 v + ones)
        v_sg_sb = stage.tile([P, 8, RV], bf)
        oT_mu_sb = stage.tile([P, 4, N], bf)
        oT_sg_sb = stage.tile([P, 4, N], bf)

        # ============ Phase A: LayerNorm + QKV ============
        with ExitStack() as actx:
            acts = actx.enter_context(tc.tile_pool(name="acts", bufs=1))
            smallA = actx.enter_context(tc.tile_pool(name="smallA", bufs=1))

            gb_sb = smallA.tile([P, 16], f32)
            nc.sync.dma_start(out=gb_sb, in_=io["gb"][:])
            g2_sb = smallA.tile([P, 8], f32)
            nc.vector.tensor_mul(g2_sb, gb_sb[:, 0:8], gb_sb[:, 0:8])
            ones_col = smallA.tile([P, 1], f32)
            nc.vector.memset(ones_col, 1.0)
            ones_row = smallA.tile([1, P], f32)
            nc.vector.memset(ones_row, 1.0)
            eps1 = smallA.tile([1, 1], f32)
            nc.vector.memset(eps1, EPS)
            scA = smallA.tile([P, 1], f32)
            nc.vector.memset(scA, SCALE)

            inv_b = acts.tile([P, N], f32)
            minv_b = acts.tile([P, N], f32)
            inv2_b = acts.tile([P, N], f32)
            mu_nT = acts.tile([P, 8, N], bf)
            sg_nT = acts.tile([P, 8, N], bf)
            a2T = acts.tile([P, 8, N], bf)

            # --- A1: stats + normalize (muT streamed twice, not resident) ---
            with ExitStack() as ctx:
                ioA = ctx.enter_context(tc.tile_pool(name="ioA", bufs=2))
                psS = ctx.enter_context(tc.tile_pool(name="psS", bufs=1, space="PSUM"))
                psA = ctx.enter_context(tc.tile_pool(name="psA", bufs=2, space="PSUM"))

                sum_ps = [psS.tile([1, 512], f32, tag=f"sum{c}", name=f"sum{c}") for c in range(2)]
                sq_ps = [psS.tile([1, 512], f32, tag=f"sq{c}", name=f"sq{c}") for c in range(2)]
                for j in range(8):
                    mut = ioA.tile([P, N], f32, tag="mut")
                    nc.sync.dma_start(out=mut, in_=io["muT"][j * P:(j + 1) * P, :])
                    mu2 = ioA.tile([P, N], f32, tag="mu2")
                    nc.gpsimd.tensor_mul(mu2, mut, mut)
                    for c in range(2):
                        cs = slice(c * 512, (c + 1) * 512)
                        nc.tensor.matmul(sum_ps[c], ones_col, mut[:, cs],
                                         start=(j == 0), stop=(j == 7), skip_group_check=True)
                        nc.tensor.matmul(sq_ps[c], ones_col, mu2[:, cs],
                                         start=(j == 0), stop=(j == 7), skip_group_check=True)

                inv_sb = smallA.tile([1, N], f32)
                minv_sb = smallA.tile([1, N], f32)
                for c in range(2):
                    cs = slice(c * 512, (c + 1) * 512)
                    mean_t = ioA.tile([1, 512], f32, tag="mean")
                    nc.vector.tensor_scalar_mul(mean_t, sum_ps[c], 1.0 / D)
                    m2_t = ioA.tile([1, 512], f32, tag="m2")
                    nc.vector.tensor_mul(m2_t, mean_t, mean_t)
                    var_t = ioA.tile([1, 512], f32, tag="var")
                    nc.vector.scalar_tensor_tensor(var_t, sq_ps[c], 1.0 / D, m2_t,
                                                   ALU.mult, ALU.subtract)
                    std_t = ioA.tile([1, 512], f32, tag="std")
                    nc.scalar.activation(std_t, var_t, AF.Sqrt, bias=eps1)
                    nc.vector.reciprocal(inv_sb[:, cs], std_t)
                    nc.vector.scalar_tensor_tensor(minv_sb[:, cs], mean_t, -1.0, inv_sb[:, cs],
                                                   ALU.mult, ALU.mult)

                for c in range(2):
                    cs = slice(c * 512, (c + 1) * 512)
                    bp1 = psA.tile([P, 512], f32, tag="bcast")
                    nc.tensor.matmul(bp1, ones_row, inv_sb[:, cs], start=True, stop=True)
                    nc.scalar.copy(inv_b[:, cs], bp1)
                    bp2 = psA.tile([P, 512], f32, tag="bcast")
                    nc.tensor.matmul(bp2, ones_row, minv_sb[:, cs], start=True, stop=True)
                    nc.scalar.copy(minv_b[:, cs], bp2)
                nc.vector.tensor_mul(inv2_b, inv_b, inv_b)

                for j in range(8):
                    mut = ioA.tile([P, N], f32, tag="mut")
                    nc.sync.dma_start(out=mut, in_=io["muT"][j * P:(j + 1) * P, :])
                    x2 = ioA.tile([P, N], f32, tag="x2")
                    nc.vector.scalar_tensor_tensor(x2, mut, gb_sb[:, j:j + 1], inv_b,
                                                   ALU.mult, ALU.mult)
                    cb = ioA.tile([P, N], f32, tag="cb")
                    nc.vector.tensor_scalar(cb, minv_b, gb_sb[:, j:j + 1],
                                            gb_sb[:, 8 + j:9 + j], ALU.mult, ALU.add)
                    nc.gpsimd.tensor_add(mu_nT[:, j, :], x2, cb)
                    sgt = ioA.tile([P, N], f32, tag="sgt")
                    nc.sync.dma_start(out=sgt, in_=io["sgT"][j * P:(j + 1) * P, :])
                    nc.vector.scalar_tensor_tensor(sg_nT[:, j, :], sgt, g2_sb[:, j:j + 1],
                                                   inv2_b, ALU.mult, ALU.mult)
                    z = ioA.tile([P, N], f32, tag="z")
                    nc.gpsimd.tensor_mul(z, mu_nT[:, j, :], mu_nT[:, j, :])
                    nc.gpsimd.tensor_add(a2T[:, j, :], z, sg_nT[:, j, :])

            # --- A2a: QKV q,k rows (transposed out), evict straight to SBUF stage ---
            with ExitStack() as ctx:
                wq = ctx.enter_context(tc.tile_pool(name="wq", bufs=2))
                psQ = ctx.enter_context(tc.tile_pool(name="psQ", bufs=2, space="PSUM"))
                for rb in range(8):
                    rsl = slice(rb * P, (rb + 1) * P)
                    wmu = wq.tile([P, 8, P], bf, tag="wmu")
                    nc.gpsimd.dma_start(out=wmu, in_=io["wqk_mu"][:, rsl].rearrange("(j p) r -> p j r", p=P))
                    wsr = wq.tile([P, 8, P], f32, tag="wsr")
                    nc.sync.dma_start(out=wsr, in_=io["wqk_sr"][:, rsl].rearrange("(j p) r -> p j r", p=P))
                    wsg0 = wq.tile([P, 8, P], f32, tag="wsg0")
                    nc.scalar.activation(wsg0, wsr, AF.Exp)
                    wsig = wq.tile([P, 8, P], bf, tag="wsig")
                    nc.scalar.activation(wsig, wsg0, AF.Ln, bias=1.0)
                    wmu2 = wq.tile([P, 8, P], bf, tag="wmu2")
                    nc.vector.tensor_mul(wmu2, wmu, wmu)
                    for c in range(2):
                        cs = slice(c * 512, (c + 1) * 512)
                        ps_mu = psQ.tile([P, 512], f32, tag="qkmu")
                        for j in range(8):
                            nc.tensor.matmul(ps_mu, wmu[:, j, :], mu_nT[:, j, cs],
                                             start=(j == 0), stop=(j == 7))
                        nc.vector.tensor_copy(qk_mu_sb[:, rb, cs], ps_mu)
                        ps_sg = psQ.tile([P, 512], f32, tag="qksg")
                        for j in range(8):
                            nc.tensor.matmul(ps_sg, wsig[:, j, :], a2T[:, j, cs],
                                             start=(j == 0), stop=False)
                        for j in range(8):
                            nc.tensor.matmul(ps_sg, wmu2[:, j, :], sg_nT[:, j, cs],
                                             start=False, stop=(j == 7))
                        if rb < 4:
                            nc.scalar.activation(qk_sg_sb[:, rb, cs], ps_sg, AF.Copy, scale=scA)
                        else:
                            nc.scalar.copy(qk_sg_sb[:, rb, cs], ps_sg)

            # --- A2b: V (natural layout), evict straight to SBUF stage ---
            with ExitStack() as ctx:
                wv = ctx.enter_context(tc.tile_pool(name="wv", bufs=1))
                psV = ctx.enter_context(tc.tile_pool(name="psV", bufs=2, space="PSUM"))
                wv_mu = wv.tile([P, 8, 512], bf)
                nc.gpsimd.dma_start(out=wv_mu, in_=io["wv_mu"][:].rearrange("(j p) r -> p j r", p=P))
                wv_sr = wv.tile([P, 8, 512], f32)
                nc.sync.dma_start(out=wv_sr, in_=io["wv_sr"][:].rearrange("(j p) r -> p j r", p=P))
                wv_sg0 = wv.tile([P, 8, 512], f32)
                nc.scalar.activation(wv_sg0, wv_sr, AF.Exp)
                wv_sig = wv.tile([P, 8, 512], bf)
                nc.scalar.activation(wv_sig, wv_sg0, AF.Ln, bias=1.0)
                wv_mu2 = wv.tile([P, 8, 512], bf)
                nc.vector.tensor_mul(wv_mu2, wv_mu, wv_mu)
                for tb in range(8):
                    tsl = slice(tb * P, (tb + 1) * P)
                    ps_mu = psV.tile([P, 512], f32, tag="vmu")
                    for j in range(8):
                        nc.tensor.matmul(ps_mu, mu_nT[:, j, tsl], wv_mu[:, j, :],
                                         start=(j == 0), stop=(j == 7))
                    nc.vector.memset(v_mu_sb[:, tb, :], 1.0)
                    nc.vector.tensor_copy(
                        v_mu_sb[:, tb, :].rearrange("p (h c) -> p h c", c=65)[:, :, 0:64],
                        ps_mu.rearrange("p (h c) -> p h c", c=64))
                    ps_sg = psV.tile([P, 512], f32, tag="vsg")
                    for j in range(8):
                        nc.tensor.matmul(ps_sg, a2T[:, j, tsl], wv_sig[:, j, :],
                                         start=(j == 0), stop=False)
                    for j in range(8):
                        nc.tensor.matmul(ps_sg, sg_nT[:, j, tsl], wv_mu2[:, j, :],
                                         start=False, stop=(j == 7))
                    nc.scalar.copy(v_sg_sb[:, tb, :], ps_sg)

        # ============ Phase B: attention (all operands already in SBUF) ============
        with ExitStack() as ctx:
            ep = ctx.enter_context(tc.tile_pool(name="ep", bufs=36))
            sb3 = ctx.enter_context(tc.tile_pool(name="sb3", bufs=6))
            outsb = ctx.enter_context(tc.tile_pool(name="outsb", bufs=4))
            smallB = ctx.enter_context(tc.tile_pool(name="smallB", bufs=4))
            onesB = ctx.enter_context(tc.tile_pool(name="onesB", bufs=1))
            psD = ctx.enter_context(tc.tile_pool(name="psD", bufs=2, space="PSUM"))
            psS2 = ctx.enter_context(tc.tile_pool(name="psS2", bufs=2, space="PSUM"))
            psAVm = ctx.enter_context(tc.tile_pool(name="psAVm", bufs=2, space="PSUM"))
            psAVs = ctx.enter_context(tc.tile_pool(name="psAVs", bufs=1, space="PSUM"))
            psDB = ctx.enter_context(tc.tile_pool(name="psDB", bufs=1, space="PSUM"))

            ones_blk = onesB.tile([P, P], f32)
            nc.vector.memset(ones_blk, 1.0)
            sc128 = onesB.tile([P, 1], f32)
            nc.vector.memset(sc128, SCALE)

            def pass1(hq, c):
                pr, hh = divmod(hq, 2)
                pb = (hq % 2) * 64
                qrb, krb = hq // 2, 4 + hq // 2
                vco = pr * 130 + hh * 65
                cs = slice(c * 512, (c + 1) * 512)
                av_mu = psAVm.tile([65, 512], f32, tag="avmu", name=f"avmu{hq}_{c}")
                e_ts = []
                for kb in range(8):
                    dots = psD.tile([P, 512], f32, tag="dots", name=f"dots{hq}_{c}_{kb}")
                    nc.tensor.matmul(dots,
                                     qk_mu_sb[pb:pb + 64, krb, kb * P:(kb + 1) * P],
                                     qk_mu_sb[pb:pb + 64, qrb, cs],
                                     start=True, stop=True)
                    e_t = ep.tile([P, 512], bf, tag="e", name=f"e{hq}_{c}_{kb}")
                    nc.scalar.activation(e_t, dots, AF.Exp, scale=sc128)
                    e_ts.append(e_t)
                    nc.tensor.matmul(av_mu, v_mu_sb[:, kb, vco:vco + 65], e_t,
                                     start=(kb == 0), stop=(kb == 7))
                r_sb = smallB.tile([P, 512], f32, tag="r", name=f"r{hq}_{c}")
                nc.vector.reciprocal(r_sb[64:65, :], av_mu[64:65, :])
                dbp = psDB.tile([P, 512], f32, tag="db", name=f"dbp{hq}_{c}")
                nc.tensor.matmul(dbp, ones_blk[64:65, :], r_sb[64:65, :], start=True, stop=True)
                db = sb3.tile([P, 512], f32, tag="db_sb", name=f"db{hq}_{c}")
                nc.scalar.copy(db, dbp)
                muo = outsb.tile([64, 512], bf, tag="muo", name=f"muo{hq}_{c}")
                nc.vector.tensor_mul(muo, av_mu[0:64, :], db[0:64, :])
                nc.sync.dma_start(out=oT_mu_sb[pb:pb + 64, qrb, cs], in_=muo)
                return (hq, c, e_ts, db)

            def pass2(stateA, stateB):
                # both heads of a pair: sigma-AV matmuls col-packed via
                # tile_position (0,0)/(0,64) -> run concurrently on the PE,
                # and the packed [128,512] result evicts straight into the
                # contiguous oT_sg_sb slice (no partition-shift DMA).
                hqA, c, e_tsA, dbA = stateA
                hqB, _, e_tsB, dbB = stateB
                pr = hqA // 2
                qrb, krb = pr, 4 + pr
                cs = slice(c * 512, (c + 1) * 512)
                av2 = psAVs.tile([P, 512], f32, tag="avsg", name=f"avsg{hqA}_{c}")
                for kb in range(8):
                    for hq, pb, e_ts, db in ((hqA, 0, e_tsA, dbA), (hqB, 64, e_tsB, dbB)):
                        sdots = psS2.tile([P, 512], f32, tag="sdots", name=f"sd{hq}_{c}_{kb}")
                        nc.tensor.matmul(sdots,
                                         qk_sg_sb[pb:pb + 64, krb, kb * P:(kb + 1) * P],
                                         qk_sg_sb[pb:pb + 64, qrb, cs],
                                         start=True, stop=True)
                        p_t = sb3.tile([P, 512], f32, tag="p", name=f"p{hq}_{c}_{kb}")
                        nc.gpsimd.tensor_mul(p_t, e_ts[kb], db)
                        t_t = sb3.tile([P, 512], f32, tag="t", name=f"t{hq}_{c}_{kb}")
                        if kb % 2 == 0:
                            nc.vector.scalar_tensor_tensor(t_t, p_t, 1.0, p_t,
                                                           ALU.subtract, ALU.mult)
                        else:
                            m_t = sb3.tile([P, 512], f32, tag="m", name=f"m{hq}_{c}_{kb}")
                            nc.gpsimd.tensor_mul(m_t, p_t, p_t)
                            nc.gpsimd.tensor_sub(t_t, p_t, m_t)
                        u_t = sb3.tile([P, 512], f32, tag="u", name=f"u{hq}_{c}_{kb}")
                        if kb % 2 == 0:
                            nc.gpsimd.tensor_mul(u_t, t_t, t_t)
                        else:
                            nc.scalar.activation(u_t, t_t, AF.Square)
                        w_t = sb3.tile([P, 512], bf, tag="w", name=f"w{hq}_{c}_{kb}")
                        nc.vector.tensor_mul(w_t, u_t, sdots)
                        nc.tensor.matmul(av2[pb:pb + 64, :],
                                         v_sg_sb[:, kb, hq * 64:(hq + 1) * 64], w_t,
                                         start=(kb == 0), stop=(kb == 7),
                                         tile_position=(0, pb),
                                         skip_group_check=True)
                nc.scalar.copy(oT_sg_sb[:, qrb, cs], av2)

            prev = None
            for pr in range(4):
                for c in range(2):
                    curA = pass1(2 * pr, c)
                    curB = pass1(2 * pr + 1, c)
                    if prev is not None:
                        pass2(*prev)
                    prev = (curA, curB)
            pass2(*prev)

        # ============ Phase C: out-projection ============
        with ExitStack() as ctx:
            wo = ctx.enter_context(tc.tile_pool(name="wo", bufs=1))
            oin = ctx.enter_context(tc.tile_pool(name="oin", bufs=1))
            evC = ctx.enter_context(tc.tile_pool(name="evC", bufs=4))
            psC = ctx.enter_context(tc.tile_pool(name="psC", bufs=2, space="PSUM"))

            wo_mu = wo.tile([P, 4, D], bf)
            nc.gpsimd.dma_start(out=wo_mu, in_=io["wo_mu"][:].rearrange("(j p) o -> p j o", p=P))
            wo_sr = wo.tile([P, 4, D], f32)
            nc.sync.dma_start(out=wo_sr, in_=io["wo_sr"][:].rearrange("(j p) o -> p j o", p=P))
            wo_sg0 = wo.tile([P, 4, D], f32)
            nc.scalar.activation(wo_sg0, wo_sr, AF.Exp)
            wo_sig = wo.tile([P, 4, D], bf)
            nc.scalar.activation(wo_sig, wo_sg0, AF.Ln, bias=1.0)
            wo_mu2 = wo.tile([P, 4, D], bf)
            nc.vector.tensor_mul(wo_mu2, wo_mu, wo_mu)

            a2o = oin.tile([P, 4, N], bf)
            zsq = oin.tile([P, 4, N], f32)
            for j in range(4):
                nc.scalar.activation(zsq[:, j, :], oT_mu_sb[:, j, :], AF.Square)
                nc.gpsimd.tensor_add(a2o[:, j, :], zsq[:, j, :], oT_sg_sb[:, j, :])

            for ob in range(8):
                osl = slice(ob * P, (ob + 1) * P)
                for c in range(2):
                    cs = slice(c * 512, (c + 1) * 512)
                    ps_mu = psC.tile([P, 512], f32, tag="ymu")
                    for j in range(4):
                        nc.tensor.matmul(ps_mu, wo_mu[:, j, osl], oT_mu_sb[:, j, cs],
                                         start=(j == 0), stop=(j == 3))
                    ev1 = evC.tile([P, 512], f32, tag="ev1")
                    nc.vector.tensor_copy(ev1, ps_mu)
                    nc.sync.dma_start(out=io["yT_mu"][osl, cs], in_=ev1)
                    ps_sg = psC.tile([P, 512], f32, tag="ysg")
                    for j in range(4):
                        nc.tensor.matmul(ps_sg, wo_sig[:, j, osl], a2o[:, j, cs],
                                         start=(j == 0), stop=False)
                    for j in range(4):
                        nc.tensor.matmul(ps_sg, wo_mu2[:, j, osl], oT_sg_sb[:, j, cs],
                                         start=False, stop=(j == 3))
                    ev2 = evC.tile([P, 512], f32, tag="ev2")
                    nc.scalar.copy(ev2, ps_sg)
                    nc.sync.dma_start(out=io["yT_sg"][osl, cs], in_=ev2)


def _get_nc():
    if "nc" not in _NC_CACHE:
        _NC_CACHE["nc"] = _build_nc()
    return _NC_CACHE["nc"]


def _prep_core_inputs(c, mu, sigma, ln_gamma, ln_beta, Wqkv_mu, Wqkv_sigma_raw,
                      Wout_mu, Wout_sigma_raw):
    f = np.float32
    asc = np.ascontiguousarray
    b, g = divmod(c, 2)
    qs = slice(512 * g, 512 * (g + 1))
    ks = slice(1024 + 512 * g, 1024 + 512 * (g + 1))
    vs = slice(2048 + 512 * g, 2048 + 512 * (g + 1))
    gb = np.zeros((P, 16), f)
    gb[:, :8] = np.asarray(ln_gamma, f).reshape(8, P).T
    gb[:, 8:] = np.asarray(ln_beta, f).reshape(8, P).T
    wqk_mu = np.concatenate([Wqkv_mu[qs], Wqkv_mu[ks]], 0)
    wqk_sr = np.concatenate([Wqkv_sigma_raw[qs], Wqkv_sigma_raw[ks]], 0)
    return {
        "muT": asc(np.asarray(mu[b], f).T),
        "sgT": asc(np.asarray(sigma[b], f).T),
        "gb": gb,
        "wqk_mu": asc(np.asarray(wqk_mu, f).T),
        "wqk_sr": asc(np.asarray(wqk_sr, f).T),
        "wv_mu": asc(np.asarray(Wqkv_mu[vs], f).T),
        "wv_sr": asc(np.asarray(Wqkv_sigma_raw[vs], f).T),
        "wo_mu": asc(np.asarray(Wout_mu[:, 512 * g:512 * (g + 1)], f).T),
        "wo_sr": asc(np.asarray(Wout_sigma_raw[:, 512 * g:512 * (g + 1)], f).T),
    }


def _emulate_core(m):
    """Pure-numpy mirror of the on-device program (for validation only)."""
    sp = lambda x: np.log1p(np.exp(x))
    muT, sgT = m["muT"], m["sgT"]
    gamma = m["gb"][:, :8].T.reshape(-1)[:, None]   # [D,1] indexed by d
    beta = m["gb"][:, 8:].T.reshape(-1)[:, None]
    mean = muT.mean(0, keepdims=True)
    var = muT.var(0, keepdims=True)
    inv = 1.0 / np.sqrt(var + EPS)
    mu_nT = (muT * inv - mean * inv) * gamma + beta
    sg_nT = sgT * gamma * gamma * inv * inv
    a2T = mu_nT * mu_nT + sg_nT
    qkT_mu = m["wqk_mu"].T @ mu_nT
    qkT_sg = sp(m["wqk_sr"]).T @ a2T + (m["wqk_mu"] ** 2).T @ sg_nT
    v_mu = mu_nT.T @ m["wv_mu"]
    v_sg = a2T.T @ sp(m["wv_sr"]) + sg_nT.T @ m["wv_mu"] ** 2
    oT_mu = np.zeros((RV, N), np.float32)
    oT_sg = np.zeros((RV, N), np.float32)
    for h in range(HPC):
        hs = slice(h * 64, (h + 1) * 64)
        sT = m_kT = qkT_mu[512 + h * 64:512 + (h + 1) * 64].T @ qkT_mu[hs]  # [kt, qt]
        e = np.exp(SCALE * sT)
        den = e.sum(0, keepdims=True)
        db = 1.0 / den
        p = e * db
        oT_mu[hs] = (v_mu[:, hs].T @ e) * db
        sdT = qkT_sg[512 + h * 64:512 + (h + 1) * 64].T @ qkT_sg[hs]
        t = (p - 1.0) * p
        w = (t * t) * SCALE * sdT
        oT_sg[hs] = v_sg[:, hs].T @ w
    a2o = oT_mu * oT_mu + oT_sg
    yT_mu = m["wo_mu"].T @ oT_mu
    yT_sg = sp(m["wo_sr"]).T @ a2o + (m["wo_mu"] ** 2).T @ oT_sg
    return yT_mu.astype(np.float32), yT_sg.astype(np.float32)


def kernel(mu, sigma, ln_gamma, ln_beta, Wqkv_mu, Wqkv_sigma_raw, Wout_mu,
           Wout_sigma_raw, _trace=False):
    from concourse.bass_utils import run_bass_kernel_spmd

    nc = _get_nc()
    args = (mu, sigma, ln_gamma, ln_beta, Wqkv_mu, Wqkv_sigma_raw, Wout_mu,
            Wout_sigma_raw)
    in_maps = [_prep_core_inputs(c, *args) for c in range(8)]
    res = run_bass_kernel_spmd(nc, in_maps, list(range(8)), trace=_trace)
    out_mu = np.zeros((B, N, D), np.float32)
    out_sg = np.zeros((B, N, D), np.float32)
    for c in range(8):
        b = c // 2
        out_mu[b] += res.results[c]["yT_mu"].T
        out_sg[b] += res.results[c]["yT_sg"].T
    if _trace:
        kernel._last_result = res
    return out_mu, out_sg
